# revision 1
# baseline (speedup 1.0000x reference)
"""Causal self-attention (B=2, T=2048, D=2048, H=16, hd=128, RoPE on masked
heads) as a Bass/Tile kernel on 8 Trainium2 NeuronCores.

Sharding: core c handles batch b=c//4 and heads 4*(c%4)..4*(c%4)+3 (data
parallel on B x tensor parallel on H).  Each core computes a partial output
projection y_b = O_local @ Wout_local^T; the host sums the 4 partials per
batch.

All heavy matmuls run as float32r (full-rate fp32 path on the PE array).
Layout strategy: host pre-transposes x and the weight slices so every device
matmul sees natural [contraction-on-partitions] operands; attention is
computed in transposed score space (S^T = K Q^T) so softmax normalization
becomes a per-free-element multiply and P^T feeds the PV matmul directly.
Scores are O(1) for this problem so softmax runs without max-subtraction;
the denominator comes from an all-ones matmul over P^T (replicated across
partitions so the normalizing multiply needs no broadcast step).
"""

import sys

sys.path.insert(0, "/opt/trn_rl_repo")

import numpy as np

import concourse.bass as bass
import concourse.mybir as mybir
import concourse.tile as tile
from concourse.bass_utils import run_bass_kernel_spmd

F32 = mybir.dt.float32
F32R = mybir.dt.float32r

B = 2
T = 2048
D = 2048
H = 16
HD = 128
N_CORES = 8
HEADS_PER_CORE = 4
CORES_PER_B = 4
P = 128
TB = 512          # t-block width for projections / attention q-tiles
KO = D // P       # 16 contraction subtiles for D-contraction
NTB = T // TB     # 4
NQK = 2 * HEADS_PER_CORE  # 8 q+k dout tiles of 128
SCALE = 1.0 / float(np.sqrt(HD))


# ---------------------------------------------------------------------------
# Walrus on this toolchain rejects instructions carrying more than one sync
# wait command; Tile can emit several (e.g. the kernel-tail drain).  Hoist
# the excess onto injected same-engine NoOps — semantically identical.
def _fix_waits(nc, cap=1):
    ctr = 0
    for f in nc.m.functions:
        for bb in f.blocks:
            insts = bb.instructions
            i = 0
            while i < len(insts):
                inst = insts[i]
                si = inst.sync_info
                if si is not None and si.on_wait and len(si.on_wait) > cap:
                    waits = list(si.on_wait)
                    keep, excess = waits[:cap], waits[cap:]
                    nops = []
                    for j in range(0, len(excess), cap):
                        ctr += 1
                        nops.append(
                            mybir.InstNoOp(
                                name=f"I-waitfix-{ctr}",
                                engine=inst.engine,
                                sync_info=mybir.SyncInfo(
                                    on_wait=excess[j : j + cap], on_update=[]
                                ),
                            )
                        )
                    inst.sync_info = mybir.SyncInfo(
                        on_wait=keep, on_update=list(si.on_update or [])
                    )
                    insts[i:i] = nops
                    i += len(nops)
                i += 1
    return ctr


def _phase1(nc, tc, xT, wqkT, wvT, qkT_scr, v_scr, qT0, kT0):
    with (
        tc.tile_pool(name="p1w", bufs=1) as p1w,
        tc.tile_pool(name="p1x", bufs=2) as p1x,
        tc.tile_pool(name="p1s", bufs=3) as p1s,
        tc.tile_pool(name="p1p", bufs=8, space="PSUM") as p1p,
    ):
        # Per-ko tiles + interleaved emission so the first accumulation
        # group starts as soon as its (weight, x) slice pair lands instead
        # of stalling on the whole 16 MB load.
        wqk_r = wqkT.rearrange("(ko p) d -> p ko d", p=P)
        wv_r = wvT.rearrange("(ko p) d -> p ko d", p=P)
        xT_r = xT.rearrange("(ko p) t -> p ko t", p=P)

        wqk_sb = []
        wv_sb = []
        xt0 = []
        for ko in range(KO):
            w = p1w.tile([P, NQK * P], F32R, tag=f"wqk{ko}", name=f"wqk{ko}")
            nc.sync.dma_start(w[:], wqk_r[:, ko])
            wqk_sb.append(w)
            x = p1x.tile([P, TB], F32R, tag=f"xt{ko}", name=f"xt0_{ko}")
            nc.sync.dma_start(x[:], xT_r[:, ko, 0:TB])
            xt0.append(x)
        for ko in range(KO):
            w = p1w.tile([P, HEADS_PER_CORE * HD], F32R, tag=f"wv{ko}", name=f"wv{ko}")
            nc.sync.dma_start(w[:], wv_r[:, ko])
            wv_sb.append(w)

        for tb in range(NTB):
            if tb == 0:
                xt = xt0
            else:
                xt = []
                for ko in range(KO):
                    x = p1x.tile([P, TB], F32R, tag=f"xt{ko}", name=f"xt{tb}_{ko}")
                    nc.sync.dma_start(x[:], xT_r[:, ko, tb * TB : (tb + 1) * TB])
                    xt.append(x)
            tsl = slice(tb * TB, (tb + 1) * TB)
            # Sweep 1: all 8 q,k dout groups, ko-outer / dout-inner — each
            # arriving (wqk, x) slice pair immediately feeds 8 matmuls so
            # the initial DMA fill overlaps compute. Sweep 2: the 4 v
            # groups (wv loads arrive during sweep 1).
            ps_qk = {d: p1p.tile([P, TB], F32, tag="ps1", name=f"ps_qk{tb}_{d}") for d in range(NQK)}
            for ko in range(KO):
                st, sp = (ko == 0), (ko == KO - 1)
                for d in range(NQK):
                    nc.tensor.matmul(
                        ps_qk[d][:],
                        wqk_sb[ko][:, d * P : (d + 1) * P],
                        xt[ko][:],
                        start=st,
                        stop=sp,
                    )
            for d in range(NQK):
                # head 0's q (d=0) and k (d=4) stay in SBUF — no DRAM
                # round-trip for the first attention head.  Copies alternate
                # DVE/ACT so the end-of-phase copy backlog halves.
                cp = (nc.vector.tensor_copy if (tb < NTB - 1 or d % 4 != 1)
                      else nc.scalar.copy)
                if d == 0:
                    cp(qT0[:, tsl], ps_qk[d][:])
                elif d == HEADS_PER_CORE:
                    cp(kT0[:, tsl], ps_qk[d][:])
                else:
                    sb = p1s.tile([P, TB], F32R, tag="sb1", name=f"sbq{tb}_{d}")
                    cp(sb[:], ps_qk[d][:])
                    nc.sync.dma_start(qkT_scr[d * P : (d + 1) * P, tsl], sb[:])
            ps_v = {t4: p1p.tile([P, HEADS_PER_CORE * HD], F32, tag="ps1", name=f"ps_v{tb}_{t4}") for t4 in range(4)}
            for ko in range(KO):
                st, sp = (ko == 0), (ko == KO - 1)
                for t4 in range(4):
                    nc.tensor.matmul(
                        ps_v[t4][:],
                        xt[ko][:, t4 * P : (t4 + 1) * P],
                        wv_sb[ko][:],
                        start=st,
                        stop=sp,
                    )
            for t4 in range(4):
                sb = p1s.tile([P, HEADS_PER_CORE * HD], F32R, tag="sb1", name=f"sbv{tb}_{t4}")
                (nc.vector.tensor_copy if (tb < NTB - 1 or t4 % 2 == 0)
                 else nc.scalar.copy)(sb[:], ps_v[t4][:])
                nc.sync.dma_start(
                    v_scr[tb * TB + t4 * P : tb * TB + (t4 + 1) * P, :], sb[:]
                )


def _phase2(nc, tc, outT, qkT_scr, v_scr, jT_sb, mask_sb, ones_sb, cs, qT0, kT0):
    with (
        tc.tile_pool(name="p2qk", bufs=2) as p2qk,
        tc.tile_pool(name="p2r", bufs=2) as p2r,
        tc.tile_pool(name="p2v", bufs=2) as p2v,
        tc.tile_pool(name="p2cs", bufs=2) as p2cs,
        tc.tile_pool(name="p2pt", bufs=8) as p2pt,
        tc.tile_pool(name="p2rec", bufs=4) as p2rec,
        tc.tile_pool(name="p2ps", bufs=4, space="PSUM") as p2ps,
        tc.tile_pool(name="p2po", bufs=2, space="PSUM") as p2po,
        tc.tile_pool(name="p2pd", bufs=2, space="PSUM") as p2pd,
    ):
        def st_tile(name):
            return p2ps.tile([P, TB], F32, tag="st", name=name)

        def load_head(h):
            if h == 0:
                qT_h, kT_h = qT0, kT0
            else:
                qT_h = p2qk.tile([P, T], F32R, tag="qT", name=f"qT{h}")
                kT_h = p2qk.tile([P, T], F32R, tag="kT", name=f"kT{h}")
                nc.sync.dma_start(qT_h[:], qkT_scr[h * P : (h + 1) * P, :])
                nc.sync.dma_start(
                    kT_h[:],
                    qkT_scr[(HEADS_PER_CORE + h) * P : (HEADS_PER_CORE + h + 1) * P, :],
                )
            cs_h = p2cs.tile([P, 2, T], F32, tag="cs", name=f"cs{h}")
            nc.sync.dma_start(cs_h[:], cs[h].rearrange("c p t -> p c t"))
            v_h = p2v.tile([P, T // P, HD], F32R, tag="vh", name=f"vh{h}")
            v_r = v_scr[:, h * HD : (h + 1) * HD].rearrange("(ko p) hd -> p ko hd", p=P)
            nc.sync.dma_start(v_h[:], v_r)
            return qT_h, kT_h, cs_h, v_h

        def rope_block(h, qr, kr, qT_h, kT_h, cs_h, tb):
            # RoPE for one 512-wide t-block: roped = C*q + S*(J q)
            sl = slice(tb * TB, (tb + 1) * TB)
            for src_t, dst in ((qT_h, qr), (kT_h, kr)):
                psj = st_tile(f"psj{h}{tb}")
                nc.tensor.matmul(psj[:], jT_sb[:], src_t[:, sl], start=True, stop=True)
                tmp = p2pt.tile([P, TB], F32, tag="ropetmp", name=f"tmp{h}{tb}")
                nc.vector.tensor_tensor(
                    tmp[:], psj[:], cs_h[:, 1, sl], mybir.AluOpType.mult
                )
                nc.vector.tensor_tensor(
                    dst[:, sl], src_t[:, sl], cs_h[:, 0, sl], mybir.AluOpType.mult
                )
                nc.vector.tensor_tensor(
                    dst[:, sl], dst[:, sl], tmp[:], mybir.AluOpType.add
                )

        def alloc_roped(h):
            qr = p2r.tile([P, T], F32R, tag="qr", name=f"qr{h}")
            kr = p2r.tile([P, T], F32R, tag="kr", name=f"kr{h}")
            return qr, kr

        def attn_tq(h, tq, qr, kr, v_h, pending):
            """Emit one q-tile of attention, software-pipelined: each ST is
            issued one block ahead of its PV/ones pair (carried in `pending`,
            a 1-deep list of (issue_pv_fn, pt))."""
            sl = slice(tq * TB, (tq + 1) * TB)
            nk = (tq + 1) * (TB // P)  # causal: only tk blocks up to diagonal
            ps_o = p2po.tile([P, TB], F32, tag="po", name=f"po{h}{tq}")
            ps_d = p2pd.tile([P, TB], F32, tag="pd", name=f"pd{h}{tq}")

            def issue_st(kb):
                ps_st = st_tile(f"st{h}{tq}{kb}")
                nc.tensor.matmul(
                    ps_st[:],
                    kr[:, kb * P : (kb + 1) * P],
                    qr[:, sl],
                    start=True,
                    stop=True,
                )
                pt = p2pt.tile([P, TB], F32R, tag="pt", name=f"pt{h}{tq}{kb}")
                nc.scalar.activation(
                    pt[:], ps_st[:], mybir.ActivationFunctionType.Exp, scale=SCALE
                )
                band = kb - tq * (TB // P)
                if band >= 0:
                    nc.vector.tensor_tensor(
                        pt[:], pt[:], mask_sb[:, band, :], mybir.AluOpType.mult
                    )
                return pt

            def make_pv(kb, pt):
                def pv():
                    nc.tensor.matmul(
                        ps_o[:], v_h[:, kb], pt[:], start=(kb == 0), stop=(kb == nk - 1)
                    )
                    nc.tensor.matmul(
                        ps_d[:], ones_sb[:], pt[:], start=(kb == 0), stop=(kb == nk - 1)
                    )
                    if kb == nk - 1:
                        rec = p2rec.tile([P, TB], F32, tag="rec", name=f"rec{h}{tq}")
                        nc.vector.reciprocal(rec[:], ps_d[:])
                        nc.vector.tensor_tensor(
                            outT[(h, tq)][:], ps_o[:], rec[:], mybir.AluOpType.mult
                        )
                return pv

            for kb in range(nk):
                pt = issue_st(kb)
                if len(pending) >= 4:
                    pending.pop(0)()
                pending.append(make_pv(kb, pt))

        # Loads run one head ahead; rope for head h+1 is interleaved into
        # head h's attention (one t-block per q-tile) so the DVE never has a
        # burst of blend work blocking the mask ops of the running head.
        loads = [load_head(0)]
        r0 = alloc_roped(0)
        for tb in range(NTB):
            rope_block(0, r0[0], r0[1], loads[0][0], loads[0][1], loads[0][2], tb)
        roped = [r0]
        pending = []
        for h in range(HEADS_PER_CORE):
            if h + 1 < HEADS_PER_CORE:
                loads.append(load_head(h + 1))
                roped.append(alloc_roped(h + 1))
            qr, kr = roped[h]
            for tq in range(NTB):
                attn_tq(h, tq, qr, kr, loads[h][3], pending)
                if h + 1 < HEADS_PER_CORE:
                    nh = loads[h + 1]
                    rope_block(h + 1, roped[h + 1][0], roped[h + 1][1],
                               nh[0], nh[1], nh[2], tq)
            if h == HEADS_PER_CORE - 1:
                while pending:
                    pending.pop(0)()


def _phase3(nc, tc, outT, woT, y):
    with (
        tc.tile_pool(name="p3w", bufs=1) as p3w,
        tc.tile_pool(name="p3s", bufs=6) as p3s,
        tc.tile_pool(name="p3p", bufs=6, space="PSUM") as p3p,
    ):

        wo_sb = p3w.tile([P, HEADS_PER_CORE, D], F32R)
        nc.sync.dma_start(wo_sb[:], woT.rearrange("(h p) d -> p h d", p=P))
        for tq in range(NTB):
            for tt in range(tq * (TB // P), (tq + 1) * (TB // P)):
                off = (tt - tq * (TB // P)) * P
                for dd in range(D // TB):
                    ps = p3p.tile([P, TB], F32, tag="ps3", name=f"ps3{tt}{dd}")
                    for h in range(HEADS_PER_CORE):
                        nc.tensor.matmul(
                            ps[:],
                            outT[(h, tq)][:, off : off + P],
                            wo_sb[:, h, dd * TB : (dd + 1) * TB],
                            start=(h == 0),
                            stop=(h == HEADS_PER_CORE - 1),
                        )
                    sb = p3s.tile([P, TB], F32, tag="sb3", name=f"sb3{tt}{dd}")
                    (nc.vector.tensor_copy if dd % 2 == 0 else nc.scalar.copy)(sb[:], ps[:])
                    nc.sync.dma_start(
                        y[tt * P : (tt + 1) * P, dd * TB : (dd + 1) * TB], sb[:]
                    )


def _build_program():
    nc = bass.Bass()

    xT = nc.dram_tensor("xT", (D, T), F32R, kind="ExternalInput")
    wqkT = nc.dram_tensor("wqkT", (D, NQK * P), F32R, kind="ExternalInput")
    wvT = nc.dram_tensor("wvT", (D, HEADS_PER_CORE * HD), F32R, kind="ExternalInput")
    woT = nc.dram_tensor("woT", (HEADS_PER_CORE * HD, D), F32R, kind="ExternalInput")
    jT = nc.dram_tensor("jT", (P, P), F32R, kind="ExternalInput")
    ones = nc.dram_tensor("ones", (P, P), F32R, kind="ExternalInput")
    cs = nc.dram_tensor("cs", (HEADS_PER_CORE, 2, P, T), F32, kind="ExternalInput")
    masks = nc.dram_tensor("masks", (TB // P, P, TB), mybir.dt.bfloat16, kind="ExternalInput")
    y = nc.dram_tensor("y", (T, D), F32, kind="ExternalOutput")

    with tile.TileContext(nc) as tc:
        with (
            tc.tile_pool(name="dram", bufs=1, space="DRAM") as dram,
            tc.tile_pool(name="consts", bufs=1) as consts,
        ):
            qkT_scr = dram.tile([NQK * P, T], F32R)  # q rows then k rows
            v_scr = dram.tile([T, HEADS_PER_CORE * HD], F32R)

            jT_sb = consts.tile([P, P], F32R)
            nc.sync.dma_start(jT_sb[:], jT[:])
            mask_sb = consts.tile([P, TB // P, TB], mybir.dt.bfloat16)
            nc.sync.dma_start(mask_sb[:], masks.rearrange("a p j -> p a j"))
            ones_sb = consts.tile([P, P], F32R)
            nc.sync.dma_start(ones_sb[:], ones[:])

            qT0 = consts.tile([P, T], F32R)
            kT0 = consts.tile([P, T], F32R)
            _phase1(nc, tc, xT, wqkT, wvT, qkT_scr, v_scr, qT0, kT0)

            with tc.tile_pool(name="outT", bufs=1) as outT_pool:
                outT = {
                    (h, tq): outT_pool.tile(
                        [P, TB], F32R, tag=f"outT{h}_{tq}", name=f"outT{h}_{tq}"
                    )
                    for h in range(HEADS_PER_CORE)
                    for tq in range(NTB)
                }
                _phase2(nc, tc, outT, qkT_scr, v_scr, jT_sb, mask_sb, ones_sb, cs, qT0, kT0)
                _phase3(nc, tc, outT, woT, y)

    _fix_waits(nc)
    return nc


_NC_CACHE = None


def _get_program():
    global _NC_CACHE
    if _NC_CACHE is None:
        _NC_CACHE = _build_program()
    return _NC_CACHE


def _host_inputs(x, Wqkv, Wout, cos, sin, rope_mask):
    """Build the 8 per-core input maps."""
    x = np.asarray(x, dtype=np.float32)
    Wqkv = np.asarray(Wqkv, dtype=np.float32)
    Wout = np.asarray(Wout, dtype=np.float32)
    cos = np.asarray(cos, dtype=np.float32)
    sin = np.asarray(sin, dtype=np.float32)
    rope_mask = np.asarray(rope_mask).astype(bool)

    # J^T for the pair-rotation matmul: (J q)[2i] = -q[2i+1], (J q)[2i+1] = q[2i]
    jT = np.zeros((P, P), dtype=np.float32)
    for i in range(P // 2):
        jT[2 * i, 2 * i + 1] = 1.0
        jT[2 * i + 1, 2 * i] = -1.0

    # causal 0/1 masks for the diagonal band blocks: valid iff i + a*128 <= j
    import ml_dtypes
    masks = np.zeros((TB // P, P, TB), dtype=ml_dtypes.bfloat16)
    ii = np.arange(P)[:, None]
    jj = np.arange(TB)[None, :]
    for a in range(TB // P):
        masks[a] = (ii + a * P <= jj).astype(ml_dtypes.bfloat16)

    C_full = np.repeat(cos[:T].T, 2, axis=0).astype(np.float32)  # [128, T]
    S_full = np.repeat(sin[:T].T, 2, axis=0).astype(np.float32)
    C_id = np.ones_like(C_full)
    S_id = np.zeros_like(S_full)

    in_maps = []
    for c in range(N_CORES):
        b = c // CORES_PER_B
        hg = c % CORES_PER_B
        heads = [hg * HEADS_PER_CORE + i for i in range(HEADS_PER_CORE)]

        qrows = np.concatenate([np.arange(h * HD, (h + 1) * HD) for h in heads])
        krows = qrows + D
        vrows = qrows + 2 * D
        wqkT_l = np.ascontiguousarray(Wqkv[np.concatenate([qrows, krows])].T)
        wvT_l = np.ascontiguousarray(Wqkv[vrows].T)
        woT_l = np.ascontiguousarray(Wout[:, qrows].T)

        cs_arr = np.empty((HEADS_PER_CORE, 2, P, T), dtype=np.float32)
        for i, h in enumerate(heads):
            cs_arr[i, 0] = C_full if rope_mask[h] else C_id
            cs_arr[i, 1] = S_full if rope_mask[h] else S_id

        in_maps.append(
            {
                "xT": np.ascontiguousarray(x[b].T),
                "wqkT": wqkT_l,
                "wvT": wvT_l,
                "woT": woT_l,
                "jT": jT,
                "ones": np.ones((P, P), dtype=np.float32),
                "cs": cs_arr,
                "masks": masks,
            }
        )
    return in_maps


def kernel(x, Wqkv, Wout, cos, sin, rope_mask, _trace=False):
    nc = _get_program()
    in_maps = _host_inputs(x, Wqkv, Wout, cos, sin, rope_mask)
    res = run_bass_kernel_spmd(nc, in_maps, core_ids=list(range(N_CORES)), trace=_trace)
    parts = [res.results[c]["y"] for c in range(N_CORES)]
    out = np.stack(
        [sum(parts[b * CORES_PER_B : (b + 1) * CORES_PER_B]) for b in range(B)]
    ).astype(np.float32)
    if _trace:
        kernel.last_result = res
    return out



# revision 4
# speedup vs baseline: 1.1542x; 1.1542x over previous
"""Causal self-attention (B=2, T=2048, D=2048, H=16, hd=128, RoPE on masked
heads) as a Bass/Tile kernel on 8 Trainium2 NeuronCores.

Sharding: core c handles batch b=c//4 and heads 4*(c%4)..4*(c%4)+3 (data
parallel on B x tensor parallel on H).  Each core computes a partial output
projection y_b = O_local @ Wout_local^T; the host sums the 4 partials per
batch.

Numerics/performance strategy:
- QKV projection runs as fp8(e4m3) DoubleRow matmuls with 3-term residual
  compensation: x*W ~ x8*W8 + xr8*W8 + x8*Wr8, where xr8/Wr8 are e4m3
  quantizations of the quantization residuals (host-prepared).  Each
  DoubleRow instruction contracts two 128-row K-slabs at half cost, so the
  projection runs at 1.5x the bf16 matmul rate with ~1e-3 relative error.
  The 3 terms are packed into 24 DoubleRow instructions per output tile via
  a chain pairing that needs no operand duplication (see _emit_3term).
- Attention (scores, softmax, PV, denominator) runs in bf16: S^T = K Q^T in
  transposed score space so softmax normalization is a per-free-element
  multiply; denominator via an all-ones stationary matmul.
- q, k, v stay resident in SBUF between phases (bf16) - no DRAM scratch.
- RoPE tables are a single per-core C/S pair (identity for NoPE cores);
  roped = C*q + S*(J q) with J applied as a PE matmul.
- Output projection in bf16 with Wout pre-scaled by the fp8 descale factor.
"""

import sys

sys.path.insert(0, "/opt/trn_rl_repo")

import numpy as np

import concourse.bass as bass
import concourse.mybir as mybir
import concourse.tile as tile
from concourse.bass_utils import run_bass_kernel_spmd

F32 = mybir.dt.float32
F8 = mybir.dt.float8e4
BF16 = mybir.dt.bfloat16
DR = mybir.MatmulPerfMode.DoubleRow

B = 2
T = 2048
D = 2048
H = 16
HD = 128
N_CORES = 8
HPC = 4           # heads per core
CORES_PER_B = 4
P = 128
TB = 512          # t-block width
NTB = T // TB     # 4
KO = D // P       # 16 contraction K-blocks of 128
NQK = 2 * HPC     # 8 q+k dout blocks of 128
SX = 16.0         # fp8 scale for x
SW = 1024.0       # fp8 scale for Wqkv
SIGMA = SX * SW   # scale carried by q,k,v in SBUF
SCALE_EFF = (1.0 / float(np.sqrt(HD))) / (SIGMA * SIGMA)


# ---------------------------------------------------------------------------
# Walrus on this toolchain rejects instructions carrying more than one sync
# wait command; Tile can emit several (e.g. the kernel-tail drain).  Hoist
# the excess onto injected same-engine NoOps — semantically identical.
def _fix_waits(nc, cap=1):
    ctr = 0
    for f in nc.m.functions:
        for bb in f.blocks:
            insts = bb.instructions
            i = 0
            while i < len(insts):
                inst = insts[i]
                si = inst.sync_info
                if si is not None and si.on_wait and len(si.on_wait) > cap:
                    waits = list(si.on_wait)
                    keep, excess = waits[:cap], waits[cap:]
                    nops = []
                    for j in range(0, len(excess), cap):
                        ctr += 1
                        nops.append(
                            mybir.InstNoOp(
                                name=f"I-waitfix-{ctr}",
                                engine=inst.engine,
                                sync_info=mybir.SyncInfo(
                                    on_wait=excess[j : j + cap], on_update=[]
                                ),
                            )
                        )
                    inst.sync_info = mybir.SyncInfo(
                        on_wait=keep, on_update=list(si.on_update or [])
                    )
                    insts[i:i] = nops
                    i += len(nops)
                i += 1
    return ctr


def _emit_3term(nc, ps, w_sb, wr_sb, xs_t, msl, tsl, w_of_pair, x_of_pair):
    """Emit the 24 DoubleRow matmuls of one 3-term-compensated K=2048
    contraction into PSUM tile `ps`.

    xs_t holds 32 K-slabs (2i = x8_i, 2i+1 = xr8_i); w_sb/wr_sb hold 16
    slabs each (W8_i / Wr8_i).  Chain pairing covers x8_i*W8_i, xr8_i*W8_i
    (A instructions) and x8_i*Wr8_i (B instructions) with constant-stride
    slab pairs only.  `w_of_pair(w_tile, s0, s1, msl)` / `x_of_pair(xs, s0,
    s1, tsl)` build the [128, 2, *] APs (orientation differs between the
    q/k and v sweeps).
    """
    seq = []
    # A_1..A_15: x slabs (2j-1, 2j), w slabs (j-1, j)
    for j in range(1, KO):
        seq.append((w_of_pair(w_sb, j - 1, j, msl), x_of_pair(xs_t, 2 * j - 1, 2 * j, tsl)))
    # B_0..B_7: x slabs (4m, 4m+2), wr slabs (2m, 2m+1)
    for m in range(KO // 2):
        seq.append((w_of_pair(wr_sb, 2 * m, 2 * m + 1, msl), x_of_pair(xs_t, 4 * m, 4 * m + 2, tsl)))
    # A_0: x slabs (0, 31), w slabs (0, 15)
    seq.append((w_of_pair(w_sb, 0, KO - 1, msl), x_of_pair(xs_t, 0, 2 * KO - 1, tsl)))
    n = len(seq)
    for i, (w_ap, x_ap) in enumerate(seq):
        nc.tensor.matmul(ps[:], w_ap, x_ap, start=(i == 0), stop=(i == n - 1), perf_mode=DR)


def _slab_pair(t, s0, s1, csl):
    """AP [128, 2, cols] selecting slabs s0 < s1 of a [P, nslab, C] tile."""
    if csl is None:
        return t[:, s0 : s1 + 1 : (s1 - s0), :] if s1 - s0 > 1 else t[:, s0 : s1 + 1, :]
    step = s1 - s0
    if step > 1:
        return t[:, s0 : s1 + 1 : step, csl]
    return t[:, s0 : s1 + 1, csl]


def _phase1(nc, tc, xs, wqks, wqkrs, wvs, wvrs, qk_sb, v_sb):
    with (
        tc.tile_pool(name="p1w", bufs=1) as p1w,
        tc.tile_pool(name="p1x", bufs=2) as p1x,
        tc.tile_pool(name="p1p", bufs=4, space="PSUM") as p1p,
    ):
        wqk_t = p1w.tile([P, KO, NQK * P], F8, name="wqks")
        wqkr_t = p1w.tile([P, KO, NQK * P], F8, name="wqkrs")
        wv_t = p1w.tile([P, KO, HPC * HD], F8, name="wvs")
        wvr_t = p1w.tile([P, KO, HPC * HD], F8, name="wvrs")

        # q/k sweep: stationary = weight slab pair, moving = x slab pair
        def w_qk(t, s0, s1, msl):
            return _slab_pair(t, s0, s1, msl)

        def x_qk(t, s0, s1, _):
            return _slab_pair(t, s0, s1, None)

        first = True
        for tb in range(NTB):
            tsl = slice(tb * TB, (tb + 1) * TB)
            xs_t = p1x.tile([P, 2 * KO, TB], F8, tag="xs", name=f"xs{tb}")
            nc.sync.dma_start(xs_t[:, 0:KO, :], xs[:, 0:KO, tsl])
            if first:
                # interleave weight loads behind the first x chunk so the A
                # chain can start as soon as W8 columns land
                nc.sync.dma_start(wqk_t[:, :, 0 : NQK * P // 2], wqks[:, :, 0 : NQK * P // 2])
                nc.sync.dma_start(xs_t[:, KO : 2 * KO, :], xs[:, KO : 2 * KO, tsl])
                nc.sync.dma_start(wqkr_t[:, :, 0 : NQK * P // 2], wqkrs[:, :, 0 : NQK * P // 2])
                nc.sync.dma_start(wqk_t[:, :, NQK * P // 2 :], wqks[:, :, NQK * P // 2 :])
                nc.sync.dma_start(wqkr_t[:, :, NQK * P // 2 :], wqkrs[:, :, NQK * P // 2 :])
                nc.sync.dma_start(wv_t[:], wvs[:])
                nc.sync.dma_start(wvr_t[:], wvrs[:])
                first = False
            else:
                nc.sync.dma_start(xs_t[:, KO : 2 * KO, :], xs[:, KO : 2 * KO, tsl])

            for m in range(NQK):
                msl = slice(m * P, (m + 1) * P)
                ps = p1p.tile([P, TB], F32, tag="ps1", name=f"psqk{tb}_{m}")
                _emit_3term(nc, ps, wqk_t, wqkr_t, xs_t, msl, None, w_qk, x_qk)
                cp = (nc.vector.tensor_copy, nc.scalar.copy)[m % 2]
                cp(qk_sb[m][:, tsl], ps[:])
            for t4 in range(4):
                t4sl = slice(t4 * P, (t4 + 1) * P)
                ps = p1p.tile([P, HPC * HD], F32, tag="ps1", name=f"psv{tb}_{t4}")
                # v: out[t, hd] — stationary x slabs sliced to t4, moving wv
                seq = []
                for j in range(1, KO):
                    seq.append((_slab_pair(xs_t, 2 * j - 1, 2 * j, t4sl), _slab_pair(wv_t, j - 1, j, None)))
                for m2 in range(KO // 2):
                    seq.append((_slab_pair(xs_t, 4 * m2, 4 * m2 + 2, t4sl), _slab_pair(wvr_t, 2 * m2, 2 * m2 + 1, None)))
                seq.append((_slab_pair(xs_t, 0, 2 * KO - 1, t4sl), _slab_pair(wv_t, 0, KO - 1, None)))
                for i, (x_ap, w_ap) in enumerate(seq):
                    nc.tensor.matmul(ps[:], x_ap, w_ap, start=(i == 0), stop=(i == len(seq) - 1), perf_mode=DR)
                cp = (nc.vector.tensor_copy, nc.scalar.copy)[t4 % 2]
                cp(v_sb[tb * 4 + t4][:], ps[:])


def _phase2(nc, tc, outT, qk_sb, v_sb, jT_sb, mask_sb, ones_sb, cs_sb):
    with (
        tc.tile_pool(name="p2r", bufs=2) as p2r,
        tc.tile_pool(name="p2pt", bufs=8) as p2pt,
        tc.tile_pool(name="p2rec", bufs=4) as p2rec,
        tc.tile_pool(name="p2ps", bufs=4, space="PSUM") as p2ps,
        tc.tile_pool(name="p2po", bufs=2, space="PSUM") as p2po,
        tc.tile_pool(name="p2pd", bufs=2, space="PSUM") as p2pd,
    ):
        def st_tile(name):
            return p2ps.tile([P, TB], F32, tag="st", name=name)

        def rope_block(h, qr, kr, tb):
            # RoPE for one 512-wide t-block: roped = C*q + S*(J q)
            sl = slice(tb * TB, (tb + 1) * TB)
            for src_t, dst in ((qk_sb[h], qr), (qk_sb[HPC + h], kr)):
                psj = st_tile(f"psj{h}{tb}")
                nc.tensor.matmul(psj[:], jT_sb[:], src_t[:, sl], start=True, stop=True)
                tmp = p2pt.tile([P, TB], BF16, tag="ropetmp", name=f"tmp{h}{tb}")
                nc.vector.tensor_tensor(tmp[:], psj[:], cs_sb[:, 1, sl], mybir.AluOpType.mult)
                nc.vector.tensor_tensor(dst[:, sl], src_t[:, sl], cs_sb[:, 0, sl], mybir.AluOpType.mult)
                nc.vector.tensor_tensor(dst[:, sl], dst[:, sl], tmp[:], mybir.AluOpType.add)

        def alloc_roped(h):
            qr = p2r.tile([P, T], BF16, tag="qr", name=f"qr{h}")
            kr = p2r.tile([P, T], BF16, tag="kr", name=f"kr{h}")
            return qr, kr

        def attn_tq(h, tq, qr, kr, pending):
            """One q-tile of attention, software-pipelined: each ST is issued
            one block ahead of its PV/ones pair (carried in `pending`)."""
            sl = slice(tq * TB, (tq + 1) * TB)
            nk = (tq + 1) * (TB // P)
            ps_o = p2po.tile([P, TB], F32, tag="po", name=f"po{h}{tq}")
            ps_d = p2pd.tile([P, TB], F32, tag="pd", name=f"pd{h}{tq}")

            def issue_st(kb):
                ps_st = st_tile(f"st{h}{tq}{kb}")
                nc.tensor.matmul(ps_st[:], kr[:, kb * P : (kb + 1) * P], qr[:, sl], start=True, stop=True)
                pt = p2pt.tile([P, TB], BF16, tag="pt", name=f"pt{h}{tq}{kb}")
                nc.scalar.activation(pt[:], ps_st[:], mybir.ActivationFunctionType.Exp, scale=SCALE_EFF)
                band = kb - tq * (TB // P)
                if band >= 0:
                    nc.vector.tensor_tensor(pt[:], pt[:], mask_sb[:, band, :], mybir.AluOpType.mult)
                return pt

            def make_pv(kb, pt):
                def pv():
                    nc.tensor.matmul(
                        ps_o[:], v_sb[kb][:, h * HD : (h + 1) * HD], pt[:],
                        start=(kb == 0), stop=(kb == nk - 1),
                    )
                    nc.tensor.matmul(
                        ps_d[:], ones_sb[:], pt[:], start=(kb == 0), stop=(kb == nk - 1)
                    )
                    if kb == nk - 1:
                        rec = p2rec.tile([P, TB], F32, tag="rec", name=f"rec{h}{tq}")
                        nc.vector.reciprocal(rec[:], ps_d[:])
                        nc.vector.tensor_tensor(
                            outT[(h, tq)][:], ps_o[:], rec[:], mybir.AluOpType.mult
                        )
                return pv

            for kb in range(nk):
                pt = issue_st(kb)
                if len(pending) >= 4:
                    pending.pop(0)()
                pending.append(make_pv(kb, pt))

        # rope for head h+1 is interleaved into head h's attention (one
        # t-block per q-tile) so the DVE never has a burst of blend work.
        r0 = alloc_roped(0)
        for tb in range(NTB):
            rope_block(0, r0[0], r0[1], tb)
        roped = [r0]
        pending = []
        for h in range(HPC):
            if h + 1 < HPC:
                roped.append(alloc_roped(h + 1))
            qr, kr = roped[h]
            for tq in range(NTB):
                attn_tq(h, tq, qr, kr, pending)
                if h + 1 < HPC:
                    rope_block(h + 1, roped[h + 1][0], roped[h + 1][1], tq)
            if h == HPC - 1:
                while pending:
                    pending.pop(0)()


def _phase3(nc, tc, outT, wo_sb, y):
    with (
        tc.tile_pool(name="p3s", bufs=3) as p3s,
        tc.tile_pool(name="p3p", bufs=6, space="PSUM") as p3p,
    ):
        for tq in range(NTB):
            for tt in range(tq * (TB // P), (tq + 1) * (TB // P)):
                off = (tt - tq * (TB // P)) * P
                ysb = p3s.tile([P, D], F32, tag="ysb", name=f"ysb{tt}")
                for dd in range(D // TB):
                    ps = p3p.tile([P, TB], F32, tag="ps3", name=f"ps3{tt}{dd}")
                    for h in range(HPC):
                        nc.tensor.matmul(
                            ps[:],
                            outT[(h, tq)][:, off : off + P],
                            wo_sb[:, h, dd * TB : (dd + 1) * TB],
                            start=(h == 0),
                            stop=(h == HPC - 1),
                        )
                    cp = (nc.vector.tensor_copy, nc.scalar.copy)[dd % 2]
                    cp(ysb[:, dd * TB : (dd + 1) * TB], ps[:])
                nc.sync.dma_start(y[tt * P : (tt + 1) * P, :], ysb[:])


def _build_program():
    nc = bass.Bass()

    xs = nc.dram_tensor("xs", (P, 2 * KO, T), F8, kind="ExternalInput")
    wqks = nc.dram_tensor("wqks", (P, KO, NQK * P), F8, kind="ExternalInput")
    wqkrs = nc.dram_tensor("wqkrs", (P, KO, NQK * P), F8, kind="ExternalInput")
    wvs = nc.dram_tensor("wvs", (P, KO, HPC * HD), F8, kind="ExternalInput")
    wvrs = nc.dram_tensor("wvrs", (P, KO, HPC * HD), F8, kind="ExternalInput")
    wo = nc.dram_tensor("wo", (P, HPC, D), BF16, kind="ExternalInput")
    cs = nc.dram_tensor("cs", (P, 2, T), BF16, kind="ExternalInput")
    masks = nc.dram_tensor("masks", (TB // P, P, TB), BF16, kind="ExternalInput")
    jT = nc.dram_tensor("jT", (P, P), BF16, kind="ExternalInput")
    ones = nc.dram_tensor("ones", (P, P), BF16, kind="ExternalInput")
    y = nc.dram_tensor("y", (T, D), F32, kind="ExternalOutput")

    with tile.TileContext(nc) as tc:
        with (
            tc.tile_pool(name="consts", bufs=1) as consts,
            tc.tile_pool(name="qkv", bufs=1) as qkvp,
        ):
            jT_sb = consts.tile([P, P], BF16)
            nc.sync.dma_start(jT_sb[:], jT[:])
            mask_sb = consts.tile([P, TB // P, TB], BF16)
            nc.sync.dma_start(mask_sb[:], masks.rearrange("a p j -> p a j"))
            ones_sb = consts.tile([P, P], BF16)
            nc.sync.dma_start(ones_sb[:], ones[:])
            cs_sb = consts.tile([P, 2, T], BF16)
            nc.sync.dma_start(cs_sb[:], cs[:])
            wo_sb = consts.tile([P, HPC, D], BF16)
            nc.sync.dma_start(wo_sb[:], wo[:])

            qk_sb = [qkvp.tile([P, T], BF16, name=f"qk{m}") for m in range(NQK)]
            v_sb = [qkvp.tile([P, HPC * HD], BF16, name=f"v{kb}") for kb in range(T // P)]

            _phase1(nc, tc, xs, wqks, wqkrs, wvs, wvrs, qk_sb, v_sb)

            with tc.tile_pool(name="outT", bufs=1) as outT_pool:
                outT = {
                    (h, tq): outT_pool.tile([P, TB], BF16, tag=f"outT{h}_{tq}", name=f"outT{h}_{tq}")
                    for h in range(HPC)
                    for tq in range(NTB)
                }
                _phase2(nc, tc, outT, qk_sb, v_sb, jT_sb, mask_sb, ones_sb, cs_sb)
                _phase3(nc, tc, outT, wo_sb, y)

    _fix_waits(nc)
    return nc


_NC_CACHE = None


def _get_program():
    global _NC_CACHE
    if _NC_CACHE is None:
        _NC_CACHE = _build_program()
    return _NC_CACHE


def _q8(a, s):
    """e4m3-quantize a*s (clipped to TRN e4m3 range); returns (fp8, residual
    fp8) with the residual on the same scale (no prescale — its values live
    in e4m3's normal range already)."""
    import ml_dtypes

    F8np = ml_dtypes.float8_e4m3
    scaled = np.clip(a * s, -240.0, 240.0)
    hi = scaled.astype(F8np)
    lo = np.clip(scaled - hi.astype(np.float32), -240.0, 240.0).astype(F8np)
    return hi, lo


def _pack_k(a):
    """[K, M] -> [P, KO', M] with slab i on partitions (rows 128i+p)."""
    ko = a.shape[0] // P
    return np.ascontiguousarray(a.reshape(ko, P, a.shape[1]).transpose(1, 0, 2))


def _host_inputs(x, Wqkv, Wout, cos, sin, rope_mask):
    import ml_dtypes

    BF = ml_dtypes.bfloat16
    x = np.asarray(x, dtype=np.float32)
    Wqkv = np.asarray(Wqkv, dtype=np.float32)
    Wout = np.asarray(Wout, dtype=np.float32)
    cos = np.asarray(cos, dtype=np.float32)
    sin = np.asarray(sin, dtype=np.float32)
    rope_mask = np.asarray(rope_mask).astype(bool)

    # J^T for the pair-rotation matmul: (J q)[2i] = -q[2i+1], (J q)[2i+1] = q[2i]
    jT = np.zeros((P, P), dtype=np.float32)
    for i in range(P // 2):
        jT[2 * i, 2 * i + 1] = 1.0
        jT[2 * i + 1, 2 * i] = -1.0

    masks = np.zeros((TB // P, P, TB), dtype=BF)
    ii = np.arange(P)[:, None]
    jj = np.arange(TB)[None, :]
    for a in range(TB // P):
        masks[a] = (ii + a * P <= jj).astype(BF)

    C_full = np.repeat(cos[:T].T, 2, axis=0).astype(np.float32)  # [128, T]
    S_full = np.repeat(sin[:T].T, 2, axis=0).astype(np.float32)

    # per-batch x packs (shared by the 4 cores of each batch)
    xs_b = []
    for b in range(B):
        x8, xr8 = _q8(x[b].T, SX)  # [D, T] fp8
        xsp = np.empty((P, 2 * KO, T), dtype=x8.dtype)
        xsp[:, 0::2] = _pack_k(x8)
        xsp[:, 1::2] = _pack_k(xr8)
        xs_b.append(xsp)

    in_maps = []
    for c in range(N_CORES):
        b = c // CORES_PER_B
        hg = c % CORES_PER_B
        heads = [hg * HPC + i for i in range(HPC)]

        qrows = np.concatenate([np.arange(h * HD, (h + 1) * HD) for h in heads])
        krows = qrows + D
        vrows = qrows + 2 * D
        wqk = Wqkv[np.concatenate([qrows, krows])].T  # [D, 1024]
        wv = Wqkv[vrows].T                            # [D, 512]
        wqk8, wqkr8 = _q8(wqk, SW)
        wv8, wvr8 = _q8(wv, SW)

        woT = np.ascontiguousarray(Wout[:, qrows].T) / SIGMA  # [512, D]
        wo_p = np.ascontiguousarray(
            woT.reshape(HPC, P, D).transpose(1, 0, 2)
        ).astype(BF)

        flags = [bool(rope_mask[h]) for h in heads]
        assert all(f == flags[0] for f in flags), (
            "heads in one core must share a rope flag for the single-table path"
        )
        cs_arr = np.empty((P, 2, T), dtype=BF)
        if flags[0]:
            cs_arr[:, 0] = C_full.astype(BF)
            cs_arr[:, 1] = S_full.astype(BF)
        else:
            cs_arr[:, 0] = np.ones((P, T), dtype=BF)
            cs_arr[:, 1] = np.zeros((P, T), dtype=BF)

        in_maps.append(
            {
                "xs": xs_b[b],
                "wqks": _pack_k(wqk8),
                "wqkrs": _pack_k(wqkr8),
                "wvs": _pack_k(wv8),
                "wvrs": _pack_k(wvr8),
                "wo": wo_p,
                "cs": cs_arr,
                "masks": masks,
                "jT": jT.astype(BF),
                "ones": np.ones((P, P), dtype=BF),
            }
        )
    return in_maps


def kernel(x, Wqkv, Wout, cos, sin, rope_mask, _trace=False):
    nc = _get_program()
    in_maps = _host_inputs(x, Wqkv, Wout, cos, sin, rope_mask)
    res = run_bass_kernel_spmd(nc, in_maps, core_ids=list(range(N_CORES)), trace=_trace)
    parts = [res.results[c]["y"] for c in range(N_CORES)]
    out = np.stack(
        [sum(parts[b * CORES_PER_B : (b + 1) * CORES_PER_B]) for b in range(B)]
    ).astype(np.float32)
    if _trace:
        kernel.last_result = res
    return out


# revision 12
# speedup vs baseline: 1.2053x; 1.0442x over previous
"""Causal self-attention (B=2, T=2048, D=2048, H=16, hd=128, RoPE on masked
heads) as a Bass/Tile kernel on 8 Trainium2 NeuronCores.

Sharding: core c handles batch b=c//4 and heads 4*(c%4)..4*(c%4)+3 (data
parallel on B x tensor parallel on H).  Each core computes a partial output
projection y_b = O_local @ Wout_local^T; the host sums the 4 partials per
batch.

Numerics/performance strategy:
- QKV projection runs as fp8(e4m3) DoubleRow matmuls with 3-term residual
  compensation: x*W ~ x8*W8 + xr8*W8 + x8*Wr8, where xr8/Wr8 are e4m3
  quantizations of the quantization residuals (host-prepared).  Each
  DoubleRow instruction contracts two 128-row K-slabs at half cost, so the
  projection runs at 1.5x the bf16 matmul rate with ~1e-3 relative error.
  The 3 terms are packed into 24 DoubleRow instructions per output tile via
  a chain pairing that needs no operand duplication (see _emit_3term).
- Attention (scores, softmax, PV, denominator) runs in bf16: S^T = K Q^T in
  transposed score space so softmax normalization is a per-free-element
  multiply; denominator via an all-ones stationary matmul.
- q, k, v stay resident in SBUF between phases (bf16) - no DRAM scratch.
- RoPE tables are a single per-core C/S pair (identity for NoPE cores);
  roped = C*q + S*(J q) with J applied as a PE matmul.
- Output projection in bf16 with Wout pre-scaled by the fp8 descale factor.
"""

import sys

sys.path.insert(0, "/opt/trn_rl_repo")

import numpy as np

import concourse.bass as bass
import concourse.mybir as mybir
import concourse.tile as tile
from concourse.bass_utils import run_bass_kernel_spmd

F32 = mybir.dt.float32
F8 = mybir.dt.float8e4
BF16 = mybir.dt.bfloat16
DR = mybir.MatmulPerfMode.DoubleRow

B = 2
T = 2048
D = 2048
H = 16
HD = 128
N_CORES = 8
HPC = 4           # heads per core
CORES_PER_B = 4
P = 128
TB = 512          # t-block width
NTB = T // TB     # 4
KO = D // P       # 16 contraction K-blocks of 128
NQK = 2 * HPC     # 8 q+k dout blocks of 128
SX = 16.0         # fp8 scale for x
SW = 1024.0       # fp8 scale for Wqkv
SIGMA = SX * SW   # scale carried by q,k,v in SBUF
SCALE_EFF = (1.0 / float(np.sqrt(HD))) / (SIGMA * SIGMA)


# ---------------------------------------------------------------------------
# Walrus on this toolchain rejects instructions carrying more than one sync
# wait command; Tile can emit several (e.g. the kernel-tail drain).  Hoist
# the excess onto injected same-engine NoOps — semantically identical.
def _fix_waits(nc, cap=1):
    ctr = 0
    for f in nc.m.functions:
        for bb in f.blocks:
            insts = bb.instructions
            i = 0
            while i < len(insts):
                inst = insts[i]
                si = inst.sync_info
                if si is not None and si.on_wait and len(si.on_wait) > cap:
                    waits = list(si.on_wait)
                    keep, excess = waits[:cap], waits[cap:]
                    nops = []
                    for j in range(0, len(excess), cap):
                        ctr += 1
                        nops.append(
                            mybir.InstNoOp(
                                name=f"I-waitfix-{ctr}",
                                engine=inst.engine,
                                sync_info=mybir.SyncInfo(
                                    on_wait=excess[j : j + cap], on_update=[]
                                ),
                            )
                        )
                    inst.sync_info = mybir.SyncInfo(
                        on_wait=keep, on_update=list(si.on_update or [])
                    )
                    insts[i:i] = nops
                    i += len(nops)
                i += 1
    return ctr


def _emit_3term(nc, ps, w_sb, wr_sb, xs_t, msl, tsl, w_of_pair, x_of_pair):
    """Emit the 24 DoubleRow matmuls of one 3-term-compensated K=2048
    contraction into PSUM tile `ps`.

    xs_t holds 32 K-slabs (2i = x8_i, 2i+1 = xr8_i); w_sb/wr_sb hold 16
    slabs each (W8_i / Wr8_i).  Chain pairing covers x8_i*W8_i, xr8_i*W8_i
    (A instructions) and x8_i*Wr8_i (B instructions) with constant-stride
    slab pairs only.  `w_of_pair(w_tile, s0, s1, msl)` / `x_of_pair(xs, s0,
    s1, tsl)` build the [128, 2, *] APs (orientation differs between the
    q/k and v sweeps).
    """
    seq = []
    # A_1..A_15: x slabs (2j-1, 2j), w slabs (j-1, j)
    for j in range(1, KO):
        seq.append((w_of_pair(w_sb, j - 1, j, msl), x_of_pair(xs_t, 2 * j - 1, 2 * j, tsl)))
    # B_0..B_7: x slabs (4m, 4m+2), wr slabs (2m, 2m+1)
    for m in range(KO // 2):
        seq.append((w_of_pair(wr_sb, 2 * m, 2 * m + 1, msl), x_of_pair(xs_t, 4 * m, 4 * m + 2, tsl)))
    # A_0: x slabs (0, 31), w slabs (0, 15)
    seq.append((w_of_pair(w_sb, 0, KO - 1, msl), x_of_pair(xs_t, 0, 2 * KO - 1, tsl)))
    n = len(seq)
    for i, (w_ap, x_ap) in enumerate(seq):
        nc.tensor.matmul(ps[:], w_ap, x_ap, start=(i == 0), stop=(i == n - 1), perf_mode=DR)


def _slab_pair(t, s0, s1, csl):
    """AP [128, 2, cols] selecting slabs s0 < s1 of a [P, nslab, C] tile."""
    if csl is None:
        return t[:, s0 : s1 + 1 : (s1 - s0), :] if s1 - s0 > 1 else t[:, s0 : s1 + 1, :]
    step = s1 - s0
    if step > 1:
        return t[:, s0 : s1 + 1 : step, csl]
    return t[:, s0 : s1 + 1, csl]


def _rope_block(nc, psum_pool, tmp_pool, qk_sb, cs_sb, jT_sb, h, qr, kr, rb, tag="psj"):
    """RoPE for one 512-wide t-block of head h: roped = C*q + S*(J q)."""
    sl = slice(rb * TB, (rb + 1) * TB)
    for si, (src_t, dst) in enumerate(((qk_sb[h], qr), (qk_sb[HPC + h], kr))):
        psj = psum_pool.tile([P, TB], F32, tag=tag, name=f"psj{h}_{rb}_{si}")
        nc.tensor.matmul(psj[:], jT_sb[:], src_t[:, sl], start=True, stop=True)
        tmp = tmp_pool.tile([P, TB], BF16, tag="ropetmp", name=f"rtmp{h}_{rb}_{si}")
        nc.vector.tensor_tensor(tmp[:], psj[:], cs_sb[:, 1, sl], mybir.AluOpType.mult)
        nc.vector.tensor_tensor(dst[:, sl], src_t[:, sl], cs_sb[:, 0, sl], mybir.AluOpType.mult)
        nc.vector.tensor_tensor(dst[:, sl], dst[:, sl], tmp[:], mybir.AluOpType.add)


def _phase1(nc, tc, xs, wqks, wqkrs, wvs, wvrs, qk_sb, v_sb, rope0, const_dmas):
    with (
        tc.tile_pool(name="p1w", bufs=1) as p1w,
        tc.tile_pool(name="p1x", bufs=2) as p1x,
        tc.tile_pool(name="p1t", bufs=2) as p1t,
        tc.tile_pool(name="p1p", bufs=4, space="PSUM") as p1p,
        tc.tile_pool(name="p1pj", bufs=2, space="PSUM") as p1pj,
    ):
        wqk_t = p1w.tile([P, KO, NQK * P], F8, name="wqks")
        wqkr_t = p1w.tile([P, KO, NQK * P], F8, name="wqkrs")
        wv_t = p1w.tile([P, KO, HPC * HD], F8, name="wvs")
        wvr_t = p1w.tile([P, KO, HPC * HD], F8, name="wvrs")

        # q/k sweep: stationary = weight slab pair, moving = x slab pair
        def w_qk(t, s0, s1, msl):
            return _slab_pair(t, s0, s1, msl)

        def x_qk(t, s0, s1, _):
            return _slab_pair(t, s0, s1, None)

        first = True
        for tb in range(NTB):
            tsl = slice(tb * TB, (tb + 1) * TB)
            xs_t = p1x.tile([P, 2 * KO, TB], F8, tag="xs", name=f"xs{tb}")
            nc.sync.dma_start(xs_t[:, 0:KO, :], xs[:, 0:KO, tsl])
            if first:
                # interleave weight loads behind the first x chunk so the A
                # chain can start as soon as W8 columns land
                nc.sync.dma_start(wqk_t[:, :, 0 : NQK * P // 2], wqks[:, :, 0 : NQK * P // 2])
                nc.sync.dma_start(xs_t[:, KO : 2 * KO, :], xs[:, KO : 2 * KO, tsl])
                nc.sync.dma_start(wqkr_t[:, :, 0 : NQK * P // 2], wqkrs[:, :, 0 : NQK * P // 2])
                nc.sync.dma_start(wqk_t[:, :, NQK * P // 2 :], wqks[:, :, NQK * P // 2 :])
                nc.sync.dma_start(wqkr_t[:, :, NQK * P // 2 :], wqkrs[:, :, NQK * P // 2 :])
                nc.sync.dma_start(wv_t[:], wvs[:])
                nc.sync.dma_start(wvr_t[:], wvrs[:])
                # const loads ride behind the critical phase-1 loads
                for dma in const_dmas:
                    dma()
                first = False
            else:
                nc.sync.dma_start(xs_t[:, KO : 2 * KO, :], xs[:, KO : 2 * KO, tsl])

            for m in range(NQK):
                msl = slice(m * P, (m + 1) * P)
                ps = p1p.tile([P, TB], F32, tag="ps1", name=f"psqk{tb}_{m}")
                _emit_3term(nc, ps, wqk_t, wqkr_t, xs_t, msl, None, w_qk, x_qk)
                cp = (nc.vector.tensor_copy, nc.scalar.copy)[m % 2]
                cp(qk_sb[m][:, tsl], ps[:])
            for t4 in range(4):
                t4sl = slice(t4 * P, (t4 + 1) * P)
                ps = p1p.tile([P, HPC * HD], F32, tag="ps1", name=f"psv{tb}_{t4}")
                # v: out[t, hd] — stationary x slabs sliced to t4, moving wv
                seq = []
                for j in range(1, KO):
                    seq.append((_slab_pair(xs_t, 2 * j - 1, 2 * j, t4sl), _slab_pair(wv_t, j - 1, j, None)))
                for m2 in range(KO // 2):
                    seq.append((_slab_pair(xs_t, 4 * m2, 4 * m2 + 2, t4sl), _slab_pair(wvr_t, 2 * m2, 2 * m2 + 1, None)))
                seq.append((_slab_pair(xs_t, 0, 2 * KO - 1, t4sl), _slab_pair(wv_t, 0, KO - 1, None)))
                for i, (x_ap, w_ap) in enumerate(seq):
                    nc.tensor.matmul(ps[:], x_ap, w_ap, start=(i == 0), stop=(i == len(seq) - 1), perf_mode=DR)
                cp = (nc.vector.tensor_copy, nc.scalar.copy)[t4 % 2]
                cp(v_sb[tb * 4 + t4][:], ps[:])
            # head-0 rope for this t-block rides inside phase 1 so the DVE
            # blend queue is warm when attention starts
            qr0, kr0, cs_sb, jT_sb = rope0
            _rope_block(nc, p1pj, p1t, qk_sb, cs_sb, jT_sb, 0, qr0, kr0, tb, tag="psj1")


def _phase2(nc, tc, outT, qk_sb, v_sb, jT_sb, mask_sb, ones_sb, cs_sb, r0, emit_p3):
    with (
        tc.tile_pool(name="p2r", bufs=2) as p2r,
        tc.tile_pool(name="p2pt", bufs=8) as p2pt,
        tc.tile_pool(name="p2rec", bufs=4) as p2rec,
        tc.tile_pool(name="p2ps", bufs=4, space="PSUM") as p2ps,
        tc.tile_pool(name="p2po", bufs=1, space="PSUM") as p2po,
        tc.tile_pool(name="p2pd", bufs=1, space="PSUM") as p2pd,
    ):
        def st_tile(name):
            return p2ps.tile([P, TB], F32, tag="st", name=name)

        def alloc_roped(h):
            qr = p2r.tile([P, T], BF16, tag="qr", name=f"qr{h}")
            kr = p2r.tile([P, T], BF16, tag="kr", name=f"kr{h}")
            return qr, kr

        def attn_tq(h, tq, qr, kr, pending):
            """One q-tile of attention, software-pipelined: each ST is issued
            one block ahead of its PV/ones pair (carried in `pending`)."""
            sl = slice(tq * TB, (tq + 1) * TB)
            nk = (tq + 1) * (TB // P)
            ps_o = p2po.tile([P, TB], F32, tag="po", name=f"po{h}{tq}")
            ps_d = p2pd.tile([P, TB], F32, tag="pd", name=f"pd{h}{tq}")

            def issue_st(kb):
                ps_st = st_tile(f"st{h}{tq}{kb}")
                nc.tensor.matmul(ps_st[:], kr[:, kb * P : (kb + 1) * P], qr[:, sl], start=True, stop=True)
                pt = p2pt.tile([P, TB], BF16, tag="pt", name=f"pt{h}{tq}{kb}")
                nc.scalar.activation(pt[:], ps_st[:], mybir.ActivationFunctionType.Exp, scale=SCALE_EFF)
                band = kb - tq * (TB // P)
                if band >= 0:
                    nc.vector.tensor_tensor(pt[:], pt[:], mask_sb[:, band, :], mybir.AluOpType.mult)
                return pt

            def make_pv(kb, pt):
                def pv():
                    nc.tensor.matmul(
                        ps_o[:], v_sb[kb][:, h * HD : (h + 1) * HD], pt[:],
                        start=(kb == 0), stop=(kb == nk - 1),
                    )
                    nc.tensor.matmul(
                        ps_d[:], ones_sb[:], pt[:], start=(kb == 0), stop=(kb == nk - 1)
                    )
                    if kb == nk - 1:
                        rec = p2rec.tile([P, TB], F32, tag="rec", name=f"rec{h}{tq}")
                        nc.vector.reciprocal(rec[:], ps_d[:])
                        nc.vector.tensor_tensor(
                            outT[(h, tq)][:], ps_o[:], rec[:], mybir.AluOpType.mult
                        )
                return pv

            for kb in range(nk):
                pt = issue_st(kb)
                if len(pending) >= 4:
                    pending.pop(0)()
                pending.append(make_pv(kb, pt))

        # rope for head h+1 is interleaved into head h's attention (one
        # t-block per q-tile); head 0 was roped inside phase 1.  During the
        # last head, phase-3 tiles are emitted one q-tile behind so output
        # projection overlaps the attention tail.
        roped = [r0]
        pending = []
        for h in range(HPC):
            if h + 1 < HPC:
                roped.append(alloc_roped(h + 1))
            qr, kr = roped[h]
            for tq in range(NTB):
                attn_tq(h, tq, qr, kr, pending)
                if h + 1 < HPC:
                    _rope_block(nc, p2ps, p2pt, qk_sb, cs_sb, jT_sb,
                                h + 1, roped[h + 1][0], roped[h + 1][1], tq, tag="st")
                elif tq >= 1:
                    emit_p3(tq - 1)
            if h == HPC - 1:
                while pending:
                    pending.pop(0)()
        emit_p3(NTB - 1)


def _make_p3(nc, p3s, p3p, outT, wo_sb, y):
    def emit_p3(tq):
        for tt in range(tq * (TB // P), (tq + 1) * (TB // P)):
            off = (tt - tq * (TB // P)) * P
            ysb = p3s.tile([P, D], F32, tag="ysb", name=f"ysb{tt}")
            last = tq == NTB - 1
            for dd in range(D // TB):
                ps = p3p.tile([P, TB], F32, tag="ps3", name=f"ps3{tt}{dd}")
                for h in range(HPC):
                    nc.tensor.matmul(
                        ps[:],
                        outT[(h, tq)][:, off : off + P],
                        wo_sb[:, h, dd * TB : (dd + 1) * TB],
                        start=(h == 0),
                        stop=(h == HPC - 1),
                    )
                cp = (nc.vector.tensor_copy, nc.scalar.copy)[dd % 2]
                cp(ysb[:, dd * TB : (dd + 1) * TB], ps[:])
                if last:
                    # small per-dd stores shrink the end-of-kernel DMA tail
                    nc.sync.dma_start(
                        y[tt * P : (tt + 1) * P, dd * TB : (dd + 1) * TB],
                        ysb[:, dd * TB : (dd + 1) * TB],
                    )
            if not last:
                nc.sync.dma_start(y[tt * P : (tt + 1) * P, :], ysb[:])
    return emit_p3


def _build_program():
    nc = bass.Bass()

    xs = nc.dram_tensor("xs", (P, 2 * KO, T), F8, kind="ExternalInput")
    wqks = nc.dram_tensor("wqks", (P, KO, NQK * P), F8, kind="ExternalInput")
    wqkrs = nc.dram_tensor("wqkrs", (P, KO, NQK * P), F8, kind="ExternalInput")
    wvs = nc.dram_tensor("wvs", (P, KO, HPC * HD), F8, kind="ExternalInput")
    wvrs = nc.dram_tensor("wvrs", (P, KO, HPC * HD), F8, kind="ExternalInput")
    wo = nc.dram_tensor("wo", (P, HPC, D), BF16, kind="ExternalInput")
    cs = nc.dram_tensor("cs", (P, 2, T), BF16, kind="ExternalInput")
    masks = nc.dram_tensor("masks", (TB // P, P, TB), BF16, kind="ExternalInput")
    jT = nc.dram_tensor("jT", (P, P), BF16, kind="ExternalInput")
    ones = nc.dram_tensor("ones", (P, P), BF16, kind="ExternalInput")
    y = nc.dram_tensor("y", (T, D), F32, kind="ExternalOutput")

    with tile.TileContext(nc) as tc:
        with (
            tc.tile_pool(name="consts", bufs=1) as consts,
            tc.tile_pool(name="qkv", bufs=1) as qkvp,
        ):
            jT_sb = consts.tile([P, P], BF16)
            mask_sb = consts.tile([P, TB // P, TB], BF16)
            ones_sb = consts.tile([P, P], BF16)
            cs_sb = consts.tile([P, 2, T], BF16)
            wo_sb = consts.tile([P, HPC, D], BF16)
            const_dmas = [
                lambda: nc.sync.dma_start(cs_sb[:], cs[:]),
                lambda: nc.sync.dma_start(jT_sb[:], jT[:]),
                lambda: nc.sync.dma_start(ones_sb[:], ones[:]),
                lambda: nc.sync.dma_start(mask_sb[:], masks.rearrange("a p j -> p a j")),
                lambda: nc.sync.dma_start(wo_sb[:], wo[:]),
            ]

            qk_sb = [qkvp.tile([P, T], BF16, name=f"qk{m}") for m in range(NQK)]
            v_sb = [qkvp.tile([P, HPC * HD], BF16, name=f"v{kb}") for kb in range(T // P)]
            qr0 = qkvp.tile([P, T], BF16, name="qr0")
            kr0 = qkvp.tile([P, T], BF16, name="kr0")

            _phase1(nc, tc, xs, wqks, wqkrs, wvs, wvrs, qk_sb, v_sb,
                    (qr0, kr0, cs_sb, jT_sb), const_dmas)

            with (
                tc.tile_pool(name="outT", bufs=1) as outT_pool,
                tc.tile_pool(name="p3s", bufs=3) as p3s,
                tc.tile_pool(name="p3p", bufs=2, space="PSUM") as p3p,
            ):
                outT = {
                    (h, tq): outT_pool.tile([P, TB], BF16, tag=f"outT{h}_{tq}", name=f"outT{h}_{tq}")
                    for h in range(HPC)
                    for tq in range(NTB)
                }
                emit_p3 = _make_p3(nc, p3s, p3p, outT, wo_sb, y)
                _phase2(nc, tc, outT, qk_sb, v_sb, jT_sb, mask_sb, ones_sb, cs_sb,
                        (qr0, kr0), emit_p3)

    _fix_waits(nc)
    return nc


_NC_CACHE = None


def _get_program():
    global _NC_CACHE
    if _NC_CACHE is None:
        _NC_CACHE = _build_program()
    return _NC_CACHE


def _q8(a, s):
    """e4m3-quantize a*s (clipped to TRN e4m3 range); returns (fp8, residual
    fp8) with the residual on the same scale (no prescale — its values live
    in e4m3's normal range already)."""
    import ml_dtypes

    F8np = ml_dtypes.float8_e4m3
    scaled = np.clip(a * s, -240.0, 240.0)
    hi = scaled.astype(F8np)
    lo = np.clip(scaled - hi.astype(np.float32), -240.0, 240.0).astype(F8np)
    return hi, lo


def _pack_k(a):
    """[K, M] -> [P, KO', M] with slab i on partitions (rows 128i+p)."""
    ko = a.shape[0] // P
    return np.ascontiguousarray(a.reshape(ko, P, a.shape[1]).transpose(1, 0, 2))


def _host_inputs(x, Wqkv, Wout, cos, sin, rope_mask):
    import ml_dtypes

    BF = ml_dtypes.bfloat16
    x = np.asarray(x, dtype=np.float32)
    Wqkv = np.asarray(Wqkv, dtype=np.float32)
    Wout = np.asarray(Wout, dtype=np.float32)
    cos = np.asarray(cos, dtype=np.float32)
    sin = np.asarray(sin, dtype=np.float32)
    rope_mask = np.asarray(rope_mask).astype(bool)

    # J^T for the pair-rotation matmul: (J q)[2i] = -q[2i+1], (J q)[2i+1] = q[2i]
    jT = np.zeros((P, P), dtype=np.float32)
    for i in range(P // 2):
        jT[2 * i, 2 * i + 1] = 1.0
        jT[2 * i + 1, 2 * i] = -1.0

    masks = np.zeros((TB // P, P, TB), dtype=BF)
    ii = np.arange(P)[:, None]
    jj = np.arange(TB)[None, :]
    for a in range(TB // P):
        masks[a] = (ii + a * P <= jj).astype(BF)

    C_full = np.repeat(cos[:T].T, 2, axis=0).astype(np.float32)  # [128, T]
    S_full = np.repeat(sin[:T].T, 2, axis=0).astype(np.float32)

    # per-batch x packs (shared by the 4 cores of each batch)
    xs_b = []
    for b in range(B):
        x8, xr8 = _q8(x[b].T, SX)  # [D, T] fp8
        xsp = np.empty((P, 2 * KO, T), dtype=x8.dtype)
        xsp[:, 0::2] = _pack_k(x8)
        xsp[:, 1::2] = _pack_k(xr8)
        xs_b.append(xsp)

    in_maps = []
    for c in range(N_CORES):
        b = c // CORES_PER_B
        hg = c % CORES_PER_B
        heads = [hg * HPC + i for i in range(HPC)]

        qrows = np.concatenate([np.arange(h * HD, (h + 1) * HD) for h in heads])
        krows = qrows + D
        vrows = qrows + 2 * D
        wqk = Wqkv[np.concatenate([qrows, krows])].T  # [D, 1024]
        wv = Wqkv[vrows].T                            # [D, 512]
        wqk8, wqkr8 = _q8(wqk, SW)
        wv8, wvr8 = _q8(wv, SW)

        woT = np.ascontiguousarray(Wout[:, qrows].T) / SIGMA  # [512, D]
        wo_p = np.ascontiguousarray(
            woT.reshape(HPC, P, D).transpose(1, 0, 2)
        ).astype(BF)

        flags = [bool(rope_mask[h]) for h in heads]
        assert all(f == flags[0] for f in flags), (
            "heads in one core must share a rope flag for the single-table path"
        )
        cs_arr = np.empty((P, 2, T), dtype=BF)
        if flags[0]:
            cs_arr[:, 0] = C_full.astype(BF)
            cs_arr[:, 1] = S_full.astype(BF)
        else:
            cs_arr[:, 0] = np.ones((P, T), dtype=BF)
            cs_arr[:, 1] = np.zeros((P, T), dtype=BF)

        in_maps.append(
            {
                "xs": xs_b[b],
                "wqks": _pack_k(wqk8),
                "wqkrs": _pack_k(wqkr8),
                "wvs": _pack_k(wv8),
                "wvrs": _pack_k(wvr8),
                "wo": wo_p,
                "cs": cs_arr,
                "masks": masks,
                "jT": jT.astype(BF),
                "ones": np.ones((P, P), dtype=BF),
            }
        )
    return in_maps


def kernel(x, Wqkv, Wout, cos, sin, rope_mask, _trace=False):
    nc = _get_program()
    in_maps = _host_inputs(x, Wqkv, Wout, cos, sin, rope_mask)
    res = run_bass_kernel_spmd(nc, in_maps, core_ids=list(range(N_CORES)), trace=_trace)
    parts = [res.results[c]["y"] for c in range(N_CORES)]
    out = np.stack(
        [sum(parts[b * CORES_PER_B : (b + 1) * CORES_PER_B]) for b in range(B)]
    ).astype(np.float32)
    if _trace:
        kernel.last_result = res
    return out


# revision 23
# speedup vs baseline: 1.2604x; 1.0458x over previous
"""Causal self-attention (B=2, T=2048, D=2048, H=16, hd=128, RoPE on masked
heads) as a Bass/Tile kernel on 8 Trainium2 NeuronCores.

Sharding: core c handles batch b=c//4 and heads 4*(c%4)..4*(c%4)+3 (data
parallel on B x tensor parallel on H).  Each core computes a partial output
projection y_b = O_local @ Wout_local^T; the host sums the 4 partials per
batch.

Numerics/performance strategy:
- QKV projection runs as fp8(e4m3) DoubleRow matmuls with 3-term residual
  compensation: x*W ~ x8*W8 + xr8*W8 + x8*Wr8, where xr8/Wr8 are e4m3
  quantizations of the quantization residuals (host-prepared).  Each
  DoubleRow instruction contracts two 128-row K-slabs at half cost, so the
  projection runs at 1.5x the bf16 matmul rate with ~1e-3 relative error.
  The 3 terms are packed into 24 DoubleRow instructions per output tile via
  a chain pairing that needs no operand duplication (see _emit_3term).
- Attention (scores, softmax, PV, denominator) runs in bf16: S^T = K Q^T in
  transposed score space so softmax normalization is a per-free-element
  multiply; denominator via an all-ones stationary matmul.
- q, k, v stay resident in SBUF between phases (bf16) - no DRAM scratch.
- RoPE tables are a single per-core C/S pair (identity for NoPE cores);
  roped = C*q + S*(J q) with J applied as a PE matmul.
- Output projection in bf16 with Wout pre-scaled by the fp8 descale factor.
"""

import sys

sys.path.insert(0, "/opt/trn_rl_repo")

import numpy as np

import concourse.bass as bass
import concourse.mybir as mybir
import concourse.tile as tile
from concourse.bass_utils import run_bass_kernel_spmd

F32 = mybir.dt.float32
F8 = mybir.dt.float8e4
BF16 = mybir.dt.bfloat16
DR = mybir.MatmulPerfMode.DoubleRow

B = 2
T = 2048
D = 2048
H = 16
HD = 128
N_CORES = 8
HPC = 4           # heads per core
CORES_PER_B = 4
P = 128
TB = 512          # t-block width (phase 1 / rope)
NTB = T // TB     # 4
TQ = 256          # attention q-tile width (phase 2)
NTQ = T // TQ     # 8
BANDS = TQ // P   # 2
KO = D // P       # 16 contraction K-blocks of 128
NQK = 2 * HPC     # 8 q+k dout blocks of 128
SX = 16.0         # fp8 scale for x
SW = 1024.0       # fp8 scale for Wqkv
SWO = 1024.0      # fp8 scale for Wout
SO = 32.0         # fp8 scale carried by the normalized attention output
SIGMA = SX * SW   # scale carried by q,k,v in SBUF
SCALE_EFF = (1.0 / float(np.sqrt(HD))) / (SIGMA * SIGMA)


# ---------------------------------------------------------------------------
# Walrus on this toolchain rejects instructions carrying more than one sync
# wait command; Tile can emit several (e.g. the kernel-tail drain).  Hoist
# the excess onto injected same-engine NoOps — semantically identical.
def _fix_waits(nc, cap=1):
    ctr = 0
    for f in nc.m.functions:
        for bb in f.blocks:
            insts = bb.instructions
            i = 0
            while i < len(insts):
                inst = insts[i]
                si = inst.sync_info
                if si is not None and si.on_wait and len(si.on_wait) > cap:
                    waits = list(si.on_wait)
                    keep, excess = waits[:cap], waits[cap:]
                    nops = []
                    for j in range(0, len(excess), cap):
                        ctr += 1
                        nops.append(
                            mybir.InstNoOp(
                                name=f"I-waitfix-{ctr}",
                                engine=inst.engine,
                                sync_info=mybir.SyncInfo(
                                    on_wait=excess[j : j + cap], on_update=[]
                                ),
                            )
                        )
                    inst.sync_info = mybir.SyncInfo(
                        on_wait=keep, on_update=list(si.on_update or [])
                    )
                    insts[i:i] = nops
                    i += len(nops)
                i += 1
    return ctr


def _emit_3term(nc, ps, w_sb, wr_sb, xs_t, msl, tsl, w_of_pair, x_of_pair):
    """Emit the 24 DoubleRow matmuls of one 3-term-compensated K=2048
    contraction into PSUM tile `ps`.

    xs_t holds 32 K-slabs (2i = x8_i, 2i+1 = xr8_i); w_sb/wr_sb hold 16
    slabs each (W8_i / Wr8_i).  Chain pairing covers x8_i*W8_i, xr8_i*W8_i
    (A instructions) and x8_i*Wr8_i (B instructions) with constant-stride
    slab pairs only.  `w_of_pair(w_tile, s0, s1, msl)` / `x_of_pair(xs, s0,
    s1, tsl)` build the [128, 2, *] APs (orientation differs between the
    q/k and v sweeps).
    """
    seq = []
    # A_1..A_15: x slabs (2j-1, 2j), w slabs (j-1, j)
    for j in range(1, KO):
        seq.append((w_of_pair(w_sb, j - 1, j, msl), x_of_pair(xs_t, 2 * j - 1, 2 * j, tsl)))
    # B_0..B_7: x slabs (4m, 4m+2), wr slabs (2m, 2m+1)
    for m in range(KO // 2):
        seq.append((w_of_pair(wr_sb, 2 * m, 2 * m + 1, msl), x_of_pair(xs_t, 4 * m, 4 * m + 2, tsl)))
    # A_0: x slabs (0, 31), w slabs (0, 15)
    seq.append((w_of_pair(w_sb, 0, KO - 1, msl), x_of_pair(xs_t, 0, 2 * KO - 1, tsl)))
    n = len(seq)
    for i, (w_ap, x_ap) in enumerate(seq):
        nc.tensor.matmul(ps[:], w_ap, x_ap, start=(i == 0), stop=(i == n - 1), perf_mode=DR)


def _slab_pair(t, s0, s1, csl):
    """AP [128, 2, cols] selecting slabs s0 < s1 of a [P, nslab, C] tile."""
    if csl is None:
        return t[:, s0 : s1 + 1 : (s1 - s0), :] if s1 - s0 > 1 else t[:, s0 : s1 + 1, :]
    step = s1 - s0
    if step > 1:
        return t[:, s0 : s1 + 1 : step, csl]
    return t[:, s0 : s1 + 1, csl]


def _rope_block(nc, psum_pool, tmp_pool, qk_sb, cs_sb, jT_sb, h, qr, kr, rb, tag="psj", psj_bufs=2):
    """RoPE for one 512-wide t-block of head h: roped = C*q + S*(J q)."""
    sl = slice(rb * TB, (rb + 1) * TB)
    for si, (src_t, dst) in enumerate(((qk_sb[h], qr), (qk_sb[HPC + h], kr))):
        psj = psum_pool.tile([P, TB], F32, tag=tag, name=f"psj{h}_{rb}_{si}", bufs=psj_bufs)
        nc.tensor.matmul(psj[:], jT_sb[:], src_t[:, sl], start=True, stop=True)
        tmp = tmp_pool.tile([P, TB], BF16, tag="ropetmp", name=f"rtmp{h}_{rb}_{si}")
        nc.vector.tensor_tensor(tmp[:], psj[:], cs_sb[:, 1, sl], mybir.AluOpType.mult)
        nc.vector.tensor_tensor(dst[:, sl], src_t[:, sl], cs_sb[:, 0, sl], mybir.AluOpType.mult)
        nc.vector.tensor_tensor(dst[:, sl], dst[:, sl], tmp[:], mybir.AluOpType.add)


def _phase1(nc, tc, xs, wqks, wqkrs, wvs, wvrs, qk_sb, v_sb, rope0, const_dmas):
    with (
        tc.tile_pool(name="p1w", bufs=1) as p1w,
        tc.tile_pool(name="p1x", bufs=2) as p1x,
        tc.tile_pool(name="p1t", bufs=2) as p1t,
        tc.tile_pool(name="p1p", bufs=4, space="PSUM") as p1p,
        tc.tile_pool(name="p1pj", bufs=2, space="PSUM") as p1pj,
    ):
        wqk_t = p1w.tile([P, KO, NQK * P], F8, name="wqks")
        wqkr_t = p1w.tile([P, KO, NQK * P], F8, name="wqkrs")
        wv_t = p1w.tile([P, KO, HPC * HD], F8, name="wvs")
        wvr_t = p1w.tile([P, KO, HPC * HD], F8, name="wvrs")

        # q/k sweep: stationary = weight slab pair, moving = x slab pair
        def w_qk(t, s0, s1, msl):
            return _slab_pair(t, s0, s1, msl)

        def x_qk(t, s0, s1, _):
            return _slab_pair(t, s0, s1, None)

        first = True
        for tb in range(NTB):
            tsl = slice(tb * TB, (tb + 1) * TB)
            xs_t = p1x.tile([P, 2 * KO, TB], F8, tag="xs", name=f"xs{tb}")
            nc.sync.dma_start(xs_t[:, 0:KO, :], xs[:, 0:KO, tsl])
            if first:
                # interleave weight loads behind the first x chunk so the A
                # chain can start as soon as W8 columns land
                nc.sync.dma_start(wqk_t[:, :, 0 : NQK * P // 2], wqks[:, :, 0 : NQK * P // 2])
                nc.sync.dma_start(xs_t[:, KO : 2 * KO, :], xs[:, KO : 2 * KO, tsl])
                nc.sync.dma_start(wqkr_t[:, :, 0 : NQK * P // 2], wqkrs[:, :, 0 : NQK * P // 2])
                nc.sync.dma_start(wqk_t[:, :, NQK * P // 2 :], wqks[:, :, NQK * P // 2 :])
                nc.sync.dma_start(wqkr_t[:, :, NQK * P // 2 :], wqkrs[:, :, NQK * P // 2 :])
                nc.sync.dma_start(wv_t[:], wvs[:])
                nc.sync.dma_start(wvr_t[:], wvrs[:])
                # const loads ride behind the critical phase-1 loads
                for dma in const_dmas:
                    dma()
                first = False
            else:
                nc.sync.dma_start(xs_t[:, KO : 2 * KO, :], xs[:, KO : 2 * KO, tsl])

            for m in range(NQK):
                msl = slice(m * P, (m + 1) * P)
                ps = p1p.tile([P, TB], F32, tag="ps1", name=f"psqk{tb}_{m}")
                _emit_3term(nc, ps, wqk_t, wqkr_t, xs_t, msl, None, w_qk, x_qk)
                cp = (nc.vector.tensor_copy, nc.scalar.copy)[m % 2]
                cp(qk_sb[m][:, tsl], ps[:])
            for t4 in range(4):
                t4sl = slice(t4 * P, (t4 + 1) * P)
                ps = p1p.tile([P, HPC * HD], F32, tag="ps1", name=f"psv{tb}_{t4}")
                # v: out[t, hd] — stationary x slabs sliced to t4, moving wv
                seq = []
                for j in range(1, KO):
                    seq.append((_slab_pair(xs_t, 2 * j - 1, 2 * j, t4sl), _slab_pair(wv_t, j - 1, j, None)))
                for m2 in range(KO // 2):
                    seq.append((_slab_pair(xs_t, 4 * m2, 4 * m2 + 2, t4sl), _slab_pair(wvr_t, 2 * m2, 2 * m2 + 1, None)))
                seq.append((_slab_pair(xs_t, 0, 2 * KO - 1, t4sl), _slab_pair(wv_t, 0, KO - 1, None)))
                for i, (x_ap, w_ap) in enumerate(seq):
                    nc.tensor.matmul(ps[:], x_ap, w_ap, start=(i == 0), stop=(i == len(seq) - 1), perf_mode=DR)
                cp = (nc.vector.tensor_copy, nc.scalar.copy)[t4 % 2]
                cp(v_sb[tb * 4 + t4][:], ps[:])
            # head-0 rope for this t-block rides inside phase 1 so the DVE
            # blend queue is warm when attention starts
            qr0, kr0, cs_sb, jT_sb = rope0
            _rope_block(nc, p1pj, p1t, qk_sb, cs_sb, jT_sb, 0, qr0, kr0, tb, tag="psj1")


def _phase2(nc, tc, outS, qk_sb, v_sb, jT_sb, mask_sb, ones_sb, cs_sb, r0, emit_p3, aux_pool):
    with (
        tc.tile_pool(name="p2r", bufs=2) as p2r,
        tc.tile_pool(name="p2pt", bufs=8) as p2pt,
        tc.tile_pool(name="p2rec", bufs=4) as p2rec,
        tc.tile_pool(name="p2ps", bufs=2, space="PSUM") as p2ps,
        tc.tile_pool(name="p2po", bufs=1, space="PSUM") as p2po,
        tc.tile_pool(name="p2pd", bufs=1, space="PSUM") as p2pd,
    ):
        def alloc_roped(h):
            qr = p2r.tile([P, T], BF16, tag="qr", name=f"qr{h}")
            kr = p2r.tile([P, T], BF16, tag="kr", name=f"kr{h}")
            return qr, kr

        def attn_tq(h, tq, qr, kr, pending):
            """One q-tile of attention, software-pipelined over PAIRS of
            128-wide k-blocks: the two STs of a pair land in two PSUM banks
            of one tile so a single exp (and, on the diagonal, a single mask
            multiply) covers both.  PV/ones matmuls trail via `pending`."""
            sl = slice(tq * TQ, (tq + 1) * TQ)
            nk = (tq + 1) * BANDS
            ps_o = p2po.tile([P, TQ], F32, tag="po", name=f"po{h}{tq}")
            ps_d = p2pd.tile([P, TQ], F32, tag="pd", name=f"pd{h}{tq}")

            def issue_pair(kp):
                # quarters 0 and 2 of a 2-bank tile: each ST owns a bank
                ps_st = p2ps.tile([P, 4, TQ], F32, tag="st", name=f"st{h}{tq}{kp}")
                for j in range(2):
                    nc.tensor.matmul(
                        ps_st[:, 2 * j, :], kr[:, (2 * kp + j) * P : (2 * kp + j + 1) * P],
                        qr[:, sl], start=True, stop=True,
                    )
                pt = p2pt.tile([P, 2, TQ], BF16, tag="pt", name=f"pt{h}{tq}{kp}")
                nc.scalar.activation(
                    pt[:], ps_st[:, 0:4:2, :], mybir.ActivationFunctionType.Exp, scale=SCALE_EFF
                )
                if kp == tq:  # diagonal pair: mask both bands at once
                    nc.vector.tensor_tensor(pt[:], pt[:], mask_sb[:], mybir.AluOpType.mult)
                return pt

            def make_pv(kp, pt):
                def pv():
                    for j in range(2):
                        kb = 2 * kp + j
                        nc.tensor.matmul(
                            ps_o[:], v_sb[kb][:, h * HD : (h + 1) * HD], pt[:, j, :],
                            start=(kb == 0), stop=(kb == nk - 1),
                        )
                        nc.tensor.matmul(
                            ps_d[:], ones_sb[:], pt[:, j, :], start=(kb == 0), stop=(kb == nk - 1)
                        )
                    if 2 * kp + 1 == nk - 1:
                        rec = p2rec.tile([P, TQ], F32, tag="rec", name=f"rec{h}{tq}")
                        nc.vector.reciprocal(rec[:], ps_d[:])
                        ob = p2rec.tile([P, TQ], BF16, tag="ob", name=f"ob{h}{tq}")
                        nc.vector.tensor_tensor(ob[:], ps_o[:], rec[:], mybir.AluOpType.mult)
                        hi = outS[tq][:, 2 * h, :]
                        nc.vector.tensor_copy(hi, ob[:])
                        nc.vector.scalar_tensor_tensor(
                            outS[tq][:, 2 * h + 1, :], ob[:], 1.0, hi,
                            mybir.AluOpType.mult, mybir.AluOpType.subtract,
                        )
                return pv

            for kp in range(nk // 2):
                pt = issue_pair(kp)
                if len(pending) >= 3:
                    pending.pop(0)()
                pending.append(make_pv(kp, pt))

        # rope for head h+1 is interleaved into head h's attention (one
        # 512-wide t-block per pair of q-tiles); head 0 was roped inside
        # phase 1.  During the last head, phase-3 tiles are emitted one
        # q-tile behind so output projection overlaps the attention tail.
        roped = [r0]
        pending = []
        for h in range(HPC):
            if h + 1 < HPC:
                roped.append(alloc_roped(h + 1))
            qr, kr = roped[h]
            for tq in range(NTQ):
                attn_tq(h, tq, qr, kr, pending)
                if h + 1 < HPC:
                    if tq % 2 == 0:
                        _rope_block(nc, aux_pool, p2pt, qk_sb, cs_sb, jT_sb,
                                    h + 1, roped[h + 1][0], roped[h + 1][1], tq // 2,
                                    tag="ps3", psj_bufs=2)
                elif tq >= 2:
                    # two q-tiles behind: guarantees head-3's normalization
                    # for tq-2 has been emitted (pending is only 3 pairs deep)
                    emit_p3(tq - 2)
            if h == HPC - 1:
                while pending:
                    pending.pop(0)()
        emit_p3(NTQ - 2)
        emit_p3(NTQ - 1)


def _make_p3(nc, p3s, p3p, outS, wos_sb, wors_sb, y):
    ydescale = 1.0 / (SO * SWO)

    def emit_p3(tq):
        for tt in range(tq * BANDS, (tq + 1) * BANDS):
            off = (tt - tq * BANDS) * P
            osl = slice(off, off + P)
            ysb = p3s.tile([P, D], F32, tag="ysb", name=f"ysb{tt}")
            last = tq == NTQ - 1
            for dd in range(D // TB):
                dsl = slice(dd * TB, (dd + 1) * TB)
                ps = p3p.tile([P, TB], F32, tag="ps3", name=f"ps3{tt}{dd}")
                seq = []
                for j in range(1, HPC):
                    seq.append((outS[tq][:, 2 * j - 1 : 2 * j + 1, osl], wos_sb[:, j - 1 : j + 1, dsl]))
                for m in range(HPC // 2):
                    seq.append((outS[tq][:, 4 * m : 4 * m + 3 : 2, osl], wors_sb[:, 2 * m : 2 * m + 2, dsl]))
                seq.append((outS[tq][:, 0 : 2 * HPC : 2 * HPC - 1, osl], wos_sb[:, 0 : HPC : HPC - 1, dsl]))
                for i, (o_ap, w_ap) in enumerate(seq):
                    nc.tensor.matmul(ps[:], o_ap, w_ap, start=(i == 0), stop=(i == len(seq) - 1), perf_mode=DR)
                if dd % 2 == 0:
                    nc.vector.tensor_scalar_mul(ysb[:, dsl], ps[:], ydescale)
                else:
                    nc.scalar.mul(ysb[:, dsl], ps[:], ydescale)
                if last:
                    # small per-dd stores shrink the end-of-kernel DMA tail
                    nc.sync.dma_start(
                        y[tt * P : (tt + 1) * P, dd * TB : (dd + 1) * TB],
                        ysb[:, dd * TB : (dd + 1) * TB],
                    )
            if not last:
                nc.sync.dma_start(y[tt * P : (tt + 1) * P, :], ysb[:])
    return emit_p3


def _build_program():
    nc = bass.Bass()

    xs = nc.dram_tensor("xs", (P, 2 * KO, T), F8, kind="ExternalInput")
    wqks = nc.dram_tensor("wqks", (P, KO, NQK * P), F8, kind="ExternalInput")
    wqkrs = nc.dram_tensor("wqkrs", (P, KO, NQK * P), F8, kind="ExternalInput")
    wvs = nc.dram_tensor("wvs", (P, KO, HPC * HD), F8, kind="ExternalInput")
    wvrs = nc.dram_tensor("wvrs", (P, KO, HPC * HD), F8, kind="ExternalInput")
    wos = nc.dram_tensor("wos", (P, HPC, D), F8, kind="ExternalInput")
    wors = nc.dram_tensor("wors", (P, HPC, D), F8, kind="ExternalInput")
    cs = nc.dram_tensor("cs", (P, 2, T), BF16, kind="ExternalInput")
    masks = nc.dram_tensor("masks", (BANDS, P, TQ), BF16, kind="ExternalInput")
    jT = nc.dram_tensor("jT", (P, P), BF16, kind="ExternalInput")
    ones = nc.dram_tensor("ones", (P, P), BF16, kind="ExternalInput")
    y = nc.dram_tensor("y", (T, D), F32, kind="ExternalOutput")

    with tile.TileContext(nc) as tc:
        with (
            tc.tile_pool(name="consts", bufs=1) as consts,
            tc.tile_pool(name="qkv", bufs=1) as qkvp,
        ):
            jT_sb = consts.tile([P, P], BF16)
            mask_sb = consts.tile([P, BANDS, TQ], BF16)
            ones_sb = consts.tile([P, P], BF16)
            cs_sb = consts.tile([P, 2, T], BF16)
            wos_sb = consts.tile([P, HPC, D], F8)
            wors_sb = consts.tile([P, HPC, D], F8)
            const_dmas = [
                lambda: nc.sync.dma_start(cs_sb[:], cs[:]),
                lambda: nc.sync.dma_start(jT_sb[:], jT[:]),
                lambda: nc.sync.dma_start(ones_sb[:], ones[:]),
                lambda: nc.sync.dma_start(mask_sb[:], masks.rearrange("a p j -> p a j")),
                lambda: nc.sync.dma_start(wos_sb[:], wos[:]),
                lambda: nc.sync.dma_start(wors_sb[:], wors[:]),
            ]

            qk_sb = [qkvp.tile([P, T], BF16, name=f"qk{m}") for m in range(NQK)]
            v_sb = [qkvp.tile([P, HPC * HD], BF16, name=f"v{kb}") for kb in range(T // P)]
            qr0 = qkvp.tile([P, T], BF16, name="qr0")
            kr0 = qkvp.tile([P, T], BF16, name="kr0")

            _phase1(nc, tc, xs, wqks, wqkrs, wvs, wvrs, qk_sb, v_sb,
                    (qr0, kr0, cs_sb, jT_sb), const_dmas)

            with (
                tc.tile_pool(name="outT", bufs=1) as outT_pool,
                tc.tile_pool(name="p3s", bufs=3) as p3s,
                tc.tile_pool(name="p3p", bufs=2, space="PSUM") as p3p,
            ):
                outS = {
                    tq: outT_pool.tile([P, 2 * HPC, TQ], F8, tag=f"outS{tq}", name=f"outS{tq}")
                    for tq in range(NTQ)
                }
                emit_p3 = _make_p3(nc, p3s, p3p, outS, wos_sb, wors_sb, y)
                _phase2(nc, tc, outS, qk_sb, v_sb, jT_sb, mask_sb, ones_sb, cs_sb,
                        (qr0, kr0), emit_p3, p3p)

    _fix_waits(nc)
    return nc


_NC_CACHE = None


def _get_program():
    global _NC_CACHE
    if _NC_CACHE is None:
        _NC_CACHE = _build_program()
    return _NC_CACHE


def _q8(a, s):
    """e4m3-quantize a*s (clipped to TRN e4m3 range); returns (fp8, residual
    fp8) with the residual on the same scale (no prescale — its values live
    in e4m3's normal range already)."""
    import ml_dtypes

    F8np = ml_dtypes.float8_e4m3
    scaled = np.clip(a * s, -240.0, 240.0)
    hi = scaled.astype(F8np)
    lo = np.clip(scaled - hi.astype(np.float32), -240.0, 240.0).astype(F8np)
    return hi, lo


def _pack_k(a):
    """[K, M] -> [P, KO', M] with slab i on partitions (rows 128i+p)."""
    ko = a.shape[0] // P
    return np.ascontiguousarray(a.reshape(ko, P, a.shape[1]).transpose(1, 0, 2))


def _host_inputs(x, Wqkv, Wout, cos, sin, rope_mask):
    import ml_dtypes

    BF = ml_dtypes.bfloat16
    x = np.asarray(x, dtype=np.float32)
    Wqkv = np.asarray(Wqkv, dtype=np.float32)
    Wout = np.asarray(Wout, dtype=np.float32)
    cos = np.asarray(cos, dtype=np.float32)
    sin = np.asarray(sin, dtype=np.float32)
    rope_mask = np.asarray(rope_mask).astype(bool)

    # J^T for the pair-rotation matmul: (J q)[2i] = -q[2i+1], (J q)[2i+1] = q[2i]
    jT = np.zeros((P, P), dtype=np.float32)
    for i in range(P // 2):
        jT[2 * i, 2 * i + 1] = 1.0
        jT[2 * i + 1, 2 * i] = -1.0

    masks = np.zeros((BANDS, P, TQ), dtype=BF)
    ii = np.arange(P)[:, None]
    jj = np.arange(TQ)[None, :]
    for a in range(BANDS):
        masks[a] = (ii + a * P <= jj).astype(BF)

    C_full = np.repeat(cos[:T].T, 2, axis=0).astype(np.float32)  # [128, T]
    S_full = np.repeat(sin[:T].T, 2, axis=0).astype(np.float32)

    # per-batch x packs (shared by the 4 cores of each batch)
    xs_b = []
    for b in range(B):
        x8, xr8 = _q8(x[b].T, SX)  # [D, T] fp8
        xsp = np.empty((P, 2 * KO, T), dtype=x8.dtype)
        xsp[:, 0::2] = _pack_k(x8)
        xsp[:, 1::2] = _pack_k(xr8)
        xs_b.append(xsp)

    in_maps = []
    for c in range(N_CORES):
        b = c // CORES_PER_B
        hg = c % CORES_PER_B
        heads = [hg * HPC + i for i in range(HPC)]

        qrows = np.concatenate([np.arange(h * HD, (h + 1) * HD) for h in heads])
        krows = qrows + D
        vrows = qrows + 2 * D
        wqk = Wqkv[np.concatenate([qrows, krows])].T  # [D, 1024]
        wv = Wqkv[vrows].T                            # [D, 512]
        wqk8, wqkr8 = _q8(wqk, SW)
        wv8, wvr8 = _q8(wv, SW)

        woT = np.ascontiguousarray(Wout[:, qrows].T)  # [512, D]
        wo8, wor8 = _q8(woT, SWO)
        wos_p = np.ascontiguousarray(wo8.reshape(HPC, P, D).transpose(1, 0, 2))
        wors_p = np.ascontiguousarray(wor8.reshape(HPC, P, D).transpose(1, 0, 2))

        flags = [bool(rope_mask[h]) for h in heads]
        assert all(f == flags[0] for f in flags), (
            "heads in one core must share a rope flag for the single-table path"
        )
        cs_arr = np.empty((P, 2, T), dtype=BF)
        if flags[0]:
            cs_arr[:, 0] = C_full.astype(BF)
            cs_arr[:, 1] = S_full.astype(BF)
        else:
            cs_arr[:, 0] = np.ones((P, T), dtype=BF)
            cs_arr[:, 1] = np.zeros((P, T), dtype=BF)

        in_maps.append(
            {
                "xs": xs_b[b],
                "wqks": _pack_k(wqk8),
                "wqkrs": _pack_k(wqkr8),
                "wvs": _pack_k(wv8),
                "wvrs": _pack_k(wvr8),
                "wos": wos_p,
                "wors": wors_p,
                "cs": cs_arr,
                "masks": masks,
                "jT": jT.astype(BF),
                "ones": np.full((P, P), SIGMA / SO, dtype=BF),
            }
        )
    return in_maps


def kernel(x, Wqkv, Wout, cos, sin, rope_mask, _trace=False):
    nc = _get_program()
    in_maps = _host_inputs(x, Wqkv, Wout, cos, sin, rope_mask)
    res = run_bass_kernel_spmd(nc, in_maps, core_ids=list(range(N_CORES)), trace=_trace)
    parts = [res.results[c]["y"] for c in range(N_CORES)]
    out = np.stack(
        [sum(parts[b * CORES_PER_B : (b + 1) * CORES_PER_B]) for b in range(B)]
    ).astype(np.float32)
    if _trace:
        kernel.last_result = res
    return out


# revision 24
# speedup vs baseline: 1.2881x; 1.0219x over previous
"""Causal self-attention (B=2, T=2048, D=2048, H=16, hd=128, RoPE on masked
heads) as a Bass/Tile kernel on 8 Trainium2 NeuronCores.

Sharding: core c handles batch b=c//4 and heads 4*(c%4)..4*(c%4)+3 (data
parallel on B x tensor parallel on H).  Each core computes a partial output
projection y_b = O_local @ Wout_local^T; the host sums the 4 partials per
batch.

Numerics/performance strategy:
- QKV projection runs as fp8(e4m3) DoubleRow matmuls with 3-term residual
  compensation: x*W ~ x8*W8 + xr8*W8 + x8*Wr8, where xr8/Wr8 are e4m3
  quantizations of the quantization residuals (host-prepared).  Each
  DoubleRow instruction contracts two 128-row K-slabs at half cost, so the
  projection runs at 1.5x the bf16 matmul rate with ~1e-3 relative error.
  The 3 terms are packed into 24 DoubleRow instructions per output tile via
  a chain pairing that needs no operand duplication (see _emit_3term).
- Attention (scores, softmax, PV, denominator) runs in bf16: S^T = K Q^T in
  transposed score space so softmax normalization is a per-free-element
  multiply; denominator via an all-ones stationary matmul.
- q, k, v stay resident in SBUF between phases (bf16) - no DRAM scratch.
- RoPE tables are a single per-core C/S pair (identity for NoPE cores);
  roped = C*q + S*(J q) with J applied as a PE matmul.
- Output projection in bf16 with Wout pre-scaled by the fp8 descale factor.
"""

import sys

sys.path.insert(0, "/opt/trn_rl_repo")

import numpy as np

import concourse.bass as bass
import concourse.mybir as mybir
import concourse.tile as tile
from concourse.bass_utils import run_bass_kernel_spmd

F32 = mybir.dt.float32
F8 = mybir.dt.float8e4
BF16 = mybir.dt.bfloat16
DR = mybir.MatmulPerfMode.DoubleRow

B = 2
T = 2048
D = 2048
H = 16
HD = 128
N_CORES = 8
HPC = 4           # heads per core
CORES_PER_B = 4
P = 128
TB = 512          # t-block width (phase 1 / rope)
NTB = T // TB     # 4
TQ = 256          # attention q-tile width (phase 2)
NTQ = T // TQ     # 8
BANDS = TQ // P   # 2
KO = D // P       # 16 contraction K-blocks of 128
NQK = 2 * HPC     # 8 q+k dout blocks of 128
SX = 16.0         # fp8 scale for x
SW = 1024.0       # fp8 scale for Wqkv
SWO = 1024.0      # fp8 scale for Wout
SO = 32.0         # fp8 scale carried by the normalized attention output
SIGMA = SX * SW   # scale carried by q,k,v in SBUF
SCALE_EFF = (1.0 / float(np.sqrt(HD))) / (SIGMA * SIGMA)


# ---------------------------------------------------------------------------
# Walrus on this toolchain rejects instructions carrying more than one sync
# wait command; Tile can emit several (e.g. the kernel-tail drain).  Hoist
# the excess onto injected same-engine NoOps — semantically identical.
def _fix_waits(nc, cap=1):
    ctr = 0
    for f in nc.m.functions:
        for bb in f.blocks:
            insts = bb.instructions
            i = 0
            while i < len(insts):
                inst = insts[i]
                si = inst.sync_info
                if si is not None and si.on_wait and len(si.on_wait) > cap:
                    waits = list(si.on_wait)
                    keep, excess = waits[:cap], waits[cap:]
                    nops = []
                    for j in range(0, len(excess), cap):
                        ctr += 1
                        nops.append(
                            mybir.InstNoOp(
                                name=f"I-waitfix-{ctr}",
                                engine=inst.engine,
                                sync_info=mybir.SyncInfo(
                                    on_wait=excess[j : j + cap], on_update=[]
                                ),
                            )
                        )
                    inst.sync_info = mybir.SyncInfo(
                        on_wait=keep, on_update=list(si.on_update or [])
                    )
                    insts[i:i] = nops
                    i += len(nops)
                i += 1
    return ctr


def _emit_3term(nc, ps, w_sb, wr_sb, xs_t, msl, tsl, w_of_pair, x_of_pair):
    """Emit the 24 DoubleRow matmuls of one 3-term-compensated K=2048
    contraction into PSUM tile `ps`.

    xs_t holds 32 K-slabs (2i = x8_i, 2i+1 = xr8_i); w_sb/wr_sb hold 16
    slabs each (W8_i / Wr8_i).  Chain pairing covers x8_i*W8_i, xr8_i*W8_i
    (A instructions) and x8_i*Wr8_i (B instructions) with constant-stride
    slab pairs only.  `w_of_pair(w_tile, s0, s1, msl)` / `x_of_pair(xs, s0,
    s1, tsl)` build the [128, 2, *] APs (orientation differs between the
    q/k and v sweeps).
    """
    seq = []
    # A_1..A_15: x slabs (2j-1, 2j), w slabs (j-1, j)
    for j in range(1, KO):
        seq.append((w_of_pair(w_sb, j - 1, j, msl), x_of_pair(xs_t, 2 * j - 1, 2 * j, tsl)))
    # B_0..B_7: x slabs (4m, 4m+2), wr slabs (2m, 2m+1)
    for m in range(KO // 2):
        seq.append((w_of_pair(wr_sb, 2 * m, 2 * m + 1, msl), x_of_pair(xs_t, 4 * m, 4 * m + 2, tsl)))
    # A_0: x slabs (0, 31), w slabs (0, 15)
    seq.append((w_of_pair(w_sb, 0, KO - 1, msl), x_of_pair(xs_t, 0, 2 * KO - 1, tsl)))
    n = len(seq)
    for i, (w_ap, x_ap) in enumerate(seq):
        nc.tensor.matmul(ps[:], w_ap, x_ap, start=(i == 0), stop=(i == n - 1), perf_mode=DR)


def _slab_pair(t, s0, s1, csl):
    """AP [128, 2, cols] selecting slabs s0 < s1 of a [P, nslab, C] tile."""
    if csl is None:
        return t[:, s0 : s1 + 1 : (s1 - s0), :] if s1 - s0 > 1 else t[:, s0 : s1 + 1, :]
    step = s1 - s0
    if step > 1:
        return t[:, s0 : s1 + 1 : step, csl]
    return t[:, s0 : s1 + 1, csl]


def _rope_block(nc, psum_pool, tmp_pool, qk_sb, cs_sb, jT_sb, h, qr, kr, rb, tag="psj", psj_bufs=2):
    """RoPE for one 512-wide t-block of head h: roped = C*q + S*(J q)."""
    sl = slice(rb * TB, (rb + 1) * TB)
    for si, (src_t, dst) in enumerate(((qk_sb[h], qr), (qk_sb[HPC + h], kr))):
        psj = psum_pool.tile([P, TB], F32, tag=tag, name=f"psj{h}_{rb}_{si}", bufs=psj_bufs)
        nc.tensor.matmul(psj[:], jT_sb[:], src_t[:, sl], start=True, stop=True)
        tmp = tmp_pool.tile([P, TB], BF16, tag="ropetmp", name=f"rtmp{h}_{rb}_{si}")
        nc.vector.tensor_tensor(tmp[:], psj[:], cs_sb[:, 1, sl], mybir.AluOpType.mult)
        nc.vector.tensor_tensor(dst[:, sl], src_t[:, sl], cs_sb[:, 0, sl], mybir.AluOpType.mult)
        nc.vector.tensor_tensor(dst[:, sl], dst[:, sl], tmp[:], mybir.AluOpType.add)


def _phase1(nc, tc, xs, wqks, wqkrs, wvs, wvrs, qk_sb, v_sb, rope0, const_dmas):
    with (
        tc.tile_pool(name="p1w", bufs=1) as p1w,
        tc.tile_pool(name="p1x", bufs=2) as p1x,
        tc.tile_pool(name="p1t", bufs=2) as p1t,
        tc.tile_pool(name="p1p", bufs=4, space="PSUM") as p1p,
        tc.tile_pool(name="p1pj", bufs=2, space="PSUM") as p1pj,
    ):
        wqk_t = p1w.tile([P, KO, NQK * P], F8, name="wqks")
        wqkr_t = p1w.tile([P, KO, NQK * P], F8, name="wqkrs")
        wv_t = p1w.tile([P, KO, HPC * HD], F8, name="wvs")
        wvr_t = p1w.tile([P, KO, HPC * HD], F8, name="wvrs")

        # q/k sweep: stationary = weight slab pair, moving = x slab pair
        def w_qk(t, s0, s1, msl):
            return _slab_pair(t, s0, s1, msl)

        def x_qk(t, s0, s1, _):
            return _slab_pair(t, s0, s1, None)

        first = True
        for tb in range(NTB):
            tsl = slice(tb * TB, (tb + 1) * TB)
            xs_t = p1x.tile([P, 2 * KO, TB], F8, tag="xs", name=f"xs{tb}")
            if first:
                # fine-grained first loads so the first m-block's A chain can
                # start after ~1MB instead of ~4MB of DMA
                Q = NQK * P // 4
                nc.sync.dma_start(xs_t[:, 0 : KO // 2, :], xs[:, 0 : KO // 2, tsl])
                nc.sync.dma_start(wqk_t[:, :, 0:Q], wqks[:, :, 0:Q])
                nc.sync.dma_start(xs_t[:, KO // 2 : KO, :], xs[:, KO // 2 : KO, tsl])
                nc.sync.dma_start(wqk_t[:, :, Q : 2 * Q], wqks[:, :, Q : 2 * Q])
                nc.sync.dma_start(xs_t[:, KO : 2 * KO, :], xs[:, KO : 2 * KO, tsl])
                nc.sync.dma_start(wqkr_t[:, :, 0 : 2 * Q], wqkrs[:, :, 0 : 2 * Q])
                nc.sync.dma_start(wqk_t[:, :, 2 * Q :], wqks[:, :, 2 * Q :])
                nc.sync.dma_start(wqkr_t[:, :, 2 * Q :], wqkrs[:, :, 2 * Q :])
                nc.sync.dma_start(wv_t[:], wvs[:])
                nc.sync.dma_start(wvr_t[:], wvrs[:])
                # const loads ride behind the critical phase-1 loads
                for dma in const_dmas:
                    dma()
                first = False
            else:
                nc.sync.dma_start(xs_t[:, 0:KO, :], xs[:, 0:KO, tsl])
                nc.sync.dma_start(xs_t[:, KO : 2 * KO, :], xs[:, KO : 2 * KO, tsl])

            for m in range(NQK):
                msl = slice(m * P, (m + 1) * P)
                ps = p1p.tile([P, TB], F32, tag="ps1", name=f"psqk{tb}_{m}")
                _emit_3term(nc, ps, wqk_t, wqkr_t, xs_t, msl, None, w_qk, x_qk)
                cp = (nc.vector.tensor_copy, nc.scalar.copy)[m % 2]
                cp(qk_sb[m][:, tsl], ps[:])
            for t4 in range(4):
                t4sl = slice(t4 * P, (t4 + 1) * P)
                ps = p1p.tile([P, HPC * HD], F32, tag="ps1", name=f"psv{tb}_{t4}")
                # v: out[t, hd] — stationary x slabs sliced to t4, moving wv
                seq = []
                for j in range(1, KO):
                    seq.append((_slab_pair(xs_t, 2 * j - 1, 2 * j, t4sl), _slab_pair(wv_t, j - 1, j, None)))
                for m2 in range(KO // 2):
                    seq.append((_slab_pair(xs_t, 4 * m2, 4 * m2 + 2, t4sl), _slab_pair(wvr_t, 2 * m2, 2 * m2 + 1, None)))
                seq.append((_slab_pair(xs_t, 0, 2 * KO - 1, t4sl), _slab_pair(wv_t, 0, KO - 1, None)))
                for i, (x_ap, w_ap) in enumerate(seq):
                    nc.tensor.matmul(ps[:], x_ap, w_ap, start=(i == 0), stop=(i == len(seq) - 1), perf_mode=DR)
                cp = (nc.vector.tensor_copy, nc.scalar.copy)[t4 % 2]
                cp(v_sb[tb * 4 + t4][:], ps[:])
            # head-0 rope for this t-block rides inside phase 1 so the DVE
            # blend queue is warm when attention starts
            qr0, kr0, cs_sb, jT_sb = rope0
            _rope_block(nc, p1pj, p1t, qk_sb, cs_sb, jT_sb, 0, qr0, kr0, tb, tag="psj1")


def _phase2(nc, tc, outS, qk_sb, v_sb, jT_sb, mask_sb, ones_sb, cs_sb, r0, emit_p3, aux_pool):
    with (
        tc.tile_pool(name="p2r", bufs=2) as p2r,
        tc.tile_pool(name="p2pt", bufs=8) as p2pt,
        tc.tile_pool(name="p2rec", bufs=4) as p2rec,
        tc.tile_pool(name="p2ps", bufs=2, space="PSUM") as p2ps,
        tc.tile_pool(name="p2po", bufs=2, space="PSUM") as p2po,
    ):
        def alloc_roped(h):
            qr = p2r.tile([P, T], BF16, tag="qr", name=f"qr{h}")
            kr = p2r.tile([P, T], BF16, tag="kr", name=f"kr{h}")
            return qr, kr

        def attn_tq(h, tq, qr, kr, pending):
            """One q-tile of attention, software-pipelined over PAIRS of
            128-wide k-blocks: the two STs of a pair land in two PSUM banks
            of one tile so a single exp (and, on the diagonal, a single mask
            multiply) covers both.  PV/ones matmuls trail via `pending`."""
            sl = slice(tq * TQ, (tq + 1) * TQ)
            nk = (tq + 1) * BANDS
            # ps_o (PV) and ps_d (denominator) share one 2KB bank: the first
            # PV's start zeroes the whole region, so the denominator chain
            # never carries start (verified region-zero semantics on hw).
            ps_od = p2po.tile([P, 2, TQ], F32, tag="po", name=f"po{h}{tq}")
            ps_o = ps_od[:, 0, :]
            ps_d = ps_od[:, 1, :]

            def issue_pair(kp):
                # quarters 0 and 2 of a 2-bank tile: each ST owns a bank
                ps_st = p2ps.tile([P, 4, TQ], F32, tag="st", name=f"st{h}{tq}{kp}")
                for j in range(2):
                    nc.tensor.matmul(
                        ps_st[:, 2 * j, :], kr[:, (2 * kp + j) * P : (2 * kp + j + 1) * P],
                        qr[:, sl], start=True, stop=True,
                    )
                pt = p2pt.tile([P, 2, TQ], BF16, tag="pt", name=f"pt{h}{tq}{kp}")
                nc.scalar.activation(
                    pt[:], ps_st[:, 0:4:2, :], mybir.ActivationFunctionType.Exp, scale=SCALE_EFF
                )
                if kp == tq:  # diagonal pair: mask both bands at once
                    nc.vector.tensor_tensor(pt[:], pt[:], mask_sb[:], mybir.AluOpType.mult)
                return pt

            def make_pv(kp, pt):
                def pv():
                    for j in range(2):
                        kb = 2 * kp + j
                        nc.tensor.matmul(
                            ps_o, v_sb[kb][:, h * HD : (h + 1) * HD], pt[:, j, :],
                            start=(kb == 0), stop=False, skip_group_check=True,
                        )
                        nc.tensor.matmul(
                            ps_d, ones_sb[:], pt[:, j, :], start=False,
                            stop=(kb == nk - 1), skip_group_check=True,
                        )
                    if 2 * kp + 1 == nk - 1:
                        rec = p2rec.tile([P, TQ], F32, tag="rec", name=f"rec{h}{tq}")
                        nc.vector.reciprocal(rec[:], ps_d)
                        ob = p2rec.tile([P, TQ], BF16, tag="ob", name=f"ob{h}{tq}")
                        nc.vector.tensor_tensor(ob[:], ps_o, rec[:], mybir.AluOpType.mult)
                        hi = outS[tq][:, 2 * h, :]
                        nc.vector.tensor_copy(hi, ob[:])
                        nc.vector.scalar_tensor_tensor(
                            outS[tq][:, 2 * h + 1, :], ob[:], 1.0, hi,
                            mybir.AluOpType.mult, mybir.AluOpType.subtract,
                        )
                return pv

            for kp in range(nk // 2):
                pt = issue_pair(kp)
                if len(pending) >= 3:
                    pending.pop(0)()
                pending.append(make_pv(kp, pt))

        # rope for head h+1 is interleaved into head h's attention (one
        # 512-wide t-block per pair of q-tiles); head 0 was roped inside
        # phase 1.  During the last head, phase-3 tiles are emitted one
        # q-tile behind so output projection overlaps the attention tail.
        roped = [r0]
        pending = []
        for h in range(HPC):
            if h + 1 < HPC:
                roped.append(alloc_roped(h + 1))
            qr, kr = roped[h]
            for tq in range(NTQ):
                attn_tq(h, tq, qr, kr, pending)
                if h + 1 < HPC:
                    if tq % 2 == 0:
                        _rope_block(nc, aux_pool, p2pt, qk_sb, cs_sb, jT_sb,
                                    h + 1, roped[h + 1][0], roped[h + 1][1], tq // 2,
                                    tag="ps3", psj_bufs=2)
                elif tq >= 2:
                    # two q-tiles behind: guarantees head-3's normalization
                    # for tq-2 has been emitted (pending is only 3 pairs deep)
                    emit_p3(tq - 2)
            if h == HPC - 1:
                while pending:
                    pending.pop(0)()
        emit_p3(NTQ - 2)
        emit_p3(NTQ - 1)


def _make_p3(nc, p3s, p3p, outS, wos_sb, wors_sb, y):
    ydescale = 1.0 / (SO * SWO)

    def emit_p3(tq):
        for tt in range(tq * BANDS, (tq + 1) * BANDS):
            off = (tt - tq * BANDS) * P
            osl = slice(off, off + P)
            ysb = p3s.tile([P, D], F32, tag="ysb", name=f"ysb{tt}")
            last = tq == NTQ - 1
            for dd in range(D // TB):
                dsl = slice(dd * TB, (dd + 1) * TB)
                ps = p3p.tile([P, TB], F32, tag="ps3", name=f"ps3{tt}{dd}")
                seq = []
                for j in range(1, HPC):
                    seq.append((outS[tq][:, 2 * j - 1 : 2 * j + 1, osl], wos_sb[:, j - 1 : j + 1, dsl]))
                for m in range(HPC // 2):
                    seq.append((outS[tq][:, 4 * m : 4 * m + 3 : 2, osl], wors_sb[:, 2 * m : 2 * m + 2, dsl]))
                seq.append((outS[tq][:, 0 : 2 * HPC : 2 * HPC - 1, osl], wos_sb[:, 0 : HPC : HPC - 1, dsl]))
                for i, (o_ap, w_ap) in enumerate(seq):
                    nc.tensor.matmul(ps[:], o_ap, w_ap, start=(i == 0), stop=(i == len(seq) - 1), perf_mode=DR)
                if dd % 2 == 0:
                    nc.vector.tensor_scalar_mul(ysb[:, dsl], ps[:], ydescale)
                else:
                    nc.scalar.mul(ysb[:, dsl], ps[:], ydescale)
                if last:
                    # small per-dd stores shrink the end-of-kernel DMA tail
                    nc.sync.dma_start(
                        y[tt * P : (tt + 1) * P, dd * TB : (dd + 1) * TB],
                        ysb[:, dd * TB : (dd + 1) * TB],
                    )
            if not last:
                nc.sync.dma_start(y[tt * P : (tt + 1) * P, :], ysb[:])
    return emit_p3


def _build_program():
    nc = bass.Bass()

    xs = nc.dram_tensor("xs", (P, 2 * KO, T), F8, kind="ExternalInput")
    wqks = nc.dram_tensor("wqks", (P, KO, NQK * P), F8, kind="ExternalInput")
    wqkrs = nc.dram_tensor("wqkrs", (P, KO, NQK * P), F8, kind="ExternalInput")
    wvs = nc.dram_tensor("wvs", (P, KO, HPC * HD), F8, kind="ExternalInput")
    wvrs = nc.dram_tensor("wvrs", (P, KO, HPC * HD), F8, kind="ExternalInput")
    wos = nc.dram_tensor("wos", (P, HPC, D), F8, kind="ExternalInput")
    wors = nc.dram_tensor("wors", (P, HPC, D), F8, kind="ExternalInput")
    cs = nc.dram_tensor("cs", (P, 2, T), BF16, kind="ExternalInput")
    masks = nc.dram_tensor("masks", (BANDS, P, TQ), BF16, kind="ExternalInput")
    jT = nc.dram_tensor("jT", (P, P), BF16, kind="ExternalInput")
    ones = nc.dram_tensor("ones", (P, P), BF16, kind="ExternalInput")
    y = nc.dram_tensor("y", (T, D), F32, kind="ExternalOutput")

    with tile.TileContext(nc) as tc:
        with (
            tc.tile_pool(name="consts", bufs=1) as consts,
            tc.tile_pool(name="qkv", bufs=1) as qkvp,
        ):
            jT_sb = consts.tile([P, P], BF16)
            mask_sb = consts.tile([P, BANDS, TQ], BF16)
            ones_sb = consts.tile([P, P], BF16)
            cs_sb = consts.tile([P, 2, T], BF16)
            wos_sb = consts.tile([P, HPC, D], F8)
            wors_sb = consts.tile([P, HPC, D], F8)
            const_dmas = [
                lambda: nc.sync.dma_start(cs_sb[:], cs[:]),
                lambda: nc.sync.dma_start(jT_sb[:], jT[:]),
                lambda: nc.sync.dma_start(ones_sb[:], ones[:]),
                lambda: nc.sync.dma_start(mask_sb[:], masks.rearrange("a p j -> p a j")),
                lambda: nc.sync.dma_start(wos_sb[:], wos[:]),
                lambda: nc.sync.dma_start(wors_sb[:], wors[:]),
            ]

            qk_sb = [qkvp.tile([P, T], BF16, name=f"qk{m}") for m in range(NQK)]
            v_sb = [qkvp.tile([P, HPC * HD], BF16, name=f"v{kb}") for kb in range(T // P)]
            qr0 = qkvp.tile([P, T], BF16, name="qr0")
            kr0 = qkvp.tile([P, T], BF16, name="kr0")

            _phase1(nc, tc, xs, wqks, wqkrs, wvs, wvrs, qk_sb, v_sb,
                    (qr0, kr0, cs_sb, jT_sb), const_dmas)

            with (
                tc.tile_pool(name="outT", bufs=1) as outT_pool,
                tc.tile_pool(name="p3s", bufs=3) as p3s,
                tc.tile_pool(name="p3p", bufs=2, space="PSUM") as p3p,
            ):
                outS = {
                    tq: outT_pool.tile([P, 2 * HPC, TQ], F8, tag=f"outS{tq}", name=f"outS{tq}")
                    for tq in range(NTQ)
                }
                emit_p3 = _make_p3(nc, p3s, p3p, outS, wos_sb, wors_sb, y)
                _phase2(nc, tc, outS, qk_sb, v_sb, jT_sb, mask_sb, ones_sb, cs_sb,
                        (qr0, kr0), emit_p3, p3p)

    _fix_waits(nc)
    return nc


_NC_CACHE = None


def _get_program():
    global _NC_CACHE
    if _NC_CACHE is None:
        _NC_CACHE = _build_program()
    return _NC_CACHE


def _q8(a, s):
    """e4m3-quantize a*s (clipped to TRN e4m3 range); returns (fp8, residual
    fp8) with the residual on the same scale (no prescale — its values live
    in e4m3's normal range already)."""
    import ml_dtypes

    F8np = ml_dtypes.float8_e4m3
    scaled = np.clip(a * s, -240.0, 240.0)
    hi = scaled.astype(F8np)
    lo = np.clip(scaled - hi.astype(np.float32), -240.0, 240.0).astype(F8np)
    return hi, lo


def _pack_k(a):
    """[K, M] -> [P, KO', M] with slab i on partitions (rows 128i+p)."""
    ko = a.shape[0] // P
    return np.ascontiguousarray(a.reshape(ko, P, a.shape[1]).transpose(1, 0, 2))


def _host_inputs(x, Wqkv, Wout, cos, sin, rope_mask):
    import ml_dtypes

    BF = ml_dtypes.bfloat16
    x = np.asarray(x, dtype=np.float32)
    Wqkv = np.asarray(Wqkv, dtype=np.float32)
    Wout = np.asarray(Wout, dtype=np.float32)
    cos = np.asarray(cos, dtype=np.float32)
    sin = np.asarray(sin, dtype=np.float32)
    rope_mask = np.asarray(rope_mask).astype(bool)

    # J^T for the pair-rotation matmul: (J q)[2i] = -q[2i+1], (J q)[2i+1] = q[2i]
    jT = np.zeros((P, P), dtype=np.float32)
    for i in range(P // 2):
        jT[2 * i, 2 * i + 1] = 1.0
        jT[2 * i + 1, 2 * i] = -1.0

    masks = np.zeros((BANDS, P, TQ), dtype=BF)
    ii = np.arange(P)[:, None]
    jj = np.arange(TQ)[None, :]
    for a in range(BANDS):
        masks[a] = (ii + a * P <= jj).astype(BF)

    C_full = np.repeat(cos[:T].T, 2, axis=0).astype(np.float32)  # [128, T]
    S_full = np.repeat(sin[:T].T, 2, axis=0).astype(np.float32)

    # per-batch x packs (shared by the 4 cores of each batch)
    xs_b = []
    for b in range(B):
        x8, xr8 = _q8(x[b].T, SX)  # [D, T] fp8
        xsp = np.empty((P, 2 * KO, T), dtype=x8.dtype)
        xsp[:, 0::2] = _pack_k(x8)
        xsp[:, 1::2] = _pack_k(xr8)
        xs_b.append(xsp)

    in_maps = []
    for c in range(N_CORES):
        b = c // CORES_PER_B
        hg = c % CORES_PER_B
        heads = [hg * HPC + i for i in range(HPC)]

        qrows = np.concatenate([np.arange(h * HD, (h + 1) * HD) for h in heads])
        krows = qrows + D
        vrows = qrows + 2 * D
        wqk = Wqkv[np.concatenate([qrows, krows])].T  # [D, 1024]
        wv = Wqkv[vrows].T                            # [D, 512]
        wqk8, wqkr8 = _q8(wqk, SW)
        wv8, wvr8 = _q8(wv, SW)

        woT = np.ascontiguousarray(Wout[:, qrows].T)  # [512, D]
        wo8, wor8 = _q8(woT, SWO)
        wos_p = np.ascontiguousarray(wo8.reshape(HPC, P, D).transpose(1, 0, 2))
        wors_p = np.ascontiguousarray(wor8.reshape(HPC, P, D).transpose(1, 0, 2))

        flags = [bool(rope_mask[h]) for h in heads]
        assert all(f == flags[0] for f in flags), (
            "heads in one core must share a rope flag for the single-table path"
        )
        cs_arr = np.empty((P, 2, T), dtype=BF)
        if flags[0]:
            cs_arr[:, 0] = C_full.astype(BF)
            cs_arr[:, 1] = S_full.astype(BF)
        else:
            cs_arr[:, 0] = np.ones((P, T), dtype=BF)
            cs_arr[:, 1] = np.zeros((P, T), dtype=BF)

        in_maps.append(
            {
                "xs": xs_b[b],
                "wqks": _pack_k(wqk8),
                "wqkrs": _pack_k(wqkr8),
                "wvs": _pack_k(wv8),
                "wvrs": _pack_k(wvr8),
                "wos": wos_p,
                "wors": wors_p,
                "cs": cs_arr,
                "masks": masks,
                "jT": jT.astype(BF),
                "ones": np.full((P, P), SIGMA / SO, dtype=BF),
            }
        )
    return in_maps


def kernel(x, Wqkv, Wout, cos, sin, rope_mask, _trace=False):
    nc = _get_program()
    in_maps = _host_inputs(x, Wqkv, Wout, cos, sin, rope_mask)
    res = run_bass_kernel_spmd(nc, in_maps, core_ids=list(range(N_CORES)), trace=_trace)
    parts = [res.results[c]["y"] for c in range(N_CORES)]
    out = np.stack(
        [sum(parts[b * CORES_PER_B : (b + 1) * CORES_PER_B]) for b in range(B)]
    ).astype(np.float32)
    if _trace:
        kernel.last_result = res
    return out


# revision 30
# speedup vs baseline: 1.3155x; 1.0213x over previous
"""Causal self-attention (B=2, T=2048, D=2048, H=16, hd=128, RoPE on masked
heads) as a Bass/Tile kernel on 8 Trainium2 NeuronCores.

Sharding: core c handles batch b=c//4 and heads 4*(c%4)..4*(c%4)+3 (data
parallel on B x tensor parallel on H).  Each core computes a partial output
projection y_b = O_local @ Wout_local^T; the host sums the 4 partials per
batch.

Numerics/performance strategy:
- QKV projection runs as fp8(e4m3) DoubleRow matmuls with 3-term residual
  compensation: x*W ~ x8*W8 + xr8*W8 + x8*Wr8, where xr8/Wr8 are e4m3
  quantizations of the quantization residuals (host-prepared).  Each
  DoubleRow instruction contracts two 128-row K-slabs at half cost, so the
  projection runs at 1.5x the bf16 matmul rate with ~1e-3 relative error.
  The 3 terms are packed into 24 DoubleRow instructions per output tile via
  a chain pairing that needs no operand duplication (see _emit_3term).
- Attention (scores, softmax, PV, denominator) runs in bf16: S^T = K Q^T in
  transposed score space so softmax normalization is a per-free-element
  multiply; denominator via an all-ones stationary matmul.
- q, k, v stay resident in SBUF between phases (bf16) - no DRAM scratch.
- RoPE tables are a single per-core C/S pair (identity for NoPE cores);
  roped = C*q + S*(J q) with J applied as a PE matmul.
- Output projection in bf16 with Wout pre-scaled by the fp8 descale factor.
"""

import sys

sys.path.insert(0, "/opt/trn_rl_repo")

import numpy as np

import concourse.bass as bass
import concourse.mybir as mybir
import concourse.tile as tile
from concourse.bass_utils import run_bass_kernel_spmd

F32 = mybir.dt.float32
F8 = mybir.dt.float8e4
BF16 = mybir.dt.bfloat16
DR = mybir.MatmulPerfMode.DoubleRow

B = 2
T = 2048
D = 2048
H = 16
HD = 128
N_CORES = 8
HPC = 4           # heads per core
CORES_PER_B = 4
P = 128
TB = 512          # t-block width (phase 1 / rope)
NTB = T // TB     # 4
TQ = 256          # attention q-tile width (phase 2)
NTQ = T // TQ     # 8
BANDS = TQ // P   # 2
KO = D // P       # 16 contraction K-blocks of 128
NQK = 2 * HPC     # 8 q+k dout blocks of 128
SX = 16.0         # fp8 scale for x
SW = 1024.0       # fp8 scale for Wqkv
SWO = 1024.0      # fp8 scale for Wout
SO = 32.0         # fp8 scale carried by the normalized attention output
SIGMA = SX * SW   # scale carried by q,k,v in SBUF
SCALE_EFF = (1.0 / float(np.sqrt(HD))) / (SIGMA * SIGMA)


# ---------------------------------------------------------------------------
# Walrus on this toolchain rejects instructions carrying more than one sync
# wait command; Tile can emit several (e.g. the kernel-tail drain).  Hoist
# the excess onto injected same-engine NoOps — semantically identical.
def _fix_waits(nc, cap=1):
    ctr = 0
    for f in nc.m.functions:
        for bb in f.blocks:
            insts = bb.instructions
            i = 0
            while i < len(insts):
                inst = insts[i]
                si = inst.sync_info
                if si is not None and si.on_wait and len(si.on_wait) > cap:
                    waits = list(si.on_wait)
                    keep, excess = waits[:cap], waits[cap:]
                    nops = []
                    for j in range(0, len(excess), cap):
                        ctr += 1
                        nops.append(
                            mybir.InstNoOp(
                                name=f"I-waitfix-{ctr}",
                                engine=inst.engine,
                                sync_info=mybir.SyncInfo(
                                    on_wait=excess[j : j + cap], on_update=[]
                                ),
                            )
                        )
                    inst.sync_info = mybir.SyncInfo(
                        on_wait=keep, on_update=list(si.on_update or [])
                    )
                    insts[i:i] = nops
                    i += len(nops)
                i += 1
    return ctr


def _emit_3term(nc, ps, w_sb, wr_sb, xs_t, msl, tsl, w_of_pair, x_of_pair):
    """Emit the 24 DoubleRow matmuls of one 3-term-compensated K=2048
    contraction into PSUM tile `ps`.

    xs_t holds 32 K-slabs (2i = x8_i, 2i+1 = xr8_i); w_sb/wr_sb hold 16
    slabs each (W8_i / Wr8_i).  Chain pairing covers x8_i*W8_i, xr8_i*W8_i
    (A instructions) and x8_i*Wr8_i (B instructions) with constant-stride
    slab pairs only.  `w_of_pair(w_tile, s0, s1, msl)` / `x_of_pair(xs, s0,
    s1, tsl)` build the [128, 2, *] APs (orientation differs between the
    q/k and v sweeps).
    """
    seq = []
    # A_1..A_15: x slabs (2j-1, 2j), w slabs (j-1, j)
    for j in range(1, KO):
        seq.append((w_of_pair(w_sb, j - 1, j, msl), x_of_pair(xs_t, 2 * j - 1, 2 * j, tsl)))
    # B_0..B_7: x slabs (4m, 4m+2), wr slabs (2m, 2m+1)
    for m in range(KO // 2):
        seq.append((w_of_pair(wr_sb, 2 * m, 2 * m + 1, msl), x_of_pair(xs_t, 4 * m, 4 * m + 2, tsl)))
    # A_0: x slabs (0, 31), w slabs (0, 15)
    seq.append((w_of_pair(w_sb, 0, KO - 1, msl), x_of_pair(xs_t, 0, 2 * KO - 1, tsl)))
    n = len(seq)
    for i, (w_ap, x_ap) in enumerate(seq):
        nc.tensor.matmul(ps[:], w_ap, x_ap, start=(i == 0), stop=(i == n - 1), perf_mode=DR)


def _slab_pair(t, s0, s1, csl):
    """AP [128, 2, cols] selecting slabs s0 < s1 of a [P, nslab, C] tile."""
    if csl is None:
        return t[:, s0 : s1 + 1 : (s1 - s0), :] if s1 - s0 > 1 else t[:, s0 : s1 + 1, :]
    step = s1 - s0
    if step > 1:
        return t[:, s0 : s1 + 1 : step, csl]
    return t[:, s0 : s1 + 1, csl]


def _rope_block(nc, psum_pool, tmp_pool, qk_sb, cs_sb, jT_sb, h, qr, kr, rb, tag="psj", psj_bufs=2):
    """RoPE for one 512-wide t-block of head h: roped = C*q + S*(J q).
    qr/kr are per-t-block tile lists so consumers only depend on their own
    block's blend, not the whole head."""
    sl = slice(rb * TB, (rb + 1) * TB)
    for si, (src_t, dst) in enumerate(((qk_sb[h], qr[rb]), (qk_sb[HPC + h], kr[rb]))):
        psj = psum_pool.tile([P, TB], F32, tag=tag, name=f"psj{h}_{rb}_{si}", bufs=psj_bufs)
        nc.tensor.matmul(psj[:], jT_sb[:], src_t[:, sl], start=True, stop=True)
        tmp = tmp_pool.tile([P, TB], BF16, tag="ropetmp", name=f"rtmp{h}_{rb}_{si}")
        nc.vector.tensor_tensor(tmp[:], psj[:], cs_sb[:, 1, sl], mybir.AluOpType.mult)
        nc.vector.tensor_tensor(dst[:], src_t[:, sl], cs_sb[:, 0, sl], mybir.AluOpType.mult)
        nc.vector.tensor_tensor(dst[:], dst[:], tmp[:], mybir.AluOpType.add)


def _phase1(nc, tc, xs, wqks, wqkrs, wvs, wvrs, qk_sb, v_sb, rope0, const_dmas):
    with (
        tc.tile_pool(name="p1w", bufs=1) as p1w,
        tc.tile_pool(name="p1x", bufs=2) as p1x,
        tc.tile_pool(name="p1t", bufs=2) as p1t,
        tc.tile_pool(name="p1p", bufs=4, space="PSUM") as p1p,
        tc.tile_pool(name="p1pj", bufs=2, space="PSUM") as p1pj,
    ):
        wqk_t = p1w.tile([P, 4, KO, NQK * P // 4], F8, name="wqks")
        wqkr_t = p1w.tile([P, 4, KO, NQK * P // 4], F8, name="wqkrs")
        wv_t = p1w.tile([P, KO, HPC * HD], F8, name="wvs")
        wvr_t = p1w.tile([P, KO, HPC * HD], F8, name="wvrs")

        # q/k sweep: stationary = weight slab pair, moving = x slab pair
        def w_qk(t, s0, s1, msl):
            return _slab_pair(t, s0, s1, msl)

        def x_qk(t, s0, s1, _):
            return _slab_pair(t, s0, s1, None)

        first = True
        for tb in range(NTB):
            tsl = slice(tb * TB, (tb + 1) * TB)
            xs_t = p1x.tile([P, 2 * KO, TB], F8, tag="xs", name=f"xs{tb}")
            if first:
                # fine-grained first loads so the first m-block's A chain can
                # start after ~1MB instead of ~4MB of DMA
                nc.sync.dma_start(xs_t[:, 0 : KO // 2, :], xs[:, 0 : KO // 2, tsl])
                nc.sync.dma_start(wqk_t[:, 0], wqks[:, 0])
                nc.sync.dma_start(xs_t[:, KO // 2 : KO, :], xs[:, KO // 2 : KO, tsl])
                nc.sync.dma_start(wqk_t[:, 1], wqks[:, 1])
                nc.sync.dma_start(xs_t[:, KO : 2 * KO, :], xs[:, KO : 2 * KO, tsl])
                nc.sync.dma_start(wqkr_t[:, 0], wqkrs[:, 0])
                nc.sync.dma_start(wqkr_t[:, 1], wqkrs[:, 1])
                nc.sync.dma_start(wqk_t[:, 2], wqks[:, 2])
                nc.sync.dma_start(wqk_t[:, 3], wqks[:, 3])
                nc.sync.dma_start(wqkr_t[:, 2], wqkrs[:, 2])
                nc.sync.dma_start(wqkr_t[:, 3], wqkrs[:, 3])
                nc.sync.dma_start(wv_t[:], wvs[:])
                nc.sync.dma_start(wvr_t[:], wvrs[:])
                # const loads ride behind the critical phase-1 loads
                for dma in const_dmas:
                    dma()
                first = False
            else:
                nc.sync.dma_start(xs_t[:, 0:KO, :], xs[:, 0:KO, tsl])
                nc.sync.dma_start(xs_t[:, KO : 2 * KO, :], xs[:, KO : 2 * KO, tsl])

            for m in range(NQK):
                qq, hh = m // 2, m % 2
                msl = slice(hh * P, (hh + 1) * P)

                def w_qk_q(t, s0, s1, _msl):
                    step = s1 - s0
                    if step > 1:
                        return t[:, qq, s0 : s1 + 1 : step, _msl]
                    return t[:, qq, s0 : s1 + 1, _msl]

                ps = p1p.tile([P, TB], F32, tag="ps1", name=f"psqk{tb}_{m}")
                _emit_3term(nc, ps, wqk_t, wqkr_t, xs_t, msl, None, w_qk_q, x_qk)
                cp = (nc.vector.tensor_copy, nc.scalar.copy)[m % 2]
                cp(qk_sb[m][:, tsl], ps[:])
            for t4 in range(4):
                t4sl = slice(t4 * P, (t4 + 1) * P)
                ps = p1p.tile([P, HPC * HD], F32, tag="ps1", name=f"psv{tb}_{t4}")
                # v: out[t, hd] — stationary x slabs sliced to t4, moving wv
                seq = []
                for j in range(1, KO):
                    seq.append((_slab_pair(xs_t, 2 * j - 1, 2 * j, t4sl), _slab_pair(wv_t, j - 1, j, None)))
                for m2 in range(KO // 2):
                    seq.append((_slab_pair(xs_t, 4 * m2, 4 * m2 + 2, t4sl), _slab_pair(wvr_t, 2 * m2, 2 * m2 + 1, None)))
                seq.append((_slab_pair(xs_t, 0, 2 * KO - 1, t4sl), _slab_pair(wv_t, 0, KO - 1, None)))
                for i, (x_ap, w_ap) in enumerate(seq):
                    nc.tensor.matmul(ps[:], x_ap, w_ap, start=(i == 0), stop=(i == len(seq) - 1), perf_mode=DR)
                cp = (nc.vector.tensor_copy, nc.scalar.copy)[t4 % 2]
                cp(v_sb[tb * 4 + t4][:], ps[:])
            # head-0 rope for this t-block rides inside phase 1 so the DVE
            # blend queue is warm when attention starts
            qr0, kr0, cs_sb, jT_sb = rope0
            _rope_block(nc, p1pj, p1t, qk_sb, cs_sb, jT_sb, 0, qr0, kr0, tb, tag="psj1")


def _phase2(nc, tc, outS, qk_sb, v_sb, jT_sb, mask_sb, ones_sb, cs_sb, r0, emit_p3, aux_pool):
    with (
        tc.tile_pool(name="p2r", bufs=2) as p2r,
        tc.tile_pool(name="p2pt", bufs=10) as p2pt,
        tc.tile_pool(name="p2rec", bufs=4) as p2rec,
        tc.tile_pool(name="p2ps", bufs=4, space="PSUM") as p2ps,
        tc.tile_pool(name="p2po", bufs=2, space="PSUM") as p2po,
    ):
        def alloc_roped(h):
            qr = [p2r.tile([P, TB], BF16, tag=f"qr{rb}", name=f"qr{h}_{rb}") for rb in range(NTB)]
            kr = [p2r.tile([P, TB], BF16, tag=f"kr{rb}", name=f"kr{h}_{rb}") for rb in range(NTB)]
            return qr, kr

        def attn_tq(h, tq, qr, kr, pending):
            """One q-tile of attention, software-pipelined over PAIRS of
            128-wide k-blocks: the two STs of a pair land in two PSUM banks
            of one tile so a single exp (and, on the diagonal, a single mask
            multiply) covers both.  PV/ones matmuls trail via `pending`."""
            sl = slice(tq * TQ, (tq + 1) * TQ)
            nk = (tq + 1) * BANDS
            # ps_o (PV) and ps_d (denominator) share one 2KB bank: the first
            # PV's start zeroes the whole region, so the denominator chain
            # never carries start (verified region-zero semantics on hw).
            ps_od = p2po.tile([P, 2, TQ], F32, tag="po", name=f"po{h}{tq}")
            ps_o = ps_od[:, 0, :]
            ps_d = ps_od[:, 1, :]

            def issue_pair(kp):
                # both STs of a pair share one 2KB bank: the first carries
                # start (zeroing the region), the second relies on the
                # region-granular pending-zero (verified on hw)
                ps_st = p2ps.tile([P, 2, TQ], F32, tag="st", name=f"st{h}{tq}{kp}")
                qr_t = qr[tq * TQ // TB]
                qsl = slice((tq * TQ) % TB, (tq * TQ) % TB + TQ)
                for j in range(2):
                    kb = 2 * kp + j
                    kr_t = kr[kb * P // TB]
                    ksl = slice((kb * P) % TB, (kb * P) % TB + P)
                    nc.tensor.matmul(
                        ps_st[:, j, :], kr_t[:, ksl], qr_t[:, qsl],
                        start=(j == 0), stop=(j == 1), skip_group_check=True,
                    )
                pt = p2pt.tile([P, 2, TQ], BF16, tag="pt", name=f"pt{h}{tq}{kp}")
                nc.scalar.activation(
                    pt[:], ps_st[:], mybir.ActivationFunctionType.Exp, scale=SCALE_EFF
                )
                if kp == tq:  # diagonal pair: mask both bands at once
                    nc.vector.tensor_tensor(pt[:], pt[:], mask_sb[:], mybir.AluOpType.mult)
                return pt

            def make_pv(kp, pt):
                def pv():
                    for j in range(2):
                        kb = 2 * kp + j
                        nc.tensor.matmul(
                            ps_o, v_sb[kb][:, h * HD : (h + 1) * HD], pt[:, j, :],
                            start=(kb == 0), stop=False, skip_group_check=True,
                        )
                        nc.tensor.matmul(
                            ps_d, ones_sb[:], pt[:, j, :], start=False,
                            stop=(kb == nk - 1), skip_group_check=True,
                        )
                    if 2 * kp + 1 == nk - 1:
                        rec = p2rec.tile([P, TQ], F32, tag="rec", name=f"rec{h}{tq}")
                        nc.vector.reciprocal(rec[:], ps_d)
                        ob = p2rec.tile([P, TQ], BF16, tag="ob", name=f"ob{h}{tq}")
                        nc.vector.tensor_tensor(ob[:], ps_o, rec[:], mybir.AluOpType.mult)
                        hi = outS[tq][:, 2 * h, :]
                        nc.vector.tensor_copy(hi, ob[:])
                        nc.vector.scalar_tensor_tensor(
                            outS[tq][:, 2 * h + 1, :], ob[:], 1.0, hi,
                            mybir.AluOpType.mult, mybir.AluOpType.subtract,
                        )
                return pv

            for kp in range(nk // 2):
                pt = issue_pair(kp)
                if len(pending) >= 3:
                    pending.pop(0)()
                pending.append(make_pv(kp, pt))

        # rope for head h+1 is interleaved into head h's attention (one
        # 512-wide t-block per pair of q-tiles); head 0 was roped inside
        # phase 1.  During the last head, phase-3 tiles are emitted one
        # q-tile behind so output projection overlaps the attention tail.
        roped = [r0]
        pending = []
        for h in range(HPC):
            if h + 1 < HPC:
                roped.append(alloc_roped(h + 1))
            qr, kr = roped[h]
            for tq in range(NTQ):
                attn_tq(h, tq, qr, kr, pending)
                if h + 1 < HPC:
                    if tq % 2 == 0:
                        _rope_block(nc, aux_pool, p2pt, qk_sb, cs_sb, jT_sb,
                                    h + 1, roped[h + 1][0], roped[h + 1][1], tq // 2,
                                    tag="ps3", psj_bufs=2)
                elif tq >= 2:
                    # two q-tiles behind: guarantees head-3's normalization
                    # for tq-2 has been emitted (pending is only 3 pairs deep)
                    emit_p3(tq - 2)
            if h == HPC - 1:
                while pending:
                    pending.pop(0)()
        emit_p3(NTQ - 2)
        emit_p3(NTQ - 1)


def _make_p3(nc, p3s, p3p, outS, wos_sb, wors_sb, y):
    ydescale = 1.0 / (SO * SWO)

    def emit_p3(tq):
        for tt in range(tq * BANDS, (tq + 1) * BANDS):
            off = (tt - tq * BANDS) * P
            osl = slice(off, off + P)
            ysb = p3s.tile([P, D], BF16, tag="ysb", name=f"ysb{tt}")
            last = tq == NTQ - 1
            for dd in range(D // TB):
                dsl = slice(dd * TB, (dd + 1) * TB)
                ps = p3p.tile([P, TB], F32, tag="ps3", name=f"ps3{tt}{dd}")
                seq = []
                for j in range(1, HPC):
                    seq.append((outS[tq][:, 2 * j - 1 : 2 * j + 1, osl], wos_sb[:, j - 1 : j + 1, dsl]))
                for m in range(HPC // 2):
                    seq.append((outS[tq][:, 4 * m : 4 * m + 3 : 2, osl], wors_sb[:, 2 * m : 2 * m + 2, dsl]))
                seq.append((outS[tq][:, 0 : 2 * HPC : 2 * HPC - 1, osl], wos_sb[:, 0 : HPC : HPC - 1, dsl]))
                for i, (o_ap, w_ap) in enumerate(seq):
                    nc.tensor.matmul(ps[:], o_ap, w_ap, start=(i == 0), stop=(i == len(seq) - 1), perf_mode=DR)
                if dd % 2 == 0:
                    nc.vector.tensor_scalar_mul(ysb[:, dsl], ps[:], ydescale)
                else:
                    nc.scalar.mul(ysb[:, dsl], ps[:], ydescale)
                if last:
                    # small per-dd stores shrink the end-of-kernel DMA tail
                    nc.sync.dma_start(
                        y[tt * P : (tt + 1) * P, dd * TB : (dd + 1) * TB],
                        ysb[:, dd * TB : (dd + 1) * TB],
                    )
            if not last:
                nc.sync.dma_start(y[tt * P : (tt + 1) * P, :], ysb[:])
    return emit_p3


def _build_program():
    nc = bass.Bass()

    xs = nc.dram_tensor("xs", (P, 2 * KO, T), F8, kind="ExternalInput")
    wqks = nc.dram_tensor("wqks", (P, 4, KO, NQK * P // 4), F8, kind="ExternalInput")
    wqkrs = nc.dram_tensor("wqkrs", (P, 4, KO, NQK * P // 4), F8, kind="ExternalInput")
    wvs = nc.dram_tensor("wvs", (P, KO, HPC * HD), F8, kind="ExternalInput")
    wvrs = nc.dram_tensor("wvrs", (P, KO, HPC * HD), F8, kind="ExternalInput")
    wos = nc.dram_tensor("wos", (P, HPC, D), F8, kind="ExternalInput")
    wors = nc.dram_tensor("wors", (P, HPC, D), F8, kind="ExternalInput")
    cs = nc.dram_tensor("cs", (P, 2, T), BF16, kind="ExternalInput")
    masks = nc.dram_tensor("masks", (BANDS, P, TQ), BF16, kind="ExternalInput")
    jT = nc.dram_tensor("jT", (P, P), BF16, kind="ExternalInput")
    ones = nc.dram_tensor("ones", (P, P), BF16, kind="ExternalInput")
    y = nc.dram_tensor("y", (T, D), BF16, kind="ExternalOutput")

    with tile.TileContext(nc) as tc:
        with (
            tc.tile_pool(name="consts", bufs=1) as consts,
            tc.tile_pool(name="qkv", bufs=1) as qkvp,
        ):
            jT_sb = consts.tile([P, P], BF16)
            mask_sb = consts.tile([P, BANDS, TQ], BF16)
            ones_sb = consts.tile([P, P], BF16)
            cs_sb = consts.tile([P, 2, T], BF16)
            wos_sb = consts.tile([P, HPC, D], F8)
            wors_sb = consts.tile([P, HPC, D], F8)
            const_dmas = [
                lambda: nc.sync.dma_start(cs_sb[:], cs[:]),
                lambda: nc.sync.dma_start(jT_sb[:], jT[:]),
                lambda: nc.sync.dma_start(ones_sb[:], ones[:]),
                lambda: nc.sync.dma_start(mask_sb[:], masks.rearrange("a p j -> p a j")),
                lambda: nc.sync.dma_start(wos_sb[:], wos[:]),
                lambda: nc.sync.dma_start(wors_sb[:], wors[:]),
            ]

            qk_sb = [qkvp.tile([P, T], BF16, name=f"qk{m}") for m in range(NQK)]
            v_sb = [qkvp.tile([P, HPC * HD], BF16, name=f"v{kb}") for kb in range(T // P)]
            qr0 = [qkvp.tile([P, TB], BF16, name=f"qr0_{rb}") for rb in range(NTB)]
            kr0 = [qkvp.tile([P, TB], BF16, name=f"kr0_{rb}") for rb in range(NTB)]

            _phase1(nc, tc, xs, wqks, wqkrs, wvs, wvrs, qk_sb, v_sb,
                    (qr0, kr0, cs_sb, jT_sb), const_dmas)

            with (
                tc.tile_pool(name="outT", bufs=1) as outT_pool,
                tc.tile_pool(name="p3s", bufs=3) as p3s,
                tc.tile_pool(name="p3p", bufs=2, space="PSUM") as p3p,
            ):
                outS = {
                    tq: outT_pool.tile([P, 2 * HPC, TQ], F8, tag=f"outS{tq}", name=f"outS{tq}")
                    for tq in range(NTQ)
                }
                emit_p3 = _make_p3(nc, p3s, p3p, outS, wos_sb, wors_sb, y)
                _phase2(nc, tc, outS, qk_sb, v_sb, jT_sb, mask_sb, ones_sb, cs_sb,
                        (qr0, kr0), emit_p3, p3p)

    _fix_waits(nc)
    return nc


_NC_CACHE = None


def _get_program():
    global _NC_CACHE
    if _NC_CACHE is None:
        _NC_CACHE = _build_program()
    return _NC_CACHE


def _q8(a, s):
    """e4m3-quantize a*s (clipped to TRN e4m3 range); returns (fp8, residual
    fp8) with the residual on the same scale (no prescale — its values live
    in e4m3's normal range already)."""
    import ml_dtypes

    F8np = ml_dtypes.float8_e4m3
    scaled = np.clip(a * s, -240.0, 240.0)
    hi = scaled.astype(F8np)
    lo = np.clip(scaled - hi.astype(np.float32), -240.0, 240.0).astype(F8np)
    return hi, lo


def _pack_k(a):
    """[K, M] -> [P, KO', M] with slab i on partitions (rows 128i+p)."""
    ko = a.shape[0] // P
    return np.ascontiguousarray(a.reshape(ko, P, a.shape[1]).transpose(1, 0, 2))


def _host_inputs(x, Wqkv, Wout, cos, sin, rope_mask):
    import ml_dtypes

    BF = ml_dtypes.bfloat16
    x = np.asarray(x, dtype=np.float32)
    Wqkv = np.asarray(Wqkv, dtype=np.float32)
    Wout = np.asarray(Wout, dtype=np.float32)
    cos = np.asarray(cos, dtype=np.float32)
    sin = np.asarray(sin, dtype=np.float32)
    rope_mask = np.asarray(rope_mask).astype(bool)

    # J^T for the pair-rotation matmul: (J q)[2i] = -q[2i+1], (J q)[2i+1] = q[2i]
    jT = np.zeros((P, P), dtype=np.float32)
    for i in range(P // 2):
        jT[2 * i, 2 * i + 1] = 1.0
        jT[2 * i + 1, 2 * i] = -1.0

    masks = np.zeros((BANDS, P, TQ), dtype=BF)
    ii = np.arange(P)[:, None]
    jj = np.arange(TQ)[None, :]
    for a in range(BANDS):
        masks[a] = (ii + a * P <= jj).astype(BF)

    C_full = np.repeat(cos[:T].T, 2, axis=0).astype(np.float32)  # [128, T]
    S_full = np.repeat(sin[:T].T, 2, axis=0).astype(np.float32)

    # per-batch x packs (shared by the 4 cores of each batch)
    xs_b = []
    for b in range(B):
        x8, xr8 = _q8(x[b].T, SX)  # [D, T] fp8
        xsp = np.empty((P, 2 * KO, T), dtype=x8.dtype)
        xsp[:, 0::2] = _pack_k(x8)
        xsp[:, 1::2] = _pack_k(xr8)
        xs_b.append(xsp)

    in_maps = []
    for c in range(N_CORES):
        b = c // CORES_PER_B
        hg = c % CORES_PER_B
        heads = [hg * HPC + i for i in range(HPC)]

        qrows = np.concatenate([np.arange(h * HD, (h + 1) * HD) for h in heads])
        krows = qrows + D
        vrows = qrows + 2 * D
        wqk = Wqkv[np.concatenate([qrows, krows])].T  # [D, 1024]
        wv = Wqkv[vrows].T                            # [D, 512]
        wqk8, wqkr8 = _q8(wqk, SW)
        wv8, wvr8 = _q8(wv, SW)

        def pack_q(a):  # [P, KO, 1024] -> [P, 4, KO, 256] quarter-major
            pk = _pack_k(a)
            return np.ascontiguousarray(
                pk.reshape(P, KO, 4, NQK * P // 4).transpose(0, 2, 1, 3)
            )

        woT = np.ascontiguousarray(Wout[:, qrows].T)  # [512, D]
        wo8, wor8 = _q8(woT, SWO)
        wos_p = np.ascontiguousarray(wo8.reshape(HPC, P, D).transpose(1, 0, 2))
        wors_p = np.ascontiguousarray(wor8.reshape(HPC, P, D).transpose(1, 0, 2))

        flags = [bool(rope_mask[h]) for h in heads]
        assert all(f == flags[0] for f in flags), (
            "heads in one core must share a rope flag for the single-table path"
        )
        cs_arr = np.empty((P, 2, T), dtype=BF)
        if flags[0]:
            cs_arr[:, 0] = C_full.astype(BF)
            cs_arr[:, 1] = S_full.astype(BF)
        else:
            cs_arr[:, 0] = np.ones((P, T), dtype=BF)
            cs_arr[:, 1] = np.zeros((P, T), dtype=BF)

        in_maps.append(
            {
                "xs": xs_b[b],
                "wqks": pack_q(wqk8),
                "wqkrs": pack_q(wqkr8),
                "wvs": _pack_k(wv8),
                "wvrs": _pack_k(wvr8),
                "wos": wos_p,
                "wors": wors_p,
                "cs": cs_arr,
                "masks": masks,
                "jT": jT.astype(BF),
                "ones": np.full((P, P), SIGMA / SO, dtype=BF),
            }
        )
    return in_maps


def kernel(x, Wqkv, Wout, cos, sin, rope_mask, _trace=False):
    nc = _get_program()
    in_maps = _host_inputs(x, Wqkv, Wout, cos, sin, rope_mask)
    res = run_bass_kernel_spmd(nc, in_maps, core_ids=list(range(N_CORES)), trace=_trace)
    parts = [np.asarray(res.results[c]["y"], dtype=np.float32) for c in range(N_CORES)]
    out = np.stack(
        [sum(parts[b * CORES_PER_B : (b + 1) * CORES_PER_B]) for b in range(B)]
    ).astype(np.float32)
    if _trace:
        kernel.last_result = res
    return out


# revision 32
# speedup vs baseline: 1.3358x; 1.0154x over previous
"""Causal self-attention (B=2, T=2048, D=2048, H=16, hd=128, RoPE on masked
heads) as a Bass/Tile kernel on 8 Trainium2 NeuronCores.

Sharding: core c handles batch b=c//4 and heads 4*(c%4)..4*(c%4)+3 (data
parallel on B x tensor parallel on H).  Each core computes a partial output
projection y_b = O_local @ Wout_local^T; the host sums the 4 partials per
batch.

Numerics/performance strategy:
- QKV projection runs as fp8(e4m3) DoubleRow matmuls with 3-term residual
  compensation: x*W ~ x8*W8 + xr8*W8 + x8*Wr8, where xr8/Wr8 are e4m3
  quantizations of the quantization residuals (host-prepared).  Each
  DoubleRow instruction contracts two 128-row K-slabs at half cost, so the
  projection runs at 1.5x the bf16 matmul rate with ~1e-3 relative error.
  The 3 terms are packed into 24 DoubleRow instructions per output tile via
  a chain pairing that needs no operand duplication (see _emit_3term).
- Attention (scores, softmax, PV, denominator) runs in bf16: S^T = K Q^T in
  transposed score space so softmax normalization is a per-free-element
  multiply; denominator via an all-ones stationary matmul.
- q, k, v stay resident in SBUF between phases (bf16) - no DRAM scratch.
- RoPE tables are a single per-core C/S pair (identity for NoPE cores);
  roped = C*q + S*(J q) with J applied as a PE matmul.
- Output projection in bf16 with Wout pre-scaled by the fp8 descale factor.
"""

import sys

sys.path.insert(0, "/opt/trn_rl_repo")

import numpy as np

import concourse.bass as bass
import concourse.mybir as mybir
import concourse.tile as tile
from concourse.bass_utils import run_bass_kernel_spmd

F32 = mybir.dt.float32
F8 = mybir.dt.float8e4
BF16 = mybir.dt.bfloat16
DR = mybir.MatmulPerfMode.DoubleRow

B = 2
T = 2048
D = 2048
H = 16
HD = 128
N_CORES = 8
HPC = 4           # heads per core
CORES_PER_B = 4
P = 128
TB = 512          # t-block width (phase 1 / rope)
NTB = T // TB     # 4
TQ = 256          # attention q-tile width (phase 2)
NTQ = T // TQ     # 8
BANDS = TQ // P   # 2
KO = D // P       # 16 contraction K-blocks of 128
NQK = 2 * HPC     # 8 q+k dout blocks of 128
SX = 16.0         # fp8 scale for x
SW = 1024.0       # fp8 scale for Wqkv
SWO = 1024.0      # fp8 scale for Wout
SO = 32.0         # fp8 scale carried by the normalized attention output
SIGMA = SX * SW   # scale carried by q,k,v in SBUF
SCALE_EFF = (1.0 / float(np.sqrt(HD))) / (SIGMA * SIGMA)


# ---------------------------------------------------------------------------
# Walrus on this toolchain rejects instructions carrying more than one sync
# wait command; Tile can emit several (e.g. the kernel-tail drain).  Hoist
# the excess onto injected same-engine NoOps — semantically identical.
def _fix_waits(nc, cap=1):
    ctr = 0
    for f in nc.m.functions:
        for bb in f.blocks:
            insts = bb.instructions
            i = 0
            while i < len(insts):
                inst = insts[i]
                si = inst.sync_info
                if si is not None and si.on_wait and len(si.on_wait) > cap:
                    waits = list(si.on_wait)
                    keep, excess = waits[:cap], waits[cap:]
                    nops = []
                    for j in range(0, len(excess), cap):
                        ctr += 1
                        nops.append(
                            mybir.InstNoOp(
                                name=f"I-waitfix-{ctr}",
                                engine=inst.engine,
                                sync_info=mybir.SyncInfo(
                                    on_wait=excess[j : j + cap], on_update=[]
                                ),
                            )
                        )
                    inst.sync_info = mybir.SyncInfo(
                        on_wait=keep, on_update=list(si.on_update or [])
                    )
                    insts[i:i] = nops
                    i += len(nops)
                i += 1
    return ctr


def _emit_3term(nc, ps, w_sb, wr_sb, xs_t, msl, tsl, w_of_pair, x_of_pair):
    """Emit the 24 DoubleRow matmuls of one 3-term-compensated K=2048
    contraction into PSUM tile `ps`.

    xs_t holds 32 K-slabs (2i = x8_i, 2i+1 = xr8_i); w_sb/wr_sb hold 16
    slabs each (W8_i / Wr8_i).  Chain pairing covers x8_i*W8_i, xr8_i*W8_i
    (A instructions) and x8_i*Wr8_i (B instructions) with constant-stride
    slab pairs only.  `w_of_pair(w_tile, s0, s1, msl)` / `x_of_pair(xs, s0,
    s1, tsl)` build the [128, 2, *] APs (orientation differs between the
    q/k and v sweeps).
    """
    seq = []
    # A_1..A_15: x slabs (2j-1, 2j), w slabs (j-1, j)
    for j in range(1, KO):
        seq.append((w_of_pair(w_sb, j - 1, j, msl), x_of_pair(xs_t, 2 * j - 1, 2 * j, tsl)))
    # B_0..B_7: x slabs (4m, 4m+2), wr slabs (2m, 2m+1)
    for m in range(KO // 2):
        seq.append((w_of_pair(wr_sb, 2 * m, 2 * m + 1, msl), x_of_pair(xs_t, 4 * m, 4 * m + 2, tsl)))
    # A_0: x slabs (0, 31), w slabs (0, 15)
    seq.append((w_of_pair(w_sb, 0, KO - 1, msl), x_of_pair(xs_t, 0, 2 * KO - 1, tsl)))
    n = len(seq)
    for i, (w_ap, x_ap) in enumerate(seq):
        nc.tensor.matmul(ps[:], w_ap, x_ap, start=(i == 0), stop=(i == n - 1), perf_mode=DR)


def _slab_pair(t, s0, s1, csl):
    """AP [128, 2, cols] selecting slabs s0 < s1 of a [P, nslab, C] tile."""
    if csl is None:
        return t[:, s0 : s1 + 1 : (s1 - s0), :] if s1 - s0 > 1 else t[:, s0 : s1 + 1, :]
    step = s1 - s0
    if step > 1:
        return t[:, s0 : s1 + 1 : step, csl]
    return t[:, s0 : s1 + 1, csl]


def _rope_block(nc, psum_pool, tmp_pool, qk_sb, cs_sb, jT_sb, h, qr, kr, rb, tag="psj", psj_bufs=2):
    """RoPE for one 512-wide t-block of head h: roped = C*q + S*(J q).
    qr/kr are per-t-block tile lists so consumers only depend on their own
    block's blend, not the whole head."""
    sl = slice(rb * TB, (rb + 1) * TB)
    for si, (src_t, dst) in enumerate(((qk_sb[h], qr[rb]), (qk_sb[HPC + h], kr[rb]))):
        psj = psum_pool.tile([P, TB], F32, tag=tag, name=f"psj{h}_{rb}_{si}", bufs=psj_bufs)
        nc.tensor.matmul(psj[:], jT_sb[:], src_t[:, sl], start=True, stop=True)
        tmp = tmp_pool.tile([P, TB], BF16, tag="ropetmp", name=f"rtmp{h}_{rb}_{si}")
        nc.vector.tensor_tensor(tmp[:], psj[:], cs_sb[:, 1, sl], mybir.AluOpType.mult)
        nc.vector.tensor_tensor(dst[:], src_t[:, sl], cs_sb[:, 0, sl], mybir.AluOpType.mult)
        nc.vector.tensor_tensor(dst[:], dst[:], tmp[:], mybir.AluOpType.add)


def _phase1(nc, tc, xs, wqks, wqkrs, wvs, wvrs, qk_sb, v_sb, rope0, const_dmas):
    with (
        tc.tile_pool(name="p1w", bufs=1) as p1w,
        tc.tile_pool(name="p1x", bufs=2) as p1x,
        tc.tile_pool(name="p1t", bufs=2) as p1t,
        tc.tile_pool(name="p1p", bufs=3, space="PSUM") as p1p,
        tc.tile_pool(name="p1pj", bufs=1, space="PSUM") as p1pj,
    ):
        wqk_t = p1w.tile([P, NQK, KO, P], F8, name="wqks")
        wqkr_t = p1w.tile([P, NQK, KO, P], F8, name="wqkrs")
        wv_t = p1w.tile([P, KO, HPC * HD], F8, name="wvs")
        wvr_t = p1w.tile([P, KO, HPC * HD], F8, name="wvrs")

        # q/k sweep: stationary = weight slab pair, moving = x slab pair
        def w_qk(t, s0, s1, msl):
            return _slab_pair(t, s0, s1, msl)

        def x_qk(t, s0, s1, _):
            return _slab_pair(t, s0, s1, None)

        first = True
        for tb in range(NTB):
            tsl = slice(tb * TB, (tb + 1) * TB)
            xs_t = p1x.tile([P, 2 * KO, TB], F8, tag="xs", name=f"xs{tb}")
            if first:
                # fine-grained first loads so the first m-block's A chain can
                # start after ~1MB instead of ~4MB of DMA
                nc.sync.dma_start(xs_t[:, 0 : KO // 2, :], xs[:, 0 : KO // 2, tsl])
                nc.sync.dma_start(wqk_t[:, 0], wqks[:, 0])
                nc.sync.dma_start(xs_t[:, KO // 2 : KO, :], xs[:, KO // 2 : KO, tsl])
                nc.sync.dma_start(wqk_t[:, 1], wqks[:, 1])
                nc.sync.dma_start(xs_t[:, KO : 2 * KO, :], xs[:, KO : 2 * KO, tsl])
                nc.sync.dma_start(wqkr_t[:, 0:2], wqkrs[:, 0:2])
                nc.sync.dma_start(wqk_t[:, 2:4], wqks[:, 2:4])
                nc.sync.dma_start(wqkr_t[:, 2:4], wqkrs[:, 2:4])
                nc.sync.dma_start(wqk_t[:, 4:8], wqks[:, 4:8])
                nc.sync.dma_start(wqkr_t[:, 4:8], wqkrs[:, 4:8])
                nc.sync.dma_start(wv_t[:], wvs[:])
                nc.sync.dma_start(wvr_t[:], wvrs[:])
                # const loads ride behind the critical phase-1 loads
                for dma in const_dmas:
                    dma()
                first = False
            else:
                nc.sync.dma_start(xs_t[:, 0:KO, :], xs[:, 0:KO, tsl])
                nc.sync.dma_start(xs_t[:, KO : 2 * KO, :], xs[:, KO : 2 * KO, tsl])

            for m in range(NQK):
                def w_qk_m(t, s0, s1, _msl, _m=m):
                    step = s1 - s0
                    if step > 1:
                        return t[:, _m, s0 : s1 + 1 : step, :]
                    return t[:, _m, s0 : s1 + 1, :]

                ps = p1p.tile([P, TB], F32, tag="ps1", name=f"psqk{tb}_{m}")
                _emit_3term(nc, ps, wqk_t, wqkr_t, xs_t, None, None, w_qk_m, x_qk)
                cp = (nc.vector.tensor_copy, nc.scalar.copy)[m % 2]
                cp(qk_sb[m][:, tsl], ps[:])
            for t4 in range(4):
                t4sl = slice(t4 * P, (t4 + 1) * P)
                ps = p1p.tile([P, HPC * HD], F32, tag="ps1", name=f"psv{tb}_{t4}")
                # v: out[t, hd] — stationary x slabs sliced to t4, moving wv
                seq = []
                for j in range(1, KO):
                    seq.append((_slab_pair(xs_t, 2 * j - 1, 2 * j, t4sl), _slab_pair(wv_t, j - 1, j, None)))
                for m2 in range(KO // 2):
                    seq.append((_slab_pair(xs_t, 4 * m2, 4 * m2 + 2, t4sl), _slab_pair(wvr_t, 2 * m2, 2 * m2 + 1, None)))
                seq.append((_slab_pair(xs_t, 0, 2 * KO - 1, t4sl), _slab_pair(wv_t, 0, KO - 1, None)))
                for i, (x_ap, w_ap) in enumerate(seq):
                    nc.tensor.matmul(ps[:], x_ap, w_ap, start=(i == 0), stop=(i == len(seq) - 1), perf_mode=DR)
                cp = (nc.vector.tensor_copy, nc.scalar.copy)[t4 % 2]
                cp(v_sb[tb * 4 + t4][:], ps[:])
            # head-0 rope for this t-block rides inside phase 1 so the DVE
            # blend queue is warm when attention starts
            qr0, kr0, cs_sb, jT_sb = rope0
            _rope_block(nc, p1pj, p1t, qk_sb, cs_sb, jT_sb, 0, qr0, kr0, tb, tag="psj1")


def _phase2(nc, tc, outS, qk_sb, v_sb, jT_sb, mask_sb, ones_sb, cs_sb, r0, emit_p3, aux_pool, p2ps):
    with (
        tc.tile_pool(name="p2r", bufs=2) as p2r,
        tc.tile_pool(name="p2pt", bufs=10) as p2pt,
        tc.tile_pool(name="p2rec", bufs=4) as p2rec,
        tc.tile_pool(name="p2po", bufs=2, space="PSUM") as p2po,
    ):
        def alloc_roped(h):
            qr = [p2r.tile([P, TB], BF16, tag=f"qr{rb}", name=f"qr{h}_{rb}") for rb in range(NTB)]
            kr = [p2r.tile([P, TB], BF16, tag=f"kr{rb}", name=f"kr{h}_{rb}") for rb in range(NTB)]
            return qr, kr

        def attn_tq(h, tq, qr, kr, pending):
            """One q-tile of attention, software-pipelined over PAIRS of
            128-wide k-blocks: the two STs of a pair land in two PSUM banks
            of one tile so a single exp (and, on the diagonal, a single mask
            multiply) covers both.  PV/ones matmuls trail via `pending`."""
            sl = slice(tq * TQ, (tq + 1) * TQ)
            nk = (tq + 1) * BANDS
            # ps_o (PV) and ps_d (denominator) share one 2KB bank: the first
            # PV's start zeroes the whole region, so the denominator chain
            # never carries start (verified region-zero semantics on hw).
            ps_od = p2po.tile([P, 2, TQ], F32, tag="po", name=f"po{h}{tq}")
            ps_o = ps_od[:, 0, :]
            ps_d = ps_od[:, 1, :]

            def issue_pair(kp):
                # both STs of a pair share one 2KB bank: the first carries
                # start (zeroing the region), the second relies on the
                # region-granular pending-zero (verified on hw)
                ps_st = p2ps.tile([P, 2, TQ], F32, tag="st", name=f"st{h}{tq}{kp}")
                qr_t = qr[tq * TQ // TB]
                qsl = slice((tq * TQ) % TB, (tq * TQ) % TB + TQ)
                for j in range(2):
                    kb = 2 * kp + j
                    kr_t = kr[kb * P // TB]
                    ksl = slice((kb * P) % TB, (kb * P) % TB + P)
                    nc.tensor.matmul(
                        ps_st[:, j, :], kr_t[:, ksl], qr_t[:, qsl],
                        start=(j == 0), stop=(j == 1), skip_group_check=True,
                    )
                pt = p2pt.tile([P, 2, TQ], BF16, tag="pt", name=f"pt{h}{tq}{kp}")
                nc.scalar.activation(
                    pt[:], ps_st[:], mybir.ActivationFunctionType.Exp, scale=SCALE_EFF
                )
                if kp == tq:  # diagonal pair: mask both bands at once
                    nc.vector.tensor_tensor(pt[:], pt[:], mask_sb[:], mybir.AluOpType.mult)
                return pt

            def make_pv(kp, pt):
                def pv():
                    for j in range(2):
                        kb = 2 * kp + j
                        nc.tensor.matmul(
                            ps_o, v_sb[kb][:, h * HD : (h + 1) * HD], pt[:, j, :],
                            start=(kb == 0), stop=False, skip_group_check=True,
                        )
                        nc.tensor.matmul(
                            ps_d, ones_sb[:], pt[:, j, :], start=False,
                            stop=(kb == nk - 1), skip_group_check=True,
                        )
                    if 2 * kp + 1 == nk - 1:
                        rec = p2rec.tile([P, TQ], F32, tag="rec", name=f"rec{h}{tq}")
                        nc.vector.reciprocal(rec[:], ps_d)
                        ob = p2rec.tile([P, TQ], BF16, tag="ob", name=f"ob{h}{tq}")
                        nc.vector.tensor_tensor(ob[:], ps_o, rec[:], mybir.AluOpType.mult)
                        hi = outS[tq][:, 2 * h, :]
                        nc.vector.tensor_copy(hi, ob[:])
                        nc.vector.scalar_tensor_tensor(
                            outS[tq][:, 2 * h + 1, :], ob[:], 1.0, hi,
                            mybir.AluOpType.mult, mybir.AluOpType.subtract,
                        )
                return pv

            for kp in range(nk // 2):
                pt = issue_pair(kp)
                if len(pending) >= 3:
                    pending.pop(0)()
                pending.append(make_pv(kp, pt))

        # rope for head h+1 is interleaved into head h's attention (one
        # 512-wide t-block per pair of q-tiles); head 0 was roped inside
        # phase 1.  During the last head, phase-3 tiles are emitted one
        # q-tile behind so output projection overlaps the attention tail.
        roped = [r0]
        pending = []
        for h in range(HPC):
            if h + 1 < HPC:
                roped.append(alloc_roped(h + 1))
            qr, kr = roped[h]
            for tq in range(NTQ):
                attn_tq(h, tq, qr, kr, pending)
                if h + 1 < HPC:
                    if tq % 2 == 0:
                        _rope_block(nc, aux_pool, p2pt, qk_sb, cs_sb, jT_sb,
                                    h + 1, roped[h + 1][0], roped[h + 1][1], tq // 2,
                                    tag="ps3", psj_bufs=3)
                elif tq >= 2:
                    # two q-tiles behind: guarantees head-3's normalization
                    # for tq-2 has been emitted (pending is only 3 pairs deep)
                    emit_p3(tq - 2)
            if h == HPC - 1:
                while pending:
                    pending.pop(0)()
        emit_p3(NTQ - 2)
        emit_p3(NTQ - 1)


def _make_p3(nc, p3s, p3p, outS, wos_sb, wors_sb, y):
    ydescale = 1.0 / (SO * SWO)

    def emit_p3(tq):
        for tt in range(tq * BANDS, (tq + 1) * BANDS):
            off = (tt - tq * BANDS) * P
            osl = slice(off, off + P)
            ysb = p3s.tile([P, D], BF16, tag="ysb", name=f"ysb{tt}")
            last = tq == NTQ - 1
            for dd in range(D // TB):
                dsl = slice(dd * TB, (dd + 1) * TB)
                ps = p3p.tile([P, TB], F32, tag="ps3", name=f"ps3{tt}{dd}")
                seq = []
                for j in range(1, HPC):
                    seq.append((outS[tq][:, 2 * j - 1 : 2 * j + 1, osl], wos_sb[:, j - 1 : j + 1, dsl]))
                for m in range(HPC // 2):
                    seq.append((outS[tq][:, 4 * m : 4 * m + 3 : 2, osl], wors_sb[:, 2 * m : 2 * m + 2, dsl]))
                seq.append((outS[tq][:, 0 : 2 * HPC : 2 * HPC - 1, osl], wos_sb[:, 0 : HPC : HPC - 1, dsl]))
                for i, (o_ap, w_ap) in enumerate(seq):
                    nc.tensor.matmul(ps[:], o_ap, w_ap, start=(i == 0), stop=(i == len(seq) - 1), perf_mode=DR)
                if dd % 2 == 0:
                    nc.vector.tensor_scalar_mul(ysb[:, dsl], ps[:], ydescale)
                else:
                    nc.scalar.mul(ysb[:, dsl], ps[:], ydescale)
                if last:
                    # small per-dd stores shrink the end-of-kernel DMA tail
                    nc.sync.dma_start(
                        y[tt * P : (tt + 1) * P, dd * TB : (dd + 1) * TB],
                        ysb[:, dd * TB : (dd + 1) * TB],
                    )
            if not last:
                nc.sync.dma_start(y[tt * P : (tt + 1) * P, :], ysb[:])
    return emit_p3


def _build_program():
    nc = bass.Bass()

    xs = nc.dram_tensor("xs", (P, 2 * KO, T), F8, kind="ExternalInput")
    wqks = nc.dram_tensor("wqks", (P, NQK, KO, P), F8, kind="ExternalInput")
    wqkrs = nc.dram_tensor("wqkrs", (P, NQK, KO, P), F8, kind="ExternalInput")
    wvs = nc.dram_tensor("wvs", (P, KO, HPC * HD), F8, kind="ExternalInput")
    wvrs = nc.dram_tensor("wvrs", (P, KO, HPC * HD), F8, kind="ExternalInput")
    wos = nc.dram_tensor("wos", (P, HPC, D), F8, kind="ExternalInput")
    wors = nc.dram_tensor("wors", (P, HPC, D), F8, kind="ExternalInput")
    cs = nc.dram_tensor("cs", (P, 2, T), BF16, kind="ExternalInput")
    masks = nc.dram_tensor("masks", (BANDS, P, TQ), BF16, kind="ExternalInput")
    jT = nc.dram_tensor("jT", (P, P), BF16, kind="ExternalInput")
    ones = nc.dram_tensor("ones", (P, P), BF16, kind="ExternalInput")
    y = nc.dram_tensor("y", (T, D), BF16, kind="ExternalOutput")

    with tile.TileContext(nc) as tc:
        with (
            tc.tile_pool(name="consts", bufs=1) as consts,
            tc.tile_pool(name="qkv", bufs=1) as qkvp,
            tc.tile_pool(name="p2ps", bufs=3, space="PSUM") as p2ps,
        ):
            jT_sb = consts.tile([P, P], BF16)
            mask_sb = consts.tile([P, BANDS, TQ], BF16)
            ones_sb = consts.tile([P, P], BF16)
            cs_sb = consts.tile([P, 2, T], BF16)
            wos_sb = consts.tile([P, HPC, D], F8)
            wors_sb = consts.tile([P, HPC, D], F8)
            const_dmas = [
                lambda: nc.sync.dma_start(cs_sb[:], cs[:]),
                lambda: nc.sync.dma_start(jT_sb[:], jT[:]),
                lambda: nc.sync.dma_start(ones_sb[:], ones[:]),
                lambda: nc.sync.dma_start(mask_sb[:], masks.rearrange("a p j -> p a j")),
                lambda: nc.sync.dma_start(wos_sb[:], wos[:]),
                lambda: nc.sync.dma_start(wors_sb[:], wors[:]),
            ]

            qk_sb = [qkvp.tile([P, T], BF16, name=f"qk{m}") for m in range(NQK)]
            v_sb = [qkvp.tile([P, HPC * HD], BF16, name=f"v{kb}") for kb in range(T // P)]
            qr0 = [qkvp.tile([P, TB], BF16, name=f"qr0_{rb}") for rb in range(NTB)]
            kr0 = [qkvp.tile([P, TB], BF16, name=f"kr0_{rb}") for rb in range(NTB)]

            _phase1(nc, tc, xs, wqks, wqkrs, wvs, wvrs, qk_sb, v_sb,
                    (qr0, kr0, cs_sb, jT_sb), const_dmas)

            with (
                tc.tile_pool(name="outT", bufs=1) as outT_pool,
                tc.tile_pool(name="p3s", bufs=3) as p3s,
                tc.tile_pool(name="p3p", bufs=3, space="PSUM") as p3p,
            ):
                outS = {
                    tq: outT_pool.tile([P, 2 * HPC, TQ], F8, tag=f"outS{tq}", name=f"outS{tq}")
                    for tq in range(NTQ)
                }
                emit_p3 = _make_p3(nc, p3s, p3p, outS, wos_sb, wors_sb, y)
                _phase2(nc, tc, outS, qk_sb, v_sb, jT_sb, mask_sb, ones_sb, cs_sb,
                        (qr0, kr0), emit_p3, p3p, p2ps)

    _fix_waits(nc)
    return nc


_NC_CACHE = None


def _get_program():
    global _NC_CACHE
    if _NC_CACHE is None:
        _NC_CACHE = _build_program()
    return _NC_CACHE


def _q8(a, s):
    """e4m3-quantize a*s (clipped to TRN e4m3 range); returns (fp8, residual
    fp8) with the residual on the same scale (no prescale — its values live
    in e4m3's normal range already)."""
    import ml_dtypes

    F8np = ml_dtypes.float8_e4m3
    scaled = np.clip(a * s, -240.0, 240.0)
    hi = scaled.astype(F8np)
    lo = np.clip(scaled - hi.astype(np.float32), -240.0, 240.0).astype(F8np)
    return hi, lo


def _pack_k(a):
    """[K, M] -> [P, KO', M] with slab i on partitions (rows 128i+p)."""
    ko = a.shape[0] // P
    return np.ascontiguousarray(a.reshape(ko, P, a.shape[1]).transpose(1, 0, 2))


def _host_inputs(x, Wqkv, Wout, cos, sin, rope_mask):
    import ml_dtypes

    BF = ml_dtypes.bfloat16
    x = np.asarray(x, dtype=np.float32)
    Wqkv = np.asarray(Wqkv, dtype=np.float32)
    Wout = np.asarray(Wout, dtype=np.float32)
    cos = np.asarray(cos, dtype=np.float32)
    sin = np.asarray(sin, dtype=np.float32)
    rope_mask = np.asarray(rope_mask).astype(bool)

    # J^T for the pair-rotation matmul: (J q)[2i] = -q[2i+1], (J q)[2i+1] = q[2i]
    jT = np.zeros((P, P), dtype=np.float32)
    for i in range(P // 2):
        jT[2 * i, 2 * i + 1] = 1.0
        jT[2 * i + 1, 2 * i] = -1.0

    masks = np.zeros((BANDS, P, TQ), dtype=BF)
    ii = np.arange(P)[:, None]
    jj = np.arange(TQ)[None, :]
    for a in range(BANDS):
        masks[a] = (ii + a * P <= jj).astype(BF)

    C_full = np.repeat(cos[:T].T, 2, axis=0).astype(np.float32)  # [128, T]
    S_full = np.repeat(sin[:T].T, 2, axis=0).astype(np.float32)

    # per-batch x packs (shared by the 4 cores of each batch)
    xs_b = []
    for b in range(B):
        x8, xr8 = _q8(x[b].T, SX)  # [D, T] fp8
        xsp = np.empty((P, 2 * KO, T), dtype=x8.dtype)
        xsp[:, 0::2] = _pack_k(x8)
        xsp[:, 1::2] = _pack_k(xr8)
        xs_b.append(xsp)

    in_maps = []
    for c in range(N_CORES):
        b = c // CORES_PER_B
        hg = c % CORES_PER_B
        heads = [hg * HPC + i for i in range(HPC)]

        qrows = np.concatenate([np.arange(h * HD, (h + 1) * HD) for h in heads])
        krows = qrows + D
        vrows = qrows + 2 * D
        wqk = Wqkv[np.concatenate([qrows, krows])].T  # [D, 1024]
        wv = Wqkv[vrows].T                            # [D, 512]
        wqk8, wqkr8 = _q8(wqk, SW)
        wv8, wvr8 = _q8(wv, SW)

        def pack_q(a):  # [P, KO, 1024] -> [P, NQK, KO, 128] m-major
            pk = _pack_k(a)
            return np.ascontiguousarray(
                pk.reshape(P, KO, NQK, P).transpose(0, 2, 1, 3)
            )

        woT = np.ascontiguousarray(Wout[:, qrows].T)  # [512, D]
        wo8, wor8 = _q8(woT, SWO)
        wos_p = np.ascontiguousarray(wo8.reshape(HPC, P, D).transpose(1, 0, 2))
        wors_p = np.ascontiguousarray(wor8.reshape(HPC, P, D).transpose(1, 0, 2))

        flags = [bool(rope_mask[h]) for h in heads]
        assert all(f == flags[0] for f in flags), (
            "heads in one core must share a rope flag for the single-table path"
        )
        cs_arr = np.empty((P, 2, T), dtype=BF)
        if flags[0]:
            cs_arr[:, 0] = C_full.astype(BF)
            cs_arr[:, 1] = S_full.astype(BF)
        else:
            cs_arr[:, 0] = np.ones((P, T), dtype=BF)
            cs_arr[:, 1] = np.zeros((P, T), dtype=BF)

        in_maps.append(
            {
                "xs": xs_b[b],
                "wqks": pack_q(wqk8),
                "wqkrs": pack_q(wqkr8),
                "wvs": _pack_k(wv8),
                "wvrs": _pack_k(wvr8),
                "wos": wos_p,
                "wors": wors_p,
                "cs": cs_arr,
                "masks": masks,
                "jT": jT.astype(BF),
                "ones": np.full((P, P), SIGMA / SO, dtype=BF),
            }
        )
    return in_maps


def kernel(x, Wqkv, Wout, cos, sin, rope_mask, _trace=False):
    nc = _get_program()
    in_maps = _host_inputs(x, Wqkv, Wout, cos, sin, rope_mask)
    res = run_bass_kernel_spmd(nc, in_maps, core_ids=list(range(N_CORES)), trace=_trace)
    parts = [np.asarray(res.results[c]["y"], dtype=np.float32) for c in range(N_CORES)]
    out = np.stack(
        [sum(parts[b * CORES_PER_B : (b + 1) * CORES_PER_B]) for b in range(B)]
    ).astype(np.float32)
    if _trace:
        kernel.last_result = res
    return out


# revision 35
# speedup vs baseline: 1.3376x; 1.0014x over previous
"""Causal self-attention (B=2, T=2048, D=2048, H=16, hd=128, RoPE on masked
heads) as a Bass/Tile kernel on 8 Trainium2 NeuronCores.

Sharding: core c handles batch b=c//4 and heads 4*(c%4)..4*(c%4)+3 (data
parallel on B x tensor parallel on H).  Each core computes a partial output
projection y_b = O_local @ Wout_local^T; the host sums the 4 partials per
batch.

Numerics/performance strategy:
- QKV projection runs as fp8(e4m3) DoubleRow matmuls with 3-term residual
  compensation: x*W ~ x8*W8 + xr8*W8 + x8*Wr8, where xr8/Wr8 are e4m3
  quantizations of the quantization residuals (host-prepared).  Each
  DoubleRow instruction contracts two 128-row K-slabs at half cost, so the
  projection runs at 1.5x the bf16 matmul rate with ~1e-3 relative error.
  The 3 terms are packed into 24 DoubleRow instructions per output tile via
  a chain pairing that needs no operand duplication (see _emit_3term).
- Attention (scores, softmax, PV, denominator) runs in bf16: S^T = K Q^T in
  transposed score space so softmax normalization is a per-free-element
  multiply; denominator via a constant-value stationary matmul whose value
  folds the fp8 descale so normalized outputs land in e4m3 range.  Score
  tiles are computed in PAIRS sharing one PSUM bank (only the first matmul
  carries start; region-granular zeroing verified on hw) so one exp covers
  two k-blocks, keeping the Activation engine off the critical path.
- The output projection also runs as 3-term fp8 DoubleRow: the normalize
  step emits an e4m3 hi part plus an exact residual lo part (one extra DVE
  subtract per tile), contracted against host-prepared Wout hi/lo slabs.
- q, k, v stay resident in SBUF between phases (bf16) - no DRAM scratch;
  y is written as bf16 partials and summed in f32 on the host.
- RoPE tables are a single per-core C/S pair (identity for NoPE cores);
  roped = C*q + S*(J q) with J applied as a PE matmul; head-0 rope is
  folded into phase 1 and phase-3 tiles are interleaved into the last
  head's attention so the PE pipeline never drains between phases.
"""

import sys

sys.path.insert(0, "/opt/trn_rl_repo")

import numpy as np

import concourse.bass as bass
import concourse.mybir as mybir
import concourse.tile as tile
from concourse.bass_utils import run_bass_kernel_spmd

F32 = mybir.dt.float32
F8 = mybir.dt.float8e4
BF16 = mybir.dt.bfloat16
DR = mybir.MatmulPerfMode.DoubleRow

B = 2
T = 2048
D = 2048
H = 16
HD = 128
N_CORES = 8
HPC = 4           # heads per core
CORES_PER_B = 4
P = 128
TB = 512          # t-block width (phase 1 / rope)
NTB = T // TB     # 4
TQ = 256          # attention q-tile width (phase 2)
NTQ = T // TQ     # 8
BANDS = TQ // P   # 2
KO = D // P       # 16 contraction K-blocks of 128
NQK = 2 * HPC     # 8 q+k dout blocks of 128
SX = 16.0         # fp8 scale for x
SW = 1024.0       # fp8 scale for Wqkv
SWO = 1024.0      # fp8 scale for Wout
SO = 32.0         # fp8 scale carried by the normalized attention output
SIGMA = SX * SW   # scale carried by q,k,v in SBUF
SCALE_EFF = (1.0 / float(np.sqrt(HD))) / (SIGMA * SIGMA)


# ---------------------------------------------------------------------------
# Walrus on this toolchain rejects instructions carrying more than one sync
# wait command; Tile can emit several (e.g. the kernel-tail drain).  Hoist
# the excess onto injected same-engine NoOps — semantically identical.
def _fix_waits(nc, cap=1):
    ctr = 0
    for f in nc.m.functions:
        for bb in f.blocks:
            insts = bb.instructions
            i = 0
            while i < len(insts):
                inst = insts[i]
                si = inst.sync_info
                if si is not None and si.on_wait and len(si.on_wait) > cap:
                    waits = list(si.on_wait)
                    keep, excess = waits[:cap], waits[cap:]
                    nops = []
                    for j in range(0, len(excess), cap):
                        ctr += 1
                        nops.append(
                            mybir.InstNoOp(
                                name=f"I-waitfix-{ctr}",
                                engine=inst.engine,
                                sync_info=mybir.SyncInfo(
                                    on_wait=excess[j : j + cap], on_update=[]
                                ),
                            )
                        )
                    inst.sync_info = mybir.SyncInfo(
                        on_wait=keep, on_update=list(si.on_update or [])
                    )
                    insts[i:i] = nops
                    i += len(nops)
                i += 1
    return ctr


def _emit_3term(nc, ps, w_sb, wr_sb, xs_t, msl, tsl, w_of_pair, x_of_pair):
    """Emit the 24 DoubleRow matmuls of one 3-term-compensated K=2048
    contraction into PSUM tile `ps`.

    xs_t holds 32 K-slabs (2i = x8_i, 2i+1 = xr8_i); w_sb/wr_sb hold 16
    slabs each (W8_i / Wr8_i).  Chain pairing covers x8_i*W8_i, xr8_i*W8_i
    (A instructions) and x8_i*Wr8_i (B instructions) with constant-stride
    slab pairs only.  `w_of_pair(w_tile, s0, s1, msl)` / `x_of_pair(xs, s0,
    s1, tsl)` build the [128, 2, *] APs (orientation differs between the
    q/k and v sweeps).
    """
    seq = []
    # A_1..A_15: x slabs (2j-1, 2j), w slabs (j-1, j)
    for j in range(1, KO):
        seq.append((w_of_pair(w_sb, j - 1, j, msl), x_of_pair(xs_t, 2 * j - 1, 2 * j, tsl)))
    # B_0..B_7: x slabs (4m, 4m+2), wr slabs (2m, 2m+1)
    for m in range(KO // 2):
        seq.append((w_of_pair(wr_sb, 2 * m, 2 * m + 1, msl), x_of_pair(xs_t, 4 * m, 4 * m + 2, tsl)))
    # A_0: x slabs (0, 31), w slabs (0, 15)
    seq.append((w_of_pair(w_sb, 0, KO - 1, msl), x_of_pair(xs_t, 0, 2 * KO - 1, tsl)))
    n = len(seq)
    for i, (w_ap, x_ap) in enumerate(seq):
        nc.tensor.matmul(ps[:], w_ap, x_ap, start=(i == 0), stop=(i == n - 1), perf_mode=DR)


def _slab_pair(t, s0, s1, csl):
    """AP [128, 2, cols] selecting slabs s0 < s1 of a [P, nslab, C] tile."""
    if csl is None:
        return t[:, s0 : s1 + 1 : (s1 - s0), :] if s1 - s0 > 1 else t[:, s0 : s1 + 1, :]
    step = s1 - s0
    if step > 1:
        return t[:, s0 : s1 + 1 : step, csl]
    return t[:, s0 : s1 + 1, csl]


def _rope_block(nc, psum_pool, tmp_pool, qk_sb, cs_sb, jT_sb, h, qr, kr, rb, tag="psj", psj_bufs=2):
    """RoPE for one 512-wide t-block of head h: roped = C*q + S*(J q).
    qr/kr are per-t-block tile lists so consumers only depend on their own
    block's blend, not the whole head."""
    sl = slice(rb * TB, (rb + 1) * TB)
    for si, (src_t, dst) in enumerate(((qk_sb[h], qr[rb]), (qk_sb[HPC + h], kr[rb]))):
        psj = psum_pool.tile([P, TB], F32, tag=tag, name=f"psj{h}_{rb}_{si}", bufs=psj_bufs)
        nc.tensor.matmul(psj[:], jT_sb[:], src_t[:, sl], start=True, stop=True)
        tmp = tmp_pool.tile([P, TB], BF16, tag="ropetmp", name=f"rtmp{h}_{rb}_{si}")
        nc.vector.tensor_tensor(tmp[:], psj[:], cs_sb[:, 1, sl], mybir.AluOpType.mult)
        nc.vector.tensor_tensor(dst[:], src_t[:, sl], cs_sb[:, 0, sl], mybir.AluOpType.mult)
        nc.vector.tensor_tensor(dst[:], dst[:], tmp[:], mybir.AluOpType.add)


def _phase1(nc, tc, xs, wqks, wqkrs, wvs, wvrs, qk_sb, v_sb, rope0, const_dmas):
    with (
        tc.tile_pool(name="p1w", bufs=1) as p1w,
        tc.tile_pool(name="p1x", bufs=2) as p1x,
        tc.tile_pool(name="p1t", bufs=2) as p1t,
        tc.tile_pool(name="p1p", bufs=3, space="PSUM") as p1p,
        tc.tile_pool(name="p1pj", bufs=1, space="PSUM") as p1pj,
    ):
        wqk_t = p1w.tile([P, NQK, KO, P], F8, name="wqks")
        wqkr_t = p1w.tile([P, NQK, KO, P], F8, name="wqkrs")
        wv_t = p1w.tile([P, KO, HPC * HD], F8, name="wvs")
        wvr_t = p1w.tile([P, KO, HPC * HD], F8, name="wvrs")

        # q/k sweep: stationary = weight slab pair, moving = x slab pair
        def w_qk(t, s0, s1, msl):
            return _slab_pair(t, s0, s1, msl)

        def x_qk(t, s0, s1, _):
            return _slab_pair(t, s0, s1, None)

        first = True
        for tb in range(NTB):
            tsl = slice(tb * TB, (tb + 1) * TB)
            xs_t = p1x.tile([P, 2 * KO, TB], F8, tag="xs", name=f"xs{tb}")
            if first:
                # fine-grained first loads so the first m-block's A chain can
                # start after ~1MB instead of ~4MB of DMA
                nc.sync.dma_start(xs_t[:, 0 : KO // 2, :], xs[:, 0 : KO // 2, tsl])
                nc.sync.dma_start(wqk_t[:, 0], wqks[:, 0])
                nc.sync.dma_start(xs_t[:, KO // 2 : KO, :], xs[:, KO // 2 : KO, tsl])
                nc.sync.dma_start(wqk_t[:, 1], wqks[:, 1])
                nc.sync.dma_start(xs_t[:, KO : 2 * KO, :], xs[:, KO : 2 * KO, tsl])
                nc.sync.dma_start(wqkr_t[:, 0:2], wqkrs[:, 0:2])
                nc.sync.dma_start(wqk_t[:, 2:4], wqks[:, 2:4])
                nc.sync.dma_start(wqkr_t[:, 2:4], wqkrs[:, 2:4])
                nc.sync.dma_start(wqk_t[:, 4:8], wqks[:, 4:8])
                nc.sync.dma_start(wqkr_t[:, 4:8], wqkrs[:, 4:8])
                nc.sync.dma_start(wv_t[:], wvs[:])
                nc.sync.dma_start(wvr_t[:], wvrs[:])
                # const loads ride behind the critical phase-1 loads
                for dma in const_dmas:
                    dma()
                first = False
            else:
                nc.sync.dma_start(xs_t[:, 0:KO, :], xs[:, 0:KO, tsl])
                nc.sync.dma_start(xs_t[:, KO : 2 * KO, :], xs[:, KO : 2 * KO, tsl])

            for m in range(NQK):
                def w_qk_m(t, s0, s1, _msl, _m=m):
                    step = s1 - s0
                    if step > 1:
                        return t[:, _m, s0 : s1 + 1 : step, :]
                    return t[:, _m, s0 : s1 + 1, :]

                ps = p1p.tile([P, TB], F32, tag="ps1", name=f"psqk{tb}_{m}")
                _emit_3term(nc, ps, wqk_t, wqkr_t, xs_t, None, None, w_qk_m, x_qk)
                cp = (nc.vector.tensor_copy, nc.scalar.copy)[m % 2]
                cp(qk_sb[m][:, tsl], ps[:])
            for t4 in range(4):
                t4sl = slice(t4 * P, (t4 + 1) * P)
                ps = p1p.tile([P, HPC * HD], F32, tag="ps1", name=f"psv{tb}_{t4}")
                # v: out[t, hd] — stationary x slabs sliced to t4, moving wv
                seq = []
                for j in range(1, KO):
                    seq.append((_slab_pair(xs_t, 2 * j - 1, 2 * j, t4sl), _slab_pair(wv_t, j - 1, j, None)))
                for m2 in range(KO // 2):
                    seq.append((_slab_pair(xs_t, 4 * m2, 4 * m2 + 2, t4sl), _slab_pair(wvr_t, 2 * m2, 2 * m2 + 1, None)))
                seq.append((_slab_pair(xs_t, 0, 2 * KO - 1, t4sl), _slab_pair(wv_t, 0, KO - 1, None)))
                for i, (x_ap, w_ap) in enumerate(seq):
                    nc.tensor.matmul(ps[:], x_ap, w_ap, start=(i == 0), stop=(i == len(seq) - 1), perf_mode=DR)
                cp = (nc.vector.tensor_copy, nc.scalar.copy)[t4 % 2]
                cp(v_sb[tb * 4 + t4][:], ps[:])
            # head-0 rope for this t-block rides inside phase 1 so the DVE
            # blend queue is warm when attention starts
            qr0, kr0, cs_sb, jT_sb = rope0
            _rope_block(nc, p1pj, p1t, qk_sb, cs_sb, jT_sb, 0, qr0, kr0, tb, tag="psj1")


def _phase2(nc, tc, outS, qk_sb, v_sb, jT_sb, mask_sb, ones_sb, cs_sb, r0, emit_p3, aux_pool, p2ps):
    with (
        tc.tile_pool(name="p2r", bufs=2) as p2r,
        tc.tile_pool(name="p2pt", bufs=10) as p2pt,
        tc.tile_pool(name="p2rec", bufs=4) as p2rec,
        tc.tile_pool(name="p2po", bufs=2, space="PSUM") as p2po,
    ):
        def alloc_roped(h):
            qr = [p2r.tile([P, TB], BF16, tag=f"qr{rb}", name=f"qr{h}_{rb}") for rb in range(NTB)]
            kr = [p2r.tile([P, TB], BF16, tag=f"kr{rb}", name=f"kr{h}_{rb}") for rb in range(NTB)]
            return qr, kr

        def attn_tq(h, tq, qr, kr, pending):
            """One q-tile of attention, software-pipelined over PAIRS of
            128-wide k-blocks: the two STs of a pair land in two PSUM banks
            of one tile so a single exp (and, on the diagonal, a single mask
            multiply) covers both.  PV/ones matmuls trail via `pending`."""
            sl = slice(tq * TQ, (tq + 1) * TQ)
            nk = (tq + 1) * BANDS
            # ps_o (PV) and ps_d (denominator) share one 2KB bank: the first
            # PV's start zeroes the whole region, so the denominator chain
            # never carries start (verified region-zero semantics on hw).
            ps_od = p2po.tile([P, 2, TQ], F32, tag="po", name=f"po{h}{tq}")
            ps_o = ps_od[:, 0, :]
            ps_d = ps_od[:, 1, :]

            def issue_pair(kp):
                # both STs of a pair share one 2KB bank: the first carries
                # start (zeroing the region), the second relies on the
                # region-granular pending-zero (verified on hw)
                ps_st = p2ps.tile([P, 2, TQ], F32, tag="st", name=f"st{h}{tq}{kp}")
                qr_t = qr[tq * TQ // TB]
                qsl = slice((tq * TQ) % TB, (tq * TQ) % TB + TQ)
                for j in range(2):
                    kb = 2 * kp + j
                    kr_t = kr[kb * P // TB]
                    ksl = slice((kb * P) % TB, (kb * P) % TB + P)
                    nc.tensor.matmul(
                        ps_st[:, j, :], kr_t[:, ksl], qr_t[:, qsl],
                        start=(j == 0), stop=(j == 1), skip_group_check=True,
                    )
                pt = p2pt.tile([P, 2, TQ], BF16, tag="pt", name=f"pt{h}{tq}{kp}")
                nc.scalar.activation(
                    pt[:], ps_st[:], mybir.ActivationFunctionType.Exp, scale=SCALE_EFF
                )
                if kp == tq:  # diagonal pair: mask both bands at once
                    nc.vector.tensor_tensor(pt[:], pt[:], mask_sb[:], mybir.AluOpType.mult)
                return pt

            def make_pv(kp, pt):
                def pv():
                    for j in range(2):
                        kb = 2 * kp + j
                        nc.tensor.matmul(
                            ps_o, v_sb[kb][:, h * HD : (h + 1) * HD], pt[:, j, :],
                            start=(kb == 0), stop=False, skip_group_check=True,
                        )
                        nc.tensor.matmul(
                            ps_d, ones_sb[:], pt[:, j, :], start=False,
                            stop=(kb == nk - 1), skip_group_check=True,
                        )
                    if 2 * kp + 1 == nk - 1:
                        rec = p2rec.tile([P, TQ], F32, tag="rec", name=f"rec{h}{tq}")
                        nc.vector.reciprocal(rec[:], ps_d)
                        ob = p2rec.tile([P, TQ], BF16, tag="ob", name=f"ob{h}{tq}")
                        nc.vector.tensor_tensor(ob[:], ps_o, rec[:], mybir.AluOpType.mult)
                        hi = outS[tq][:, 2 * h, :]
                        nc.vector.tensor_copy(hi, ob[:])
                        nc.vector.scalar_tensor_tensor(
                            outS[tq][:, 2 * h + 1, :], ob[:], 1.0, hi,
                            mybir.AluOpType.mult, mybir.AluOpType.subtract,
                        )
                return pv

            for kp in range(nk // 2):
                pt = issue_pair(kp)
                if len(pending) >= 3:
                    pending.pop(0)()
                pending.append(make_pv(kp, pt))

        # rope for head h+1 is interleaved into head h's attention (one
        # 512-wide t-block per pair of q-tiles); head 0 was roped inside
        # phase 1.  During the last head, phase-3 tiles are emitted one
        # q-tile behind so output projection overlaps the attention tail.
        roped = [r0]
        pending = []
        for h in range(HPC):
            if h + 1 < HPC:
                roped.append(alloc_roped(h + 1))
            qr, kr = roped[h]
            for tq in range(NTQ):
                attn_tq(h, tq, qr, kr, pending)
                if h + 1 < HPC:
                    if tq % 2 == 0:
                        _rope_block(nc, aux_pool, p2pt, qk_sb, cs_sb, jT_sb,
                                    h + 1, roped[h + 1][0], roped[h + 1][1], tq // 2,
                                    tag="ps3", psj_bufs=3)
                elif tq >= 2:
                    # two q-tiles behind: guarantees head-3's normalization
                    # for tq-2 has been emitted (pending is only 3 pairs deep)
                    emit_p3(tq - 2)
            if h == HPC - 1:
                while pending:
                    pending.pop(0)()
        emit_p3(NTQ - 2)
        emit_p3(NTQ - 1)


def _make_p3(nc, p3s, p3p, outS, wos_sb, wors_sb, y):
    ydescale = 1.0 / (SO * SWO)

    def emit_p3(tq):
        for tt in range(tq * BANDS, (tq + 1) * BANDS):
            off = (tt - tq * BANDS) * P
            osl = slice(off, off + P)
            ysb = p3s.tile([P, D], BF16, tag="ysb", name=f"ysb{tt}")
            last = tq == NTQ - 1
            for dd in range(D // TB):
                dsl = slice(dd * TB, (dd + 1) * TB)
                ps = p3p.tile([P, TB], F32, tag="ps3", name=f"ps3{tt}{dd}")
                seq = []
                for j in range(1, HPC):
                    seq.append((outS[tq][:, 2 * j - 1 : 2 * j + 1, osl], wos_sb[:, j - 1 : j + 1, dsl]))
                for m in range(HPC // 2):
                    seq.append((outS[tq][:, 4 * m : 4 * m + 3 : 2, osl], wors_sb[:, 2 * m : 2 * m + 2, dsl]))
                seq.append((outS[tq][:, 0 : 2 * HPC : 2 * HPC - 1, osl], wos_sb[:, 0 : HPC : HPC - 1, dsl]))
                for i, (o_ap, w_ap) in enumerate(seq):
                    nc.tensor.matmul(ps[:], o_ap, w_ap, start=(i == 0), stop=(i == len(seq) - 1), perf_mode=DR)
                if dd % 2 == 0:
                    nc.vector.tensor_scalar_mul(ysb[:, dsl], ps[:], ydescale)
                else:
                    nc.scalar.mul(ysb[:, dsl], ps[:], ydescale)
                if last:
                    # small per-dd stores shrink the end-of-kernel DMA tail
                    nc.sync.dma_start(
                        y[tt * P : (tt + 1) * P, dd * TB : (dd + 1) * TB],
                        ysb[:, dd * TB : (dd + 1) * TB],
                    )
            if not last:
                nc.sync.dma_start(y[tt * P : (tt + 1) * P, :], ysb[:])
    return emit_p3


def _build_program():
    nc = bass.Bass()

    xs = nc.dram_tensor("xs", (P, 2 * KO, T), F8, kind="ExternalInput")
    wqks = nc.dram_tensor("wqks", (P, NQK, KO, P), F8, kind="ExternalInput")
    wqkrs = nc.dram_tensor("wqkrs", (P, NQK, KO, P), F8, kind="ExternalInput")
    wvs = nc.dram_tensor("wvs", (P, KO, HPC * HD), F8, kind="ExternalInput")
    wvrs = nc.dram_tensor("wvrs", (P, KO, HPC * HD), F8, kind="ExternalInput")
    wos = nc.dram_tensor("wos", (P, HPC, D), F8, kind="ExternalInput")
    wors = nc.dram_tensor("wors", (P, HPC, D), F8, kind="ExternalInput")
    cs = nc.dram_tensor("cs", (P, 2, T), BF16, kind="ExternalInput")
    masks = nc.dram_tensor("masks", (BANDS, P, TQ), BF16, kind="ExternalInput")
    jT = nc.dram_tensor("jT", (P, P), BF16, kind="ExternalInput")
    ones = nc.dram_tensor("ones", (P, P), BF16, kind="ExternalInput")
    y = nc.dram_tensor("y", (T, D), BF16, kind="ExternalOutput")

    with tile.TileContext(nc) as tc:
        with (
            tc.tile_pool(name="consts", bufs=1) as consts,
            tc.tile_pool(name="qkv", bufs=1) as qkvp,
            tc.tile_pool(name="p2ps", bufs=3, space="PSUM") as p2ps,
        ):
            jT_sb = consts.tile([P, P], BF16)
            mask_sb = consts.tile([P, BANDS, TQ], BF16)
            ones_sb = consts.tile([P, P], BF16)
            cs_sb = consts.tile([P, 2, T], BF16)
            wos_sb = consts.tile([P, HPC, D], F8)
            wors_sb = consts.tile([P, HPC, D], F8)
            const_dmas = [
                lambda: nc.sync.dma_start(cs_sb[:], cs[:]),
                lambda: nc.sync.dma_start(jT_sb[:], jT[:]),
                lambda: nc.sync.dma_start(ones_sb[:], ones[:]),
                lambda: nc.sync.dma_start(mask_sb[:], masks.rearrange("a p j -> p a j")),
                lambda: nc.sync.dma_start(wos_sb[:], wos[:]),
                lambda: nc.sync.dma_start(wors_sb[:], wors[:]),
            ]

            qk_sb = [qkvp.tile([P, T], BF16, name=f"qk{m}") for m in range(NQK)]
            v_sb = [qkvp.tile([P, HPC * HD], BF16, name=f"v{kb}") for kb in range(T // P)]
            qr0 = [qkvp.tile([P, TB], BF16, name=f"qr0_{rb}") for rb in range(NTB)]
            kr0 = [qkvp.tile([P, TB], BF16, name=f"kr0_{rb}") for rb in range(NTB)]

            _phase1(nc, tc, xs, wqks, wqkrs, wvs, wvrs, qk_sb, v_sb,
                    (qr0, kr0, cs_sb, jT_sb), const_dmas)

            with (
                tc.tile_pool(name="outT", bufs=1) as outT_pool,
                tc.tile_pool(name="p3s", bufs=3) as p3s,
                tc.tile_pool(name="p3p", bufs=3, space="PSUM") as p3p,
            ):
                outS = {
                    tq: outT_pool.tile([P, 2 * HPC, TQ], F8, tag=f"outS{tq}", name=f"outS{tq}")
                    for tq in range(NTQ)
                }
                emit_p3 = _make_p3(nc, p3s, p3p, outS, wos_sb, wors_sb, y)
                _phase2(nc, tc, outS, qk_sb, v_sb, jT_sb, mask_sb, ones_sb, cs_sb,
                        (qr0, kr0), emit_p3, p3p, p2ps)

    _fix_waits(nc)
    return nc


_NC_CACHE = None


def _get_program():
    global _NC_CACHE
    if _NC_CACHE is None:
        _NC_CACHE = _build_program()
    return _NC_CACHE


def _q8(a, s):
    """e4m3-quantize a*s (clipped to TRN e4m3 range); returns (fp8, residual
    fp8) with the residual on the same scale (no prescale — its values live
    in e4m3's normal range already)."""
    import ml_dtypes

    F8np = ml_dtypes.float8_e4m3
    scaled = np.clip(a * s, -240.0, 240.0)
    hi = scaled.astype(F8np)
    lo = np.clip(scaled - hi.astype(np.float32), -240.0, 240.0).astype(F8np)
    return hi, lo


def _pack_k(a):
    """[K, M] -> [P, KO', M] with slab i on partitions (rows 128i+p)."""
    ko = a.shape[0] // P
    return np.ascontiguousarray(a.reshape(ko, P, a.shape[1]).transpose(1, 0, 2))


def _host_inputs(x, Wqkv, Wout, cos, sin, rope_mask):
    import ml_dtypes

    BF = ml_dtypes.bfloat16
    x = np.asarray(x, dtype=np.float32)
    Wqkv = np.asarray(Wqkv, dtype=np.float32)
    Wout = np.asarray(Wout, dtype=np.float32)
    cos = np.asarray(cos, dtype=np.float32)
    sin = np.asarray(sin, dtype=np.float32)
    rope_mask = np.asarray(rope_mask).astype(bool)

    # J^T for the pair-rotation matmul: (J q)[2i] = -q[2i+1], (J q)[2i+1] = q[2i]
    jT = np.zeros((P, P), dtype=np.float32)
    for i in range(P // 2):
        jT[2 * i, 2 * i + 1] = 1.0
        jT[2 * i + 1, 2 * i] = -1.0

    masks = np.zeros((BANDS, P, TQ), dtype=BF)
    ii = np.arange(P)[:, None]
    jj = np.arange(TQ)[None, :]
    for a in range(BANDS):
        masks[a] = (ii + a * P <= jj).astype(BF)

    C_full = np.repeat(cos[:T].T, 2, axis=0).astype(np.float32)  # [128, T]
    S_full = np.repeat(sin[:T].T, 2, axis=0).astype(np.float32)

    # per-batch x packs (shared by the 4 cores of each batch)
    xs_b = []
    for b in range(B):
        x8, xr8 = _q8(x[b].T, SX)  # [D, T] fp8
        xsp = np.empty((P, 2 * KO, T), dtype=x8.dtype)
        xsp[:, 0::2] = _pack_k(x8)
        xsp[:, 1::2] = _pack_k(xr8)
        xs_b.append(xsp)

    in_maps = []
    for c in range(N_CORES):
        b = c // CORES_PER_B
        hg = c % CORES_PER_B
        heads = [hg * HPC + i for i in range(HPC)]

        qrows = np.concatenate([np.arange(h * HD, (h + 1) * HD) for h in heads])
        krows = qrows + D
        vrows = qrows + 2 * D
        wqk = Wqkv[np.concatenate([qrows, krows])].T  # [D, 1024]
        wv = Wqkv[vrows].T                            # [D, 512]
        wqk8, wqkr8 = _q8(wqk, SW)
        wv8, wvr8 = _q8(wv, SW)

        def pack_q(a):  # [P, KO, 1024] -> [P, NQK, KO, 128] m-major
            pk = _pack_k(a)
            return np.ascontiguousarray(
                pk.reshape(P, KO, NQK, P).transpose(0, 2, 1, 3)
            )

        woT = np.ascontiguousarray(Wout[:, qrows].T)  # [512, D]
        wo8, wor8 = _q8(woT, SWO)
        wos_p = np.ascontiguousarray(wo8.reshape(HPC, P, D).transpose(1, 0, 2))
        wors_p = np.ascontiguousarray(wor8.reshape(HPC, P, D).transpose(1, 0, 2))

        flags = [bool(rope_mask[h]) for h in heads]
        assert all(f == flags[0] for f in flags), (
            "heads in one core must share a rope flag for the single-table path"
        )
        cs_arr = np.empty((P, 2, T), dtype=BF)
        if flags[0]:
            cs_arr[:, 0] = C_full.astype(BF)
            cs_arr[:, 1] = S_full.astype(BF)
        else:
            cs_arr[:, 0] = np.ones((P, T), dtype=BF)
            cs_arr[:, 1] = np.zeros((P, T), dtype=BF)

        in_maps.append(
            {
                "xs": xs_b[b],
                "wqks": pack_q(wqk8),
                "wqkrs": pack_q(wqkr8),
                "wvs": _pack_k(wv8),
                "wvrs": _pack_k(wvr8),
                "wos": wos_p,
                "wors": wors_p,
                "cs": cs_arr,
                "masks": masks,
                "jT": jT.astype(BF),
                "ones": np.full((P, P), SIGMA / SO, dtype=BF),
            }
        )
    return in_maps


def kernel(x, Wqkv, Wout, cos, sin, rope_mask, _trace=False):
    nc = _get_program()
    in_maps = _host_inputs(x, Wqkv, Wout, cos, sin, rope_mask)
    res = run_bass_kernel_spmd(nc, in_maps, core_ids=list(range(N_CORES)), trace=_trace)
    parts = [np.asarray(res.results[c]["y"], dtype=np.float32) for c in range(N_CORES)]
    out = np.stack(
        [sum(parts[b * CORES_PER_B : (b + 1) * CORES_PER_B]) for b in range(B)]
    ).astype(np.float32)
    if _trace:
        kernel.last_result = res
    return out


# revision 40
# speedup vs baseline: 1.3425x; 1.0037x over previous
"""Causal self-attention (B=2, T=2048, D=2048, H=16, hd=128, RoPE on masked
heads) as a Bass/Tile kernel on 8 Trainium2 NeuronCores.

Sharding: core c handles batch b=c//4 and heads 4*(c%4)..4*(c%4)+3 (data
parallel on B x tensor parallel on H).  Each core computes a partial output
projection y_b = O_local @ Wout_local^T; the host sums the 4 partials per
batch.

Numerics/performance strategy:
- QKV projection runs as fp8(e4m3) DoubleRow matmuls with 3-term residual
  compensation: x*W ~ x8*W8 + xr8*W8 + x8*Wr8, where xr8/Wr8 are e4m3
  quantizations of the quantization residuals (host-prepared).  Each
  DoubleRow instruction contracts two 128-row K-slabs at half cost, so the
  projection runs at 1.5x the bf16 matmul rate with ~1e-3 relative error.
  The 3 terms are packed into 24 DoubleRow instructions per output tile via
  a chain pairing that needs no operand duplication (see _emit_3term).
- Attention (scores, softmax, PV, denominator) runs in bf16: S^T = K Q^T in
  transposed score space so softmax normalization is a per-free-element
  multiply; denominator via a constant-value stationary matmul whose value
  folds the fp8 descale so normalized outputs land in e4m3 range.  Score
  tiles are computed in PAIRS sharing one PSUM bank (only the first matmul
  carries start; region-granular zeroing verified on hw) so one exp covers
  two k-blocks, keeping the Activation engine off the critical path.
- The output projection also runs as 3-term fp8 DoubleRow: the normalize
  step emits an e4m3 hi part plus an exact residual lo part (one extra DVE
  subtract per tile), contracted against host-prepared Wout hi/lo slabs.
- q, k, v stay resident in SBUF between phases (bf16) - no DRAM scratch;
  y is written as bf16 partials and summed in f32 on the host.
- RoPE tables are a single per-core C/S pair (identity for NoPE cores);
  roped = C*q + S*(J q) with J applied as a PE matmul; head-0 rope is
  folded into phase 1 and phase-3 tiles are interleaved into the last
  head's attention so the PE pipeline never drains between phases.
"""

import sys

sys.path.insert(0, "/opt/trn_rl_repo")

import numpy as np

import concourse.bass as bass
import concourse.mybir as mybir
import concourse.tile as tile
from concourse.bass_utils import run_bass_kernel_spmd

F32 = mybir.dt.float32
F8 = mybir.dt.float8e4
BF16 = mybir.dt.bfloat16
DR = mybir.MatmulPerfMode.DoubleRow

B = 2
T = 2048
D = 2048
H = 16
HD = 128
N_CORES = 8
HPC = 4           # heads per core
CORES_PER_B = 4
P = 128
TB = 512          # t-block width (phase 1 / rope)
NTB = T // TB     # 4
TQ = 256          # attention q-tile width (phase 2)
NTQ = T // TQ     # 8
BANDS = TQ // P   # 2
KO = D // P       # 16 contraction K-blocks of 128
NQK = 2 * HPC     # 8 q+k dout blocks of 128
SX = 16.0         # fp8 scale for x
SW = 1024.0       # fp8 scale for Wqkv
SWO = 1024.0      # fp8 scale for Wout
SO = 32.0         # fp8 scale carried by the normalized attention output
SIGMA = SX * SW   # scale carried by q,k,v in SBUF
SCALE_EFF = (1.0 / float(np.sqrt(HD))) / (SIGMA * SIGMA)


# ---------------------------------------------------------------------------
# Walrus on this toolchain rejects instructions carrying more than one sync
# wait command; Tile can emit several (e.g. the kernel-tail drain).  Hoist
# the excess onto injected same-engine NoOps — semantically identical.
def _fix_waits(nc, cap=1):
    ctr = 0
    for f in nc.m.functions:
        for bb in f.blocks:
            insts = bb.instructions
            i = 0
            while i < len(insts):
                inst = insts[i]
                si = inst.sync_info
                if si is not None and si.on_wait and len(si.on_wait) > cap:
                    waits = list(si.on_wait)
                    keep, excess = waits[:cap], waits[cap:]
                    nops = []
                    for j in range(0, len(excess), cap):
                        ctr += 1
                        nops.append(
                            mybir.InstNoOp(
                                name=f"I-waitfix-{ctr}",
                                engine=inst.engine,
                                sync_info=mybir.SyncInfo(
                                    on_wait=excess[j : j + cap], on_update=[]
                                ),
                            )
                        )
                    inst.sync_info = mybir.SyncInfo(
                        on_wait=keep, on_update=list(si.on_update or [])
                    )
                    insts[i:i] = nops
                    i += len(nops)
                i += 1
    return ctr


def _emit_3term(nc, ps, w_sb, wr_sb, xs_t, msl, tsl, w_of_pair, x_of_pair):
    """Emit the 24 DoubleRow matmuls of one 3-term-compensated K=2048
    contraction into PSUM tile `ps`.

    xs_t holds 32 K-slabs (2i = x8_i, 2i+1 = xr8_i); w_sb/wr_sb hold 16
    slabs each (W8_i / Wr8_i).  Chain pairing covers x8_i*W8_i, xr8_i*W8_i
    (A instructions) and x8_i*Wr8_i (B instructions) with constant-stride
    slab pairs only.  `w_of_pair(w_tile, s0, s1, msl)` / `x_of_pair(xs, s0,
    s1, tsl)` build the [128, 2, *] APs (orientation differs between the
    q/k and v sweeps).
    """
    seq = []
    # A_1..A_15: x slabs (2j-1, 2j), w slabs (j-1, j)
    for j in range(1, KO):
        seq.append((w_of_pair(w_sb, j - 1, j, msl), x_of_pair(xs_t, 2 * j - 1, 2 * j, tsl)))
    # B_0..B_7: x slabs (4m, 4m+2), wr slabs (2m, 2m+1)
    for m in range(KO // 2):
        seq.append((w_of_pair(wr_sb, 2 * m, 2 * m + 1, msl), x_of_pair(xs_t, 4 * m, 4 * m + 2, tsl)))
    # A_0: x slabs (0, 31), w slabs (0, 15)
    seq.append((w_of_pair(w_sb, 0, KO - 1, msl), x_of_pair(xs_t, 0, 2 * KO - 1, tsl)))
    n = len(seq)
    for i, (w_ap, x_ap) in enumerate(seq):
        nc.tensor.matmul(ps[:], w_ap, x_ap, start=(i == 0), stop=(i == n - 1), perf_mode=DR)


def _slab_pair(t, s0, s1, csl):
    """AP [128, 2, cols] selecting slabs s0 < s1 of a [P, nslab, C] tile."""
    if csl is None:
        return t[:, s0 : s1 + 1 : (s1 - s0), :] if s1 - s0 > 1 else t[:, s0 : s1 + 1, :]
    step = s1 - s0
    if step > 1:
        return t[:, s0 : s1 + 1 : step, csl]
    return t[:, s0 : s1 + 1, csl]


def _rope_block(nc, psum_pool, tmp_pool, qk_sb, cs_sb, jT_sb, h, qr, kr, rb, tag="psj", psj_bufs=2):
    """RoPE for one 512-wide t-block of head h: roped = C*q + S*(J q).
    qr/kr are per-t-block tile lists so consumers only depend on their own
    block's blend, not the whole head."""
    sl = slice(rb * TB, (rb + 1) * TB)
    for si, (src_t, dst) in enumerate(((qk_sb[h], qr[rb]), (qk_sb[HPC + h], kr[rb]))):
        psj = psum_pool.tile([P, TB], F32, tag=tag, name=f"psj{h}_{rb}_{si}", bufs=psj_bufs)
        nc.tensor.matmul(psj[:], jT_sb[:], src_t[:, sl], start=True, stop=True)
        tmp = tmp_pool.tile([P, TB], BF16, tag="ropetmp", name=f"rtmp{h}_{rb}_{si}")
        nc.vector.tensor_tensor(tmp[:], psj[:], cs_sb[:, 1, sl], mybir.AluOpType.mult)
        nc.vector.tensor_tensor(dst[:], src_t[:, sl], cs_sb[:, 0, sl], mybir.AluOpType.mult)
        nc.vector.tensor_tensor(dst[:], dst[:], tmp[:], mybir.AluOpType.add)


def _phase1(nc, tc, xs, wqks, wqkrs, wvs, wvrs, qk_sb, v_sb, rope0, const_dmas):
    with (
        tc.tile_pool(name="p1w", bufs=1) as p1w,
        tc.tile_pool(name="p1x", bufs=2) as p1x,
        tc.tile_pool(name="p1t", bufs=2) as p1t,
        tc.tile_pool(name="p1p", bufs=3, space="PSUM") as p1p,
        tc.tile_pool(name="p1pj", bufs=1, space="PSUM") as p1pj,
    ):
        wqk_t = p1w.tile([P, NQK, KO, P], F8, name="wqks")
        wqkr_t = p1w.tile([P, NQK, KO, P], F8, name="wqkrs")
        wv_t = p1w.tile([P, KO, HPC * HD], F8, name="wvs")
        wvr_t = p1w.tile([P, KO, HPC * HD], F8, name="wvrs")

        # q/k sweep: stationary = weight slab pair, moving = x slab pair
        def w_qk(t, s0, s1, msl):
            return _slab_pair(t, s0, s1, msl)

        def x_qk(t, s0, s1, _):
            return _slab_pair(t, s0, s1, None)

        first = True
        for tb in range(NTB):
            tsl = slice(tb * TB, (tb + 1) * TB)
            xs_t = p1x.tile([P, 2 * KO, TB], F8, tag="xs", name=f"xs{tb}")
            if first:
                # fine-grained first loads so the first m-block's A chain can
                # start after ~1MB instead of ~4MB of DMA
                nc.sync.dma_start(xs_t[:, 0 : KO // 4, :], xs[:, 0 : KO // 4, tsl])
                nc.sync.dma_start(wqk_t[:, 0], wqks[:, 0])
                nc.sync.dma_start(xs_t[:, KO // 4 : KO // 2, :], xs[:, KO // 4 : KO // 2, tsl])
                nc.sync.dma_start(wqk_t[:, 1], wqks[:, 1])
                nc.sync.dma_start(xs_t[:, KO // 2 : KO, :], xs[:, KO // 2 : KO, tsl])
                nc.sync.dma_start(xs_t[:, KO : 2 * KO, :], xs[:, KO : 2 * KO, tsl])
                nc.sync.dma_start(wqkr_t[:, 0:2], wqkrs[:, 0:2])
                nc.sync.dma_start(wqk_t[:, 2:4], wqks[:, 2:4])
                nc.sync.dma_start(wqkr_t[:, 2:4], wqkrs[:, 2:4])
                nc.sync.dma_start(wqk_t[:, 4:8], wqks[:, 4:8])
                nc.sync.dma_start(wqkr_t[:, 4:8], wqkrs[:, 4:8])
                nc.sync.dma_start(wv_t[:], wvs[:])
                nc.sync.dma_start(wvr_t[:], wvrs[:])
                # const loads ride behind the critical phase-1 loads
                for dma in const_dmas:
                    dma()
                first = False
            else:
                nc.sync.dma_start(xs_t[:, 0:KO, :], xs[:, 0:KO, tsl])
                nc.sync.dma_start(xs_t[:, KO : 2 * KO, :], xs[:, KO : 2 * KO, tsl])

            for m in range(NQK):
                def w_qk_m(t, s0, s1, _msl, _m=m):
                    step = s1 - s0
                    if step > 1:
                        return t[:, _m, s0 : s1 + 1 : step, :]
                    return t[:, _m, s0 : s1 + 1, :]

                ps = p1p.tile([P, TB], F32, tag="ps1", name=f"psqk{tb}_{m}")
                _emit_3term(nc, ps, wqk_t, wqkr_t, xs_t, None, None, w_qk_m, x_qk)
                cp = (nc.vector.tensor_copy, nc.scalar.copy)[m % 2]
                cp(qk_sb[m][:, tsl], ps[:])
            qr0, kr0, cs_sb, jT_sb = rope0
            _rope_block(nc, p1pj, p1t, qk_sb, cs_sb, jT_sb, 0, qr0, kr0, tb,
                        tag="psj1", psj_bufs=1)
            for t4 in range(4):
                t4sl = slice(t4 * P, (t4 + 1) * P)
                ps = p1p.tile([P, HPC * HD], F32, tag="ps1", name=f"psv{tb}_{t4}")
                # v: out[t, hd] — stationary x slabs sliced to t4, moving wv
                seq = []
                for j in range(1, KO):
                    seq.append((_slab_pair(xs_t, 2 * j - 1, 2 * j, t4sl), _slab_pair(wv_t, j - 1, j, None)))
                for m2 in range(KO // 2):
                    seq.append((_slab_pair(xs_t, 4 * m2, 4 * m2 + 2, t4sl), _slab_pair(wvr_t, 2 * m2, 2 * m2 + 1, None)))
                seq.append((_slab_pair(xs_t, 0, 2 * KO - 1, t4sl), _slab_pair(wv_t, 0, KO - 1, None)))
                for i, (x_ap, w_ap) in enumerate(seq):
                    nc.tensor.matmul(ps[:], x_ap, w_ap, start=(i == 0), stop=(i == len(seq) - 1), perf_mode=DR)
                cp = (nc.vector.tensor_copy, nc.scalar.copy)[t4 % 2]
                cp(v_sb[tb * 4 + t4][:], ps[:])


def _phase2(nc, tc, outS, qk_sb, v_sb, jT_sb, mask_sb, ones_sb, cs_sb, r0, emit_p3, aux_pool, p2ps):
    with (
        tc.tile_pool(name="p2r", bufs=2) as p2r,
        tc.tile_pool(name="p2pt", bufs=10) as p2pt,
        tc.tile_pool(name="p2rec", bufs=4) as p2rec,
        tc.tile_pool(name="p2po", bufs=2, space="PSUM") as p2po,
    ):
        def alloc_roped(h):
            qr = [p2r.tile([P, TB], BF16, tag=f"qr{rb}", name=f"qr{h}_{rb}") for rb in range(NTB)]
            kr = [p2r.tile([P, TB], BF16, tag=f"kr{rb}", name=f"kr{h}_{rb}") for rb in range(NTB)]
            return qr, kr

        def attn_tq(h, tq, qr, kr, pending):
            """One q-tile of attention, software-pipelined over PAIRS of
            128-wide k-blocks: the two STs of a pair land in two PSUM banks
            of one tile so a single exp (and, on the diagonal, a single mask
            multiply) covers both.  PV/ones matmuls trail via `pending`."""
            sl = slice(tq * TQ, (tq + 1) * TQ)
            nk = (tq + 1) * BANDS
            # ps_o (PV) and ps_d (denominator) share one 2KB bank: the first
            # PV's start zeroes the whole region, so the denominator chain
            # never carries start (verified region-zero semantics on hw).
            ps_od = p2po.tile([P, 2, TQ], F32, tag="po", name=f"po{h}{tq}")
            ps_o = ps_od[:, 0, :]
            ps_d = ps_od[:, 1, :]

            def issue_pair(kp):
                # both STs of a pair share one 2KB bank: the first carries
                # start (zeroing the region), the second relies on the
                # region-granular pending-zero (verified on hw)
                ps_st = p2ps.tile([P, 2, TQ], F32, tag="st", name=f"st{h}{tq}{kp}")
                qr_t = qr[tq * TQ // TB]
                qsl = slice((tq * TQ) % TB, (tq * TQ) % TB + TQ)
                for j in range(2):
                    kb = 2 * kp + j
                    kr_t = kr[kb * P // TB]
                    ksl = slice((kb * P) % TB, (kb * P) % TB + P)
                    nc.tensor.matmul(
                        ps_st[:, j, :], kr_t[:, ksl], qr_t[:, qsl],
                        start=(j == 0), stop=(j == 1), skip_group_check=True,
                    )
                pt = p2pt.tile([P, 2, TQ], BF16, tag="pt", name=f"pt{h}{tq}{kp}")
                nc.scalar.activation(
                    pt[:], ps_st[:], mybir.ActivationFunctionType.Exp, scale=SCALE_EFF
                )
                if kp == tq:  # diagonal pair: mask both bands at once
                    nc.vector.tensor_tensor(pt[:], pt[:], mask_sb[:], mybir.AluOpType.mult)
                return pt

            def make_pv(kp, pt):
                def pv():
                    for j in range(2):
                        kb = 2 * kp + j
                        nc.tensor.matmul(
                            ps_o, v_sb[kb][:, h * HD : (h + 1) * HD], pt[:, j, :],
                            start=(kb == 0), stop=False, skip_group_check=True,
                        )
                        nc.tensor.matmul(
                            ps_d, ones_sb[:], pt[:, j, :], start=False,
                            stop=(kb == nk - 1), skip_group_check=True,
                        )
                    if 2 * kp + 1 == nk - 1:
                        rec = p2rec.tile([P, TQ], F32, tag="rec", name=f"rec{h}{tq}")
                        nc.vector.reciprocal(rec[:], ps_d)
                        ob = p2rec.tile([P, TQ], BF16, tag="ob", name=f"ob{h}{tq}")
                        nc.vector.tensor_tensor(ob[:], ps_o, rec[:], mybir.AluOpType.mult)
                        hi = outS[tq][:, 2 * h, :]
                        nc.vector.tensor_copy(hi, ob[:])
                        nc.vector.scalar_tensor_tensor(
                            outS[tq][:, 2 * h + 1, :], ob[:], 1.0, hi,
                            mybir.AluOpType.mult, mybir.AluOpType.subtract,
                        )
                return pv

            for kp in range(nk // 2):
                pt = issue_pair(kp)
                if len(pending) >= 3:
                    pending.pop(0)()
                pending.append(make_pv(kp, pt))

        # rope for head h+1 is interleaved into head h's attention (one
        # 512-wide t-block per pair of q-tiles); head 0 was roped inside
        # phase 1.  During the last head, phase-3 tiles are emitted one
        # q-tile behind so output projection overlaps the attention tail.
        roped = [r0]
        pending = []
        for h in range(HPC):
            if h + 1 < HPC:
                roped.append(alloc_roped(h + 1))
            qr, kr = roped[h]
            for tq in range(NTQ):
                attn_tq(h, tq, qr, kr, pending)
                if h + 1 < HPC:
                    if tq % 2 == 0:
                        _rope_block(nc, aux_pool, p2pt, qk_sb, cs_sb, jT_sb,
                                    h + 1, roped[h + 1][0], roped[h + 1][1], tq // 2,
                                    tag="ps3", psj_bufs=3)
                elif tq >= 2:
                    # two q-tiles behind: head-3's normalization for tq-2 is
                    # guaranteed emitted (pending is only 3 pairs deep)
                    emit_p3(tq - 2)
            if h == HPC - 1:
                while pending:
                    pending.pop(0)()
        emit_p3(NTQ - 2)
        emit_p3(NTQ - 1)


def _make_p3(nc, p3s, p3p, outS, wos_sb, wors_sb, y):
    ydescale = 1.0 / (SO * SWO)

    def emit_p3(tq):
        for tt in range(tq * BANDS, (tq + 1) * BANDS):
            off = (tt - tq * BANDS) * P
            osl = slice(off, off + P)
            ysb = p3s.tile([P, D], BF16, tag="ysb", name=f"ysb{tt}")
            last = tq == NTQ - 1
            for dd in range(D // TB):
                dsl = slice(dd * TB, (dd + 1) * TB)
                ps = p3p.tile([P, TB], F32, tag="ps3", name=f"ps3{tt}{dd}")
                seq = []
                for j in range(1, HPC):
                    seq.append((outS[tq][:, 2 * j - 1 : 2 * j + 1, osl], wos_sb[:, j - 1 : j + 1, dsl]))
                for m in range(HPC // 2):
                    seq.append((outS[tq][:, 4 * m : 4 * m + 3 : 2, osl], wors_sb[:, 2 * m : 2 * m + 2, dsl]))
                seq.append((outS[tq][:, 0 : 2 * HPC : 2 * HPC - 1, osl], wos_sb[:, 0 : HPC : HPC - 1, dsl]))
                for i, (o_ap, w_ap) in enumerate(seq):
                    nc.tensor.matmul(ps[:], o_ap, w_ap, start=(i == 0), stop=(i == len(seq) - 1), perf_mode=DR)
                if dd % 2 == 0:
                    nc.vector.tensor_scalar_mul(ysb[:, dsl], ps[:], ydescale)
                else:
                    nc.scalar.mul(ysb[:, dsl], ps[:], ydescale)
                if last:
                    # small per-dd stores shrink the end-of-kernel DMA tail
                    nc.sync.dma_start(y[tt * P : (tt + 1) * P, dsl], ysb[:, dsl])
            if not last:
                nc.sync.dma_start(y[tt * P : (tt + 1) * P, :], ysb[:])
    return emit_p3


def _build_program():
    nc = bass.Bass()

    xs = nc.dram_tensor("xs", (P, 2 * KO, T), F8, kind="ExternalInput")
    wqks = nc.dram_tensor("wqks", (P, NQK, KO, P), F8, kind="ExternalInput")
    wqkrs = nc.dram_tensor("wqkrs", (P, NQK, KO, P), F8, kind="ExternalInput")
    wvs = nc.dram_tensor("wvs", (P, KO, HPC * HD), F8, kind="ExternalInput")
    wvrs = nc.dram_tensor("wvrs", (P, KO, HPC * HD), F8, kind="ExternalInput")
    wos = nc.dram_tensor("wos", (P, HPC, D), F8, kind="ExternalInput")
    wors = nc.dram_tensor("wors", (P, HPC, D), F8, kind="ExternalInput")
    cs = nc.dram_tensor("cs", (P, 2, T), BF16, kind="ExternalInput")
    masks = nc.dram_tensor("masks", (BANDS, P, TQ), BF16, kind="ExternalInput")
    jT = nc.dram_tensor("jT", (P, P), BF16, kind="ExternalInput")
    ones = nc.dram_tensor("ones", (P, P), BF16, kind="ExternalInput")
    y = nc.dram_tensor("y", (T, D), BF16, kind="ExternalOutput")

    with tile.TileContext(nc) as tc:
        with (
            tc.tile_pool(name="consts", bufs=1) as consts,
            tc.tile_pool(name="qkv", bufs=1) as qkvp,
            tc.tile_pool(name="p2ps", bufs=3, space="PSUM") as p2ps,
        ):
            jT_sb = consts.tile([P, P], BF16)
            mask_sb = consts.tile([P, BANDS, TQ], BF16)
            ones_sb = consts.tile([P, P], BF16)
            cs_sb = consts.tile([P, 2, T], BF16)
            wos_sb = consts.tile([P, HPC, D], F8)
            wors_sb = consts.tile([P, HPC, D], F8)
            const_dmas = [
                lambda: nc.sync.dma_start(cs_sb[:], cs[:]),
                lambda: nc.sync.dma_start(jT_sb[:], jT[:]),
                lambda: nc.sync.dma_start(ones_sb[:], ones[:]),
                lambda: nc.sync.dma_start(mask_sb[:], masks.rearrange("a p j -> p a j")),
                lambda: nc.sync.dma_start(wos_sb[:], wos[:]),
                lambda: nc.sync.dma_start(wors_sb[:], wors[:]),
            ]

            qk_sb = [qkvp.tile([P, T], BF16, name=f"qk{m}") for m in range(NQK)]
            v_sb = [qkvp.tile([P, HPC * HD], BF16, name=f"v{kb}") for kb in range(T // P)]
            qr0 = [qkvp.tile([P, TB], BF16, name=f"qr0_{rb}") for rb in range(NTB)]
            kr0 = [qkvp.tile([P, TB], BF16, name=f"kr0_{rb}") for rb in range(NTB)]

            _phase1(nc, tc, xs, wqks, wqkrs, wvs, wvrs, qk_sb, v_sb,
                    (qr0, kr0, cs_sb, jT_sb), const_dmas)

            with (
                tc.tile_pool(name="outT", bufs=1) as outT_pool,
                tc.tile_pool(name="p3s", bufs=3) as p3s,
                tc.tile_pool(name="p3p", bufs=3, space="PSUM") as p3p,
            ):
                outS = {
                    tq: outT_pool.tile([P, 2 * HPC, TQ], F8, tag=f"outS{tq}", name=f"outS{tq}")
                    for tq in range(NTQ)
                }
                emit_p3 = _make_p3(nc, p3s, p3p, outS, wos_sb, wors_sb, y)
                _phase2(nc, tc, outS, qk_sb, v_sb, jT_sb, mask_sb, ones_sb, cs_sb,
                        (qr0, kr0), emit_p3, p3p, p2ps)

    _fix_waits(nc)
    return nc


_NC_CACHE = None


def _get_program():
    global _NC_CACHE
    if _NC_CACHE is None:
        _NC_CACHE = _build_program()
    return _NC_CACHE


def _q8(a, s):
    """e4m3-quantize a*s (clipped to TRN e4m3 range); returns (fp8, residual
    fp8) with the residual on the same scale (no prescale — its values live
    in e4m3's normal range already)."""
    import ml_dtypes

    F8np = ml_dtypes.float8_e4m3
    scaled = np.clip(a * s, -240.0, 240.0)
    hi = scaled.astype(F8np)
    lo = np.clip(scaled - hi.astype(np.float32), -240.0, 240.0).astype(F8np)
    return hi, lo


def _pack_k(a):
    """[K, M] -> [P, KO', M] with slab i on partitions (rows 128i+p)."""
    ko = a.shape[0] // P
    return np.ascontiguousarray(a.reshape(ko, P, a.shape[1]).transpose(1, 0, 2))


def _host_inputs(x, Wqkv, Wout, cos, sin, rope_mask):
    import ml_dtypes

    BF = ml_dtypes.bfloat16
    x = np.asarray(x, dtype=np.float32)
    Wqkv = np.asarray(Wqkv, dtype=np.float32)
    Wout = np.asarray(Wout, dtype=np.float32)
    cos = np.asarray(cos, dtype=np.float32)
    sin = np.asarray(sin, dtype=np.float32)
    rope_mask = np.asarray(rope_mask).astype(bool)

    # J^T for the pair-rotation matmul: (J q)[2i] = -q[2i+1], (J q)[2i+1] = q[2i]
    jT = np.zeros((P, P), dtype=np.float32)
    for i in range(P // 2):
        jT[2 * i, 2 * i + 1] = 1.0
        jT[2 * i + 1, 2 * i] = -1.0

    masks = np.zeros((BANDS, P, TQ), dtype=BF)
    ii = np.arange(P)[:, None]
    jj = np.arange(TQ)[None, :]
    for a in range(BANDS):
        masks[a] = (ii + a * P <= jj).astype(BF)

    C_full = np.repeat(cos[:T].T, 2, axis=0).astype(np.float32)  # [128, T]
    S_full = np.repeat(sin[:T].T, 2, axis=0).astype(np.float32)

    # per-batch x packs (shared by the 4 cores of each batch)
    xs_b = []
    for b in range(B):
        x8, xr8 = _q8(x[b].T, SX)  # [D, T] fp8
        xsp = np.empty((P, 2 * KO, T), dtype=x8.dtype)
        xsp[:, 0::2] = _pack_k(x8)
        xsp[:, 1::2] = _pack_k(xr8)
        xs_b.append(xsp)

    in_maps = []
    for c in range(N_CORES):
        b = c // CORES_PER_B
        hg = c % CORES_PER_B
        heads = [hg * HPC + i for i in range(HPC)]

        qrows = np.concatenate([np.arange(h * HD, (h + 1) * HD) for h in heads])
        krows = qrows + D
        vrows = qrows + 2 * D
        wqk = Wqkv[np.concatenate([qrows, krows])].T  # [D, 1024]
        wv = Wqkv[vrows].T                            # [D, 512]
        wqk8, wqkr8 = _q8(wqk, SW)
        wv8, wvr8 = _q8(wv, SW)

        def pack_q(a):  # [P, KO, 1024] -> [P, NQK, KO, 128] m-major
            pk = _pack_k(a)
            return np.ascontiguousarray(
                pk.reshape(P, KO, NQK, P).transpose(0, 2, 1, 3)
            )

        woT = np.ascontiguousarray(Wout[:, qrows].T)  # [512, D]
        wo8, wor8 = _q8(woT, SWO)
        wos_p = np.ascontiguousarray(wo8.reshape(HPC, P, D).transpose(1, 0, 2))
        wors_p = np.ascontiguousarray(wor8.reshape(HPC, P, D).transpose(1, 0, 2))

        flags = [bool(rope_mask[h]) for h in heads]
        assert all(f == flags[0] for f in flags), (
            "heads in one core must share a rope flag for the single-table path"
        )
        cs_arr = np.empty((P, 2, T), dtype=BF)
        if flags[0]:
            cs_arr[:, 0] = C_full.astype(BF)
            cs_arr[:, 1] = S_full.astype(BF)
        else:
            cs_arr[:, 0] = np.ones((P, T), dtype=BF)
            cs_arr[:, 1] = np.zeros((P, T), dtype=BF)

        in_maps.append(
            {
                "xs": xs_b[b],
                "wqks": pack_q(wqk8),
                "wqkrs": pack_q(wqkr8),
                "wvs": _pack_k(wv8),
                "wvrs": _pack_k(wvr8),
                "wos": wos_p,
                "wors": wors_p,
                "cs": cs_arr,
                "masks": masks,
                "jT": jT.astype(BF),
                "ones": np.full((P, P), SIGMA / SO, dtype=BF),
            }
        )
    return in_maps


def kernel(x, Wqkv, Wout, cos, sin, rope_mask, _trace=False):
    nc = _get_program()
    in_maps = _host_inputs(x, Wqkv, Wout, cos, sin, rope_mask)
    res = run_bass_kernel_spmd(nc, in_maps, core_ids=list(range(N_CORES)), trace=_trace)
    parts = [np.asarray(res.results[c]["y"], dtype=np.float32) for c in range(N_CORES)]
    out = np.stack(
        [sum(parts[b * CORES_PER_B : (b + 1) * CORES_PER_B]) for b in range(B)]
    ).astype(np.float32)
    if _trace:
        kernel.last_result = res
    return out
